# revision 1
# baseline (speedup 1.0000x reference)
"""Trainium2 Bass kernel for nn_LinkerEncoder (3-layer GCN + mean-pool +
MLP + LayerNorm), SPMD over 8 NeuronCores.

v2 changes vs baseline:
- x pre-scaled by dinv on host (row scaling commutes with @W) -> table build
  is a pure matmul.
- Table builds batched: 8 chunks share one PSUM bank and one 256KB DMA store.
- Next-layer transform fused into the gather sink: L1 produces hW2own (fp16)
  and L2 produces hW3own (fp32) per block, so the standalone table-build
  phases (and their 800 tiny DMAs) disappear.
- AllGather ships the transformed tables (hW2own fp16 / hW3own fp32).
- Gathers batched into superblocks (up to 64 columns = 8192 idx per
  dma_gather call) to amortize the ~1us SWDGE fixed cost.
- Pooling tail: ReduceScatter -> per-core 125-graph MLP -> host concatenates
  the 8 output slices.
"""
import numpy as np

# ==== prep (identical stream construction to baseline) ====

N = 50000
NC = 8
SHARD = N // NC          # 6250
NBLK = (SHARD + 127) // 128   # 49
BASE_B = 17234           # B gather window starts here (idx 32767 -> row 50001)
B_PAD_IDX = N + 1 - BASE_B    # 32767 -> guard row
# source classes by core: cores 0-2 fixed-A (pos < 18750 <= 32766), cores 3-4
# flexible (either window), cores 5-7 fixed-B (pos >= 31250 >= BASE_B)
CLS_OF_CORE = np.array([0, 0, 0, 1, 1, 2, 2, 2], np.int32)
G = 1000
GBLK = 8                  # graph blocks
GPER = G // GBLK          # 125 graphs per block


def wrap16(stream):
    """stream [L] (L % 16 == 0) -> wrapped [16, L//16] -> replicated [128, L//16] int16."""
    L = len(stream)
    w = np.asarray(stream, np.int32).reshape(L // 16, 16).T.astype(np.int16)
    return np.tile(w, (8, 1))


def preprocess(edge_index, batch):
    src = np.asarray(edge_index[0], np.int64)
    dst = np.asarray(edge_index[1], np.int64)
    batch = np.asarray(batch, np.int64)

    indeg = np.bincount(dst, minlength=N).astype(np.int64) + 1  # + self loop
    dinv = 1.0 / np.sqrt(indeg.astype(np.float64))

    # ---- core assignment: round-robin by in-degree rank
    rank = np.argsort(-indeg, kind="stable")      # node ids, desc degree
    core_of = np.empty(N, np.int32)
    core_of[rank] = np.arange(N, dtype=np.int32) % NC

    # ---- per-dest stream counts.
    # L2/L3 use O/A/B: O = own-core sources incl the self loop (gathered from
    # the local per-core table, overlappable with the preceding collective);
    # A/B = remote sources balanced between the two int16 windows using the
    # flexible middle region (cores 3-4).
    # L1 has no collective to hide O work behind, so it uses flat A1/B1
    # streams (all sources incl self, balanced the same way).
    own = core_of[src] == core_of[dst]
    cls = CLS_OF_CORE[core_of[src]]
    o_cnt = np.bincount(dst[own], minlength=N).astype(np.int64) + 1  # + self
    rem = ~own

    def balance(sel_dst, sel_cls, extra_a=0, extra_l=0, extra_b=0):
        fa = np.bincount(sel_dst, weights=(sel_cls == 0).astype(np.float64),
                         minlength=N).astype(np.int64) + extra_a
        fl = np.bincount(sel_dst, weights=(sel_cls == 1).astype(np.float64),
                         minlength=N).astype(np.int64) + extra_l
        fb = np.bincount(sel_dst, weights=(sel_cls == 2).astype(np.float64),
                         minlength=N).astype(np.int64) + extra_b
        x = np.clip((fb - fa + fl + 1) // 2, 0, fl)
        return fa + x, fb + fl - x, x

    a_cnt, b_cnt, xflex = balance(dst[rem], cls[rem])
    selfcls = CLS_OF_CORE[core_of]
    a1_cnt, b1_cnt, xflex1 = balance(
        dst, cls, (selfcls == 0).astype(np.int64),
        (selfcls == 1).astype(np.int64), (selfcls == 2).astype(np.int64))

    # ---- within-core ordering by (max(a1,b1), max(a,b))
    pos_of = np.empty(N, np.int64)
    nodes_of_core = []
    mx23 = np.maximum(a_cnt, b_cnt)
    mx1 = np.maximum(a1_cnt, b1_cnt)
    for c in range(NC):
        nodes = np.where(core_of == c)[0]
        order = np.lexsort((mx23[nodes], mx1[nodes]))
        nodes = nodes[order]
        nodes_of_core.append(nodes)
        pos_of[nodes] = SHARD * c + np.arange(SHARD)
    node_at = np.empty(N, np.int64)
    node_at[pos_of] = np.arange(N)

    # ---- per-core per-block slot counts
    def block_max(cnt):
        S = np.zeros((NC, NBLK), np.int64)
        for c in range(NC):
            s = cnt[nodes_of_core[c]]
            for k in range(NBLK):
                lo, hi = 128 * k, min(128 * (k + 1), SHARD)
                S[c, k] = s[lo:hi].max()
        return S.max(axis=0)

    SOu = block_max(o_cnt)
    SAu = block_max(a_cnt)
    SBu = block_max(b_cnt)
    SA1u = block_max(a1_cnt)
    SB1u = block_max(b1_cnt)

    # ---- build per-core idx streams (slot-major within block)
    order = np.argsort(dst, kind="stable")
    s_sorted = src[order]
    d_sorted = dst[order]
    starts = np.searchsorted(d_sorted, np.arange(N))
    ends = np.searchsorted(d_sorted, np.arange(N) + 1)

    lenO = int(SOu.sum()) * 128
    lenA = int(SAu.sum()) * 128
    lenB = int(SBu.sum()) * 128
    lenA1 = int(SA1u.sum()) * 128
    lenB1 = int(SB1u.sum()) * 128
    streamsO = np.zeros((NC, lenO), np.int32)           # pad idx 0 (guard row)
    streamsA = np.zeros((NC, lenA), np.int32)           # pad idx 0
    streamsB = np.full((NC, lenB), B_PAD_IDX, np.int32)  # pad idx -> zero row
    streamsA1 = np.zeros((NC, lenA1), np.int32)
    streamsB1 = np.full((NC, lenB1), B_PAD_IDX, np.int32)
    for c in range(NC):
        offO = offA = offB = offA1 = offB1 = 0
        for k in range(NBLK):
            lo, hi = 128 * k, min(128 * (k + 1), SHARD)
            gridO = np.zeros((int(SOu[k]), 128), np.int32)
            gridA = np.zeros((int(SAu[k]), 128), np.int32)
            gridB = np.full((int(SBu[k]), 128), B_PAD_IDX, np.int32)
            gridA1 = np.zeros((int(SA1u[k]), 128), np.int32)
            gridB1 = np.full((int(SB1u[k]), 128), B_PAD_IDX, np.int32)
            for lane in range(hi - lo):
                d = nodes_of_core[c][lo + lane]
                srcs = s_sorted[starts[d]:ends[d]]
                ps = pos_of[srcs]
                so = core_of[srcs] == c
                selfpos = SHARD * c + 128 * k + lane
                # O/A/B (L2/L3)
                po = np.concatenate([ps[so] - SHARD * c,
                                     [128 * k + lane]]) + 1  # + self
                rps = ps[~so]
                scls = CLS_OF_CORE[core_of[srcs[~so]]]
                flex = rps[scls == 1]
                x = int(xflex[d])
                pa = np.concatenate([rps[scls == 0], flex[:x]]) + 1
                pb = np.concatenate([rps[scls == 2], flex[x:]]) + 1 - BASE_B
                gridO[: len(po), lane] = po
                gridA[: len(pa), lane] = pa
                gridB[: len(pb), lane] = pb
                # flat A1/B1 (L1): all sources + self
                aps = np.concatenate([ps, [selfpos]])
                acls = np.concatenate([CLS_OF_CORE[core_of[srcs]],
                                       [CLS_OF_CORE[c]]])
                flex1 = aps[acls == 1]
                x1 = int(xflex1[d])
                pa1 = np.concatenate([aps[acls == 0], flex1[:x1]]) + 1
                pb1 = np.concatenate([aps[acls == 2], flex1[x1:]]) + 1 - BASE_B
                gridA1[: len(pa1), lane] = pa1
                gridB1[: len(pb1), lane] = pb1
            streamsO[c, offO:offO + gridO.size] = gridO.ravel()
            streamsA[c, offA:offA + gridA.size] = gridA.ravel()
            streamsB[c, offB:offB + gridB.size] = gridB.ravel()
            streamsA1[c, offA1:offA1 + gridA1.size] = gridA1.ravel()
            streamsB1[c, offB1:offB1 + gridB1.size] = gridB1.ravel()
            offO += gridO.size
            offA += gridA.size
            offB += gridB.size
            offA1 += gridA1.size
            offB1 += gridB1.size

    # ---- pooling tables: graph -> member locals per core
    SP = np.zeros((NC, GBLK), np.int64)
    members = {}
    for c in range(NC):
        g_of_local = batch[nodes_of_core[c]]          # [SHARD] graph id per local
        for gb in range(GBLK):
            cnts = np.bincount(
                g_of_local[(g_of_local >= GPER * gb) & (g_of_local < GPER * (gb + 1))] - GPER * gb,
                minlength=GPER)
            SP[c, gb] = max(cnts.max(), 1)
        members[c] = g_of_local
    SPu = SP.max(axis=0)

    lenP = int(SPu.sum()) * 128
    streamsP = np.zeros((NC, lenP), np.int32)          # pad 0 -> zero row
    for c in range(NC):
        g_of_local = members[c]
        off = 0
        for gb in range(GBLK):
            grid = np.zeros((int(SPu[gb]), 128), np.int32)
            for gl in range(GPER):
                locs = np.where(g_of_local == GPER * gb + gl)[0] + 1  # 1-based
                grid[: len(locs), gl] = locs
            streamsP[c, off:off + grid.size] = grid.ravel()
            off += grid.size

    cnts = np.bincount(batch, minlength=G).astype(np.float64)
    inv_cnt = (1.0 / np.maximum(cnts, 1.0)).astype(np.float32)

    return dict(
        core_of=core_of, pos_of=pos_of, node_at=node_at,
        dinv=dinv.astype(np.float32),
        SOu=SOu, SAu=SAu, SBu=SBu, SA1u=SA1u, SB1u=SB1u, SPu=SPu,
        streamsO=streamsO, streamsA=streamsA, streamsB=streamsB,
        streamsA1=streamsA1, streamsB1=streamsB1, streamsP=streamsP,
        inv_cnt=inv_cnt,
    )


# ==== host IO staging ====

def make_in_maps(inputs, P):
    node_at = P["node_at"]           # position -> original node id
    dinv = P["dinv"]                 # original node order
    x = np.asarray(inputs["x"], np.float32)
    xd = x * dinv[:, None]           # pre-scale rows by dinv
    xp = xd[node_at]                 # position order
    # L1 gather table: raw pre-scaled features, zero-padded 37 -> 64 cols,
    # with guard rows 0 and N+1 (W1 is applied after aggregation).
    xdp = np.zeros((N + 2, 64), np.float32)
    xdp[1:N + 1, :37] = xp

    dinv_pos = dinv[node_at].astype(np.float32)

    def rep(v, width):
        return np.tile(np.asarray(v, np.float32)[None, :], (128, 1))

    common = dict(
        xdp=xdp,
        W1=np.asarray(inputs["W1"], np.float32),
        W2=np.asarray(inputs["W2"], np.float32),
        W3=np.asarray(inputs["W3"], np.float32),
        Wf1=np.asarray(inputs["Wf1"], np.float32),
        Wf2=np.asarray(inputs["Wf2"], np.float32),
        b1r=rep(inputs["b1"], 64), b2r=rep(inputs["b2"], 128),
        b3r=rep(inputs["b3"], 64), bf1r=rep(inputs["bf1"], 128),
        bf2r=rep(inputs["bf2"], 64), gammar=rep(inputs["gamma"], 64),
        betar=rep(inputs["beta"], 64),
    )

    in_maps = []
    for c in range(NC):
        dvo = np.zeros((128, NBLK), np.float32)
        own = dinv_pos[SHARD * c:SHARD * (c + 1)]
        for k in range(NBLK):
            rows = min(128, SHARD - 128 * k)
            dvo[:rows, k] = own[128 * k:128 * k + rows]
        invc = np.zeros((128, 1), np.float32)
        invc[:GPER, 0] = P["inv_cnt"][GPER * c:GPER * (c + 1)]
        m = dict(common)
        m["dinv_own"] = dvo
        m["invc_own"] = invc
        m["idxO"] = wrap16(P["streamsO"][c])
        m["idxA"] = wrap16(P["streamsA"][c])
        m["idxB"] = wrap16(P["streamsB"][c])
        m["idxA1"] = wrap16(P["streamsA1"][c])
        m["idxB1"] = wrap16(P["streamsB1"][c])
        m["idxP"] = wrap16(P["streamsP"][c])
        in_maps.append(m)
    return in_maps


# ==== bass kernel ====
from contextlib import ExitStack

import concourse.bass as bass
import concourse.bacc as bacc
import concourse.mybir as mybir
from concourse.tile import TileContext
from concourse.masks import make_identity

F32 = mybir.dt.float32
F16 = mybir.dt.float16
I16 = mybir.dt.int16
AX = mybir.AxisListType
AF = mybir.ActivationFunctionType
OP = mybir.AluOpType

NCHUNK = 391          # ceil(50000/128), last chunk 80 rows
D_IN, D1, D2, D3 = 37, 64, 128, 64
GW = 8                # max gather columns per dma_gather call (1024 idx =
                      # the SWDGE ucode limit; larger wedges the device)
SUPER_COLS = 64       # max A (and B) columns per superblock


def make_supers(SAu, SBu):
    """Group consecutive blocks so that sum(SAu) <= SUPER_COLS and
    sum(SBu) <= SUPER_COLS per group. Returns list of (k0, k1)."""
    supers = []
    k0 = 0
    na = nb = 0
    for k in range(NBLK):
        if k > k0 and (na + SAu[k] > SUPER_COLS or nb + SBu[k] > SUPER_COLS
                       or k - k0 >= 16):
            supers.append((k0, k))
            k0, na, nb = k, 0, 0
        na += SAu[k]
        nb += SBu[k]
    supers.append((k0, NBLK))
    # split the final block into its own super: shortens the trailing
    # gather->fold->sink drain that delays the next collective's dispatch
    if supers[-1][1] - supers[-1][0] > 1:
        k0, k1 = supers[-1]
        supers[-1] = (k0, k1 - 1)
        supers.append((k1 - 1, k1))
    return supers


def build(S, debug=False):
    """S: dict with SOu/SAu/SBu/SA1u/SB1u[49], SPu[8] (python ints)."""
    SOu, SAu, SBu, SA1u, SB1u, SPu = [
        list(map(int, S[k]))
        for k in ("SOu", "SAu", "SBu", "SA1u", "SB1u", "SPu")]
    lenO, lenA, lenB, lenP = (128 * sum(SOu), 128 * sum(SAu),
                              128 * sum(SBu), 128 * sum(SPu))
    lenA1, lenB1 = 128 * sum(SA1u), 128 * sum(SB1u)
    supers = make_supers(SAu, SBu)
    supersO = make_supers(SOu, SOu)
    supers1 = make_supers(SA1u, SB1u)

    nc = bacc.Bacc()

    # ---------------- IO ----------------
    xdp = nc.dram_tensor("xdp", [N + 2, 64], F32, kind="ExternalInput")
    dinv_own = nc.dram_tensor("dinv_own", [128, NBLK], F32, kind="ExternalInput")
    invc_own = nc.dram_tensor("invc_own", [128, 1], F32, kind="ExternalInput")
    idxO = nc.dram_tensor("idxO", [128, lenO // 16], I16, kind="ExternalInput")
    idxA = nc.dram_tensor("idxA", [128, lenA // 16], I16, kind="ExternalInput")
    idxB = nc.dram_tensor("idxB", [128, lenB // 16], I16, kind="ExternalInput")
    idxA1 = nc.dram_tensor("idxA1", [128, lenA1 // 16], I16, kind="ExternalInput")
    idxB1 = nc.dram_tensor("idxB1", [128, lenB1 // 16], I16, kind="ExternalInput")
    idxP = nc.dram_tensor("idxP", [128, lenP // 16], I16, kind="ExternalInput")
    W1 = nc.dram_tensor("W1", [D_IN, D1], F32, kind="ExternalInput")
    W2 = nc.dram_tensor("W2", [D1, D2], F32, kind="ExternalInput")
    W3 = nc.dram_tensor("W3", [D2, D3], F32, kind="ExternalInput")
    Wf1 = nc.dram_tensor("Wf1", [64, 128], F32, kind="ExternalInput")
    Wf2 = nc.dram_tensor("Wf2", [128, 64], F32, kind="ExternalInput")
    b1r = nc.dram_tensor("b1r", [128, D1], F32, kind="ExternalInput")
    b2r = nc.dram_tensor("b2r", [128, D2], F32, kind="ExternalInput")
    b3r = nc.dram_tensor("b3r", [128, D3], F32, kind="ExternalInput")
    bf1r = nc.dram_tensor("bf1r", [128, 128], F32, kind="ExternalInput")
    bf2r = nc.dram_tensor("bf2r", [128, 64], F32, kind="ExternalInput")
    gammar = nc.dram_tensor("gammar", [128, 64], F32, kind="ExternalInput")
    betar = nc.dram_tensor("betar", [128, 64], F32, kind="ExternalInput")

    out_slice = nc.dram_tensor("out_slice", [GPER, 64], F32, kind="ExternalOutput")

    # internal DRAM
    hW2own = nc.dram_tensor("hW2own", [1 + SHARD, D2], F16)
    hW2t = nc.dram_tensor("hW2t", [N + 2, D2], F16, addr_space="Shared")
    hW3own = nc.dram_tensor("hW3own", [1 + SHARD, D3], F32)
    hW3t = nc.dram_tensor("hW3t", [N + 2, D3], F32, addr_space="Shared")
    h3ot = nc.dram_tensor("h3ot", [1 + SHARD, D3], F32)
    poolin = nc.dram_tensor("poolin", [G, 64], F32)
    rs_out = nc.dram_tensor("rs_out", [GPER, 64], F32)

    dbg = {}
    if debug:
        for name, shape in [("d_hW2own", [SHARD, D2]),
                            ("d_hW2t", [N + 2, D2]), ("d_hW3own", [SHARD, D3]),
                            ("d_h3", [SHARD, D3]), ("d_poolin", [G, 64]),
                            ("d_rs", [GPER, 64])]:
            dt = F16 if name in ("d_hW2own", "d_hW2t") else F32
            dbg[name] = nc.dram_tensor(name, shape, dt, kind="ExternalOutput")

    rg = [list(range(NC))]

    with TileContext(nc) as tc, ExitStack() as ctx:
        const = ctx.enter_context(tc.tile_pool(name="const", bufs=1))
        idxp = ctx.enter_context(tc.tile_pool(name="idxp", bufs=1))

        # ---- constants
        ident = const.tile([128, 128], F32, tag="ident")
        make_identity(nc, ident[:])
        zt = const.tile([128, 128], F32, tag="zt")
        nc.vector.memset(zt[:], 0.0)
        zth = const.tile([128, 128], F16, tag="zth")
        nc.vector.memset(zth[:], 0.0)
        epss = const.tile([128, 1], F32, tag="epss")
        nc.vector.memset(epss[:], 1e-5)

        def csb(t, p0, p1, tag, dt=F32):
            tl = const.tile([p0, p1], dt, tag=tag)
            nc.sync.dma_start(out=tl[:], in_=t[:])
            return tl

        W1s = csb(W1, D_IN, D1, "W1s")
        W2s = csb(W2, D1, D2, "W2s")
        W3s = csb(W3, D2, D3, "W3s")
        Wf1s = csb(Wf1, 64, 128, "Wf1s")
        Wf2s = csb(Wf2, 128, 64, "Wf2s")
        b1s = csb(b1r, 128, D1, "b1s")
        b2s = csb(b2r, 128, D2, "b2s")
        b3s = csb(b3r, 128, D3, "b3s")
        bf1s = csb(bf1r, 128, 128, "bf1s")
        bf2s = csb(bf2r, 128, 64, "bf2s")
        gams = csb(gammar, 128, 64, "gams")
        bets = csb(betar, 128, 64, "bets")
        dvo = csb(dinv_own, 128, NBLK, "dvo")
        ics = csb(invc_own, 128, 1, "ics")

        idxO_s = idxp.tile([128, lenO // 16], I16, tag="io")
        nc.sync.dma_start(out=idxO_s[:], in_=idxO[:])
        idxA_s = idxp.tile([128, lenA // 16], I16, tag="ia")
        nc.sync.dma_start(out=idxA_s[:], in_=idxA[:])
        idxB_s = idxp.tile([128, lenB // 16], I16, tag="ib")
        nc.sync.dma_start(out=idxB_s[:], in_=idxB[:])
        idxA1_s = idxp.tile([128, lenA1 // 16], I16, tag="ia1")
        nc.sync.dma_start(out=idxA1_s[:], in_=idxA1[:])
        idxB1_s = idxp.tile([128, lenB1 // 16], I16, tag="ib1")
        nc.sync.dma_start(out=idxB1_s[:], in_=idxB1[:])
        idxP_s = idxp.tile([128, lenP // 16], I16, tag="ip")
        nc.sync.dma_start(out=idxP_s[:], in_=idxP[:])

        # table guard rows
        nc.scalar.dma_start(out=hW2own[0:1, :], in_=zth[0:1, 0:D2])
        nc.scalar.dma_start(out=hW3own[0:1, :], in_=zt[0:1, 0:D3])
        nc.scalar.dma_start(out=hW2t[0:1, :], in_=zth[0:1, 0:D2])
        nc.scalar.dma_start(out=hW2t[N + 1:N + 2, :], in_=zth[0:1, 0:D2])
        nc.scalar.dma_start(out=hW3t[0:1, :], in_=zt[0:1, 0:D3])
        nc.scalar.dma_start(out=hW3t[N + 1:N + 2, :], in_=zt[0:1, 0:D3])
        nc.scalar.dma_start(out=h3ot[0:1, :], in_=zt[0:1, 0:D3])

        # ---- fused sink helper ---------------------------------------------
        # store stage: collects per-block results, flushes one DMA per group
        def make_store_sink(dst, fout, dt, pname, row_off=0):
            """Returns (sink, finish). Block k -> dst rows row_off+128k.."""
            state = {"tile": None, "k0": None, "n": 0}
            pool = ctx.enter_context(tc.tile_pool(name=pname, bufs=2))

            def flush():
                t, k0_, n_ = state["tile"], state["k0"], state["n"]
                if t is None or n_ == 0:
                    return
                row0 = row_off + 128 * k0_
                nrows = min(SHARD, 128 * (k0_ + n_)) - 128 * k0_
                full = n_ - 1 if nrows < 128 * n_ else n_
                if full > 0:
                    nc.scalar.dma_start(
                        out=dst[row0:row0 + 128 * full, :].rearrange(
                            "(j p) f -> p j f", p=128),
                        in_=t[:, :full * fout].rearrange("p (j f) -> p j f", j=full))
                if full < n_:
                    rl = nrows - 128 * full
                    nc.scalar.dma_start(
                        out=dst[row0 + 128 * full:row0 + nrows, :],
                        in_=t[:rl, full * fout:(full + 1) * fout])
                state["tile"], state["k0"], state["n"] = None, None, 0

            def sink_store(k, rows, res):
                # res: [128, fout] tile (dtype dt) to be stored at block k
                if state["tile"] is None:
                    state["tile"] = pool.tile([128, 8 * fout], dt, tag="stg",
                                              name=pname + "_stg")
                    state["k0"], state["n"] = k, 0
                j = state["n"]
                nc.scalar.activation(
                    state["tile"][:, j * fout:(j + 1) * fout], res[:], AF.Copy)
                state["n"] += 1
                if state["n"] == 8:
                    flush()

            return sink_store, flush

        # ---- gather layer driver (superblock batched, O/A/B streams) ----
        def fold(gt, elem, c0, s, upto=1):
            while s > upto:
                h = min(s // 2, s - upto)
                nc.vector.tensor_tensor(
                    out=gt[:, c0 * elem:(c0 + h) * elem],
                    in0=gt[:, c0 * elem:(c0 + h) * elem],
                    in1=gt[:, (c0 + s - h) * elem:(c0 + s) * elem],
                    op=OP.add)
                s -= h

        def gcn_layer(elem, dt, bias_sb, sink, pre, mainA, mainB, supers_m):
            """pre:   (tab_ap, S, supers_p, idx_tile) — gathered+folded first
            into a per-block partial acc; its table is available before the
            main tables (local own-table, or the early-written hW1tA region),
            so this pass overlaps the collective / main table build.
            mainA:  (tab_ap, S, idx_tile) or None; mainB likewise.
            bias_sb None: tail applies only the dinv scale (no bias/relu) —
            used by L1, whose W1 matmul happens in the sink after aggregation.
            sink(k, rows, h_tile): consume finished block (h f32 [128, elem])."""
            with tc.tile_pool(name="acco", bufs=1) as acco_pool, \
                 tc.tile_pool(name="hacc", bufs=3) as hacc_pool:
                accO = None
                if pre is not None:
                    ptab, pS, psupers, pidx = pre
                    accO = acco_pool.tile([128, NBLK * elem], F32, tag="accO")
                    # ---- pre pass
                    with tc.tile_pool(name="gto", bufs=4) as gto_pool:
                        offO = 0
                        for (k0, k1) in psupers:
                            nO = sum(pS[k0:k1])
                            gtO = gto_pool.tile([128, SUPER_COLS * elem], dt,
                                                tag="gtO")
                            done = 0
                            while done < nO:
                                w = min(GW, nO - done)
                                nc.gpsimd.dma_gather(
                                    out_ap=gtO[:, done * elem:(done + w) * elem].rearrange(
                                        "p (c f) -> p c f", c=w),
                                    in_ap=ptab,
                                    idxs_ap=pidx[:, 8 * (offO + done):8 * (offO + done + w)],
                                    num_idxs=128 * w, num_idxs_reg=128 * w,
                                    elem_size=elem)
                                done += w
                            cO = 0
                            for k in range(k0, k1):
                                so = pS[k]
                                dst = accO[:, k * elem:(k + 1) * elem]
                                if so > 2:
                                    fold(gtO, elem, cO, so, upto=2)
                                    nc.vector.tensor_tensor(
                                        out=dst,
                                        in0=gtO[:, cO * elem:(cO + 1) * elem],
                                        in1=gtO[:, (cO + 1) * elem:(cO + 2) * elem],
                                        op=OP.add)
                                elif so == 2:
                                    nc.vector.tensor_tensor(
                                        out=dst,
                                        in0=gtO[:, cO * elem:(cO + 1) * elem],
                                        in1=gtO[:, (cO + 1) * elem:(cO + 2) * elem],
                                        op=OP.add)
                                elif so == 1:
                                    nc.vector.tensor_copy(
                                        out=dst,
                                        in_=gtO[:, cO * elem:(cO + 1) * elem])
                                else:
                                    nc.vector.memset(dst, 0.0)
                                cO += so
                            offO += nO
                # ---- main pass
                SA_l = mainA[1] if mainA else [0] * NBLK
                SB_l = mainB[1]
                with tc.tile_pool(name="gta", bufs=3) as gta_pool, \
                     tc.tile_pool(name="gtb", bufs=3) as gtb_pool:
                    offA = offB = 0
                    for (k0, k1) in supers_m:
                        nA = sum(SA_l[k0:k1])
                        nB = sum(SB_l[k0:k1])
                        gtA = gtB = None
                        if nA > 0:
                            gtA = gta_pool.tile([128, SUPER_COLS * elem], dt,
                                                tag="gtA", name="gtA")
                            done = 0
                            while done < nA:
                                w = min(GW, nA - done)
                                nc.gpsimd.dma_gather(
                                    out_ap=gtA[:, done * elem:(done + w) * elem].rearrange(
                                        "p (c f) -> p c f", c=w),
                                    in_ap=mainA[0],
                                    idxs_ap=mainA[2][:, 8 * (offA + done):8 * (offA + done + w)],
                                    num_idxs=128 * w, num_idxs_reg=128 * w,
                                    elem_size=elem)
                                done += w
                        if nB > 0:
                            gtB = gtb_pool.tile([128, SUPER_COLS * elem], dt,
                                                tag="gtB", name="gtB")
                            done = 0
                            while done < nB:
                                w = min(GW, nB - done)
                                nc.gpsimd.dma_gather(
                                    out_ap=gtB[:, done * elem:(done + w) * elem].rearrange(
                                        "p (c f) -> p c f", c=w),
                                    in_ap=mainB[0],
                                    idxs_ap=mainB[2][:, 8 * (offB + done):8 * (offB + done + w)],
                                    num_idxs=128 * w, num_idxs_reg=128 * w,
                                    elem_size=elem)
                                done += w
                        cA = cB = 0
                        for k in range(k0, k1):
                            sa, sb_ = SA_l[k], SB_l[k]
                            rows = min(128, SHARD - 128 * k)
                            acc = hacc_pool.tile([128, elem], F32, tag="acc")
                            have_acc = False
                            cols = []
                            for gt, c0, s in ((gtA, cA, sa), (gtB, cB, sb_)):
                                if s >= 2 and not have_acc:
                                    fold(gt, elem, c0, s, upto=2)
                                    nc.vector.tensor_tensor(
                                        out=acc[:],
                                        in0=gt[:, c0 * elem:(c0 + 1) * elem],
                                        in1=gt[:, (c0 + 1) * elem:(c0 + 2) * elem],
                                        op=OP.add)
                                    have_acc = True
                                elif s >= 1:
                                    fold(gt, elem, c0, s, upto=1)
                                    cols.append(gt[:, c0 * elem:(c0 + 1) * elem])
                            if accO is not None:
                                cols.append(accO[:, k * elem:(k + 1) * elem])
                            if not have_acc:
                                if len(cols) >= 2:
                                    nc.vector.tensor_tensor(
                                        out=acc[:], in0=cols[0], in1=cols[1],
                                        op=OP.add)
                                    cols = cols[2:]
                                else:
                                    nc.vector.tensor_copy(out=acc[:], in_=cols[0])
                                    cols = cols[1:]
                            for p in cols:
                                nc.vector.tensor_tensor(
                                    out=acc[:], in0=acc[:], in1=p, op=OP.add)
                            cA += sa
                            cB += sb_
                            # h = relu(acc * dinv + bias)   (bias/relu skipped
                            # when bias_sb is None — L1 applies W1 first)
                            nc.scalar.activation(acc[:], acc[:], AF.Copy,
                                                 scale=dvo[:, k:k + 1])
                            if bias_sb is not None:
                                nc.vector.tensor_tensor(
                                    out=acc[:], in0=acc[:], in1=bias_sb[:], op=OP.add)
                                nc.scalar.activation(acc[:], acc[:], AF.Relu)
                            sink(k, rows, acc)
                        offA += nA
                        offB += nB

        # ---- L1: gather raw xdp -> agg -> @W1 -> h1 -> fused hW2own (fp16) ----
        store2, flush2 = make_store_sink(hW2own, D2, F16, "st2", row_off=1)
        with tc.tile_pool(name="tp1", bufs=2, space="PSUM") as tp1, \
             tc.tile_pool(name="mm0", bufs=2, space="PSUM") as mm0, \
             tc.tile_pool(name="mm1", bufs=2, space="PSUM") as mm1, \
             tc.tile_pool(name="hT1", bufs=2) as hT1p, \
             tc.tile_pool(name="h1b", bufs=2) as h1bp, \
             tc.tile_pool(name="r1", bufs=2) as r1p:
            def sink1(k, rows, agg):
                # agg: [128, 64] f32 = dinv * sum of xd rows (37 valid cols).
                # h1 = relu(agg @ W1 + b1); then source-scale, @W2 -> fp16.
                tp0 = tp1.tile([64, 128], F32, tag="tp0", name="tp0")
                nc.tensor.transpose(out=tp0[:], in_=agg[:], identity=ident[:])
                aT = hT1p.tile([64, 128], F32, tag="aT", name="aT")
                nc.scalar.activation(aT[:], tp0[:], AF.Copy)
                ps0 = mm0.tile([128, D1], F32, tag="ps0", name="ps0")
                nc.tensor.matmul(out=ps0[:], lhsT=aT[:D_IN, :], rhs=W1s[:],
                                 start=True, stop=True)
                h = h1bp.tile([128, D1], F32, tag="h1t", name="h1t")
                nc.vector.tensor_tensor(out=h[:], in0=ps0[:], in1=b1s[:], op=OP.add)
                nc.scalar.activation(h[:], h[:], AF.Relu)
                nc.vector.tensor_tensor(
                    out=h[:], in0=h[:],
                    in1=dvo[:, k:k + 1].to_broadcast([128, D1]), op=OP.mult)
                tp = tp1.tile([D1, 128], F32, tag="tp")
                nc.tensor.transpose(out=tp[:], in_=h[:], identity=ident[:])
                hT = hT1p.tile([D1, 128], F32, tag="hT")
                nc.vector.tensor_copy(out=hT[:], in_=tp[:])
                ps = mm1.tile([128, D2], F32, tag="ps")
                nc.tensor.matmul(out=ps[:], lhsT=hT[:], rhs=W2s[:],
                                 start=True, stop=True)
                res = r1p.tile([128, D2], F16, tag="res")
                nc.scalar.activation(res[:], ps[:], AF.Copy)
                store2(k, rows, res)
            gcn_layer(D1, F32, None, sink1,
                      pre=None,
                      mainA=(xdp[:], SA1u, idxA1_s),
                      mainB=(xdp[BASE_B:, :], SB1u, idxB1_s),
                      supers_m=supers1)
            flush2()

        # ---- AllGather hW2own (fp16) ----
        nc.gpsimd.collective_compute(
            "AllGather", OP.bypass, replica_groups=rg,
            ins=[hW2own[1:, :]], outs=[hW2t[1:N + 1, :]])

        # ---- L2: gather hW2t (fp16) -> h2 -> fused hW3own (fp32) ----
        store3, flush3 = make_store_sink(hW3own, D3, F32, "st3", row_off=1)
        with tc.tile_pool(name="tp2", bufs=2, space="PSUM") as tp2, \
             tc.tile_pool(name="mm2", bufs=2, space="PSUM") as mm2, \
             tc.tile_pool(name="hT2", bufs=2) as hT2p, \
             tc.tile_pool(name="r2", bufs=2) as r2p:
            def sink2(k, rows, h):
                # h: [128, 128] f32. Source-scale, then @W3.
                nc.vector.tensor_tensor(
                    out=h[:], in0=h[:],
                    in1=dvo[:, k:k + 1].to_broadcast([128, D2]), op=OP.mult)
                tp = tp2.tile([D2, 128], F32, tag="tp")
                nc.tensor.transpose(out=tp[:], in_=h[:], identity=ident[:])
                hT = hT2p.tile([D2, 128], F32, tag="hT")
                nc.vector.tensor_copy(out=hT[:], in_=tp[:])
                ps = mm2.tile([128, D3], F32, tag="ps")
                nc.tensor.matmul(out=ps[:], lhsT=hT[:], rhs=W3s[:],
                                 start=True, stop=True)
                res = r2p.tile([128, D3], F32, tag="res")
                nc.scalar.activation(res[:], ps[:], AF.Copy)
                store3(k, rows, res)
            gcn_layer(D2, F16, b2s, sink2,
                      pre=(hW2own[:], SOu, supersO, idxO_s),
                      mainA=(hW2t[:], SAu, idxA_s),
                      mainB=(hW2t[BASE_B:, :], SBu, idxB_s),
                      supers_m=supers)
            flush3()

        # ---- AllGather hW3own (fp32) ----
        nc.gpsimd.collective_compute(
            "AllGather", OP.bypass, replica_groups=rg,
            ins=[hW3own[1:, :]], outs=[hW3t[1:N + 1, :]])

        # ---- L3: gather hW3t -> h3 -> h3ot ----
        storeh3, flushh3 = make_store_sink(h3ot, D3, F32, "sth3", row_off=1)
        gcn_layer(D3, F32, b3s, storeh3,
                  pre=(hW3own[:], SOu, supersO, idxO_s),
                  mainA=(hW3t[:], SAu, idxA_s),
                  mainB=(hW3t[BASE_B:, :], SBu, idxB_s),
                  supers_m=supers)
        flushh3()

        # ---- pooling: one batched gather over all graph-blocks, fold per gb ----
        with tc.tile_pool(name="gp", bufs=1) as gp, \
             tc.tile_pool(name="pstg", bufs=1) as pstg:
            stg = pstg.tile([128, GBLK * 64], F32, tag="pstg")
            spT = sum(SPu)
            gt = gp.tile([128, spT * 64], F32, tag="gtp")
            done = 0
            while done < spT:
                w = min(GW, spT - done)
                nc.gpsimd.dma_gather(
                    out_ap=gt[:, done * 64:(done + w) * 64].rearrange(
                        "p (c f) -> p c f", c=w),
                    in_ap=h3ot[:],
                    idxs_ap=idxP_s[:, 8 * done:8 * (done + w)],
                    num_idxs=128 * w, num_idxs_reg=128 * w, elem_size=64)
                done += w
            offP = 0
            for gb in range(GBLK):
                sp = SPu[gb]
                fold(gt, 64, offP, sp, upto=2)
                nc.vector.tensor_tensor(
                    out=stg[:, 64 * gb:64 * (gb + 1)],
                    in0=gt[:, offP * 64:(offP + 1) * 64],
                    in1=gt[:, (offP + 1) * 64:(offP + 2) * 64], op=OP.add)
                offP += sp
            nc.scalar.dma_start(
                out=poolin[:, :].rearrange("(j p) f -> p j f", p=GPER),
                in_=stg[:GPER, :].rearrange("p (j f) -> p j f", j=GBLK))

        # ---- ReduceScatter pooled partial sums -> own 125 graphs ----
        nc.gpsimd.collective_compute(
            "ReduceScatter", OP.add, replica_groups=rg,
            ins=[poolin[:]], outs=[rs_out[:]])

        # ---- MLP + LayerNorm on own 125 graphs ----
        with tc.tile_pool(name="mlp", bufs=1) as mlp, \
             tc.tile_pool(name="mps", bufs=2, space="PSUM") as mps:
            gtl = mlp.tile([128, 64], F32, tag="g0")
            nc.vector.memset(gtl[:], 0.0)
            nc.sync.dma_start(out=gtl[:GPER, :], in_=rs_out[:])
            # mean pool scale (per-partition scalar)
            nc.vector.tensor_tensor(
                out=gtl[:], in0=gtl[:],
                in1=ics[:].to_broadcast([128, 64]), op=OP.mult)
            # dense1: relu(g @ Wf1 + bf1)
            tp = mps.tile([64, 128], F32, tag="t1")
            nc.tensor.transpose(out=tp[:], in_=gtl[:], identity=ident[:])
            gT = mlp.tile([64, 128], F32, tag="gT")
            nc.vector.tensor_copy(out=gT[:], in_=tp[:])
            p1 = mps.tile([128, 128], F32, tag="p1")
            nc.tensor.matmul(out=p1[:], lhsT=gT[:], rhs=Wf1s[:],
                             start=True, stop=True)
            g1t = mlp.tile([128, 128], F32, tag="g1t")
            nc.vector.tensor_tensor(out=g1t[:], in0=p1[:], in1=bf1s[:], op=OP.add)
            nc.scalar.activation(g1t[:], g1t[:], AF.Relu)
            # dense2: relu(g1 @ Wf2 + bf2)
            tp2_ = mps.tile([128, 128], F32, tag="t2")
            nc.tensor.transpose(out=tp2_[:], in_=g1t[:], identity=ident[:])
            g1T = mlp.tile([128, 128], F32, tag="g1T")
            nc.vector.tensor_copy(out=g1T[:], in_=tp2_[:])
            p2 = mps.tile([128, 64], F32, tag="p2")
            nc.tensor.matmul(out=p2[:], lhsT=g1T[:], rhs=Wf2s[:],
                             start=True, stop=True)
            g2t = mlp.tile([128, 64], F32, tag="g2t")
            nc.vector.tensor_tensor(out=g2t[:], in0=p2[:], in1=bf2s[:], op=OP.add)
            nc.scalar.activation(g2t[:], g2t[:], AF.Relu)
            # layernorm over 64 features
            mu = mlp.tile([128, 1], F32, tag="mu")
            nc.vector.reduce_sum(mu[:], g2t[:], axis=AX.X)
            nc.vector.tensor_scalar_mul(mu[:], in0=mu[:], scalar1=1.0 / 64)
            xm = mlp.tile([128, 64], F32, tag="xm")
            nc.vector.tensor_tensor(out=xm[:], in0=g2t[:],
                                    in1=mu[:].to_broadcast([128, 64]),
                                    op=OP.subtract)
            sq = mlp.tile([128, 64], F32, tag="sq")
            nc.vector.tensor_tensor(out=sq[:], in0=xm[:], in1=xm[:], op=OP.mult)
            var = mlp.tile([128, 1], F32, tag="var")
            nc.vector.reduce_sum(var[:], sq[:], axis=AX.X)
            rstd = mlp.tile([128, 1], F32, tag="rstd")
            nc.vector.tensor_scalar_mul(var[:], in0=var[:], scalar1=1.0 / 64)
            nc.vector.tensor_tensor(out=var[:], in0=var[:], in1=epss[:],
                                    op=OP.add)
            nc.scalar.activation(rstd[:], var[:], AF.Sqrt)
            nc.vector.reciprocal(rstd[:], rstd[:])
            nc.vector.tensor_tensor(out=xm[:], in0=xm[:],
                                    in1=rstd[:].to_broadcast([128, 64]),
                                    op=OP.mult)
            nc.vector.tensor_tensor(out=xm[:], in0=xm[:], in1=gams[:], op=OP.mult)
            nc.vector.tensor_tensor(out=xm[:], in0=xm[:], in1=bets[:], op=OP.add)
            nc.sync.dma_start(out=out_slice[:, :], in_=xm[:GPER, :])

        # ---- debug dumps
        if debug:
            with tc.tile_pool(name="dbg", bufs=2) as dp:
                def dump(src, dst, nrows, width, dt=F32):
                    for c in range((nrows + 127) // 128):
                        rows = min(128, nrows - 128 * c)
                        t = dp.tile([128, width], dt, tag="dt")
                        nc.sync.dma_start(out=t[:rows, :],
                                          in_=src[128 * c:128 * c + rows, :])
                        nc.sync.dma_start(out=dst[128 * c:128 * c + rows, :],
                                          in_=t[:rows, :])
                dump(hW2own[1:, :], dbg["d_hW2own"], SHARD, D2, F16)
                dump(hW2t, dbg["d_hW2t"], N + 2, D2, F16)
                dump(hW3own[1:, :], dbg["d_hW3own"], SHARD, D3)
                dump(h3ot[1:, :], dbg["d_h3"], SHARD, D3)
                dump(poolin, dbg["d_poolin"], G, 64)
                dump(rs_out, dbg["d_rs"], GPER, 64)

    nc.compile()
    nc.finalize()
    return nc


# ==== SPMD runner (same as baseline) ====
import jax
from jax.sharding import Mesh, PartitionSpec
from jax.experimental.shard_map import shard_map

from concourse import bass2jax


class SpmdRunner:
    def __init__(self, nc, n_cores=8):
        bass2jax.install_neuronx_cc_hook()
        self.nc = nc
        self.n_cores = n_cores
        partition_name = nc.partition_id_tensor.name if nc.partition_id_tensor else None
        in_names, out_names, out_avals, zero_outs = [], [], [], []
        for alloc in nc.m.functions[0].allocations:
            if not isinstance(alloc, mybir.MemoryLocationSet):
                continue
            name = alloc.memorylocations[0].name
            if alloc.kind == "ExternalInput":
                if name != partition_name:
                    in_names.append(name)
            elif alloc.kind == "ExternalOutput":
                shape = tuple(alloc.tensor_shape)
                dtype = mybir.dt.np(alloc.dtype)
                out_names.append(name)
                out_avals.append(jax.core.ShapedArray(shape, dtype))
                zero_outs.append(np.zeros(shape, dtype))
        self.in_names = list(in_names)
        self.out_names = out_names
        self.out_avals = out_avals
        self.zero_outs = zero_outs
        n_params = len(in_names)
        n_outs = len(out_avals)
        all_in_names = in_names + out_names + ([partition_name] if partition_name else [])
        self.n_params = n_params

        def _body(*args):
            operands = list(args)
            if partition_name is not None:
                operands.append(bass2jax.partition_id_tensor())
            outs = bass2jax._bass_exec_p.bind(
                *operands,
                out_avals=tuple(out_avals),
                in_names=tuple(all_in_names),
                out_names=tuple(out_names),
                lowering_input_output_aliases=(),
                sim_require_finite=True,
                sim_require_nnan=True,
                nc=nc,
            )
            return tuple(outs)

        try:
            devices = jax.devices("axon")[:n_cores]
        except RuntimeError:
            devices = jax.devices()[:n_cores]
        mesh = Mesh(np.asarray(devices), ("core",))
        in_specs = (PartitionSpec("core"),) * (n_params + n_outs)
        out_specs = (PartitionSpec("core"),) * n_outs
        self.fn = jax.jit(
            shard_map(_body, mesh=mesh, in_specs=in_specs, out_specs=out_specs,
                      check_rep=False),
            keep_unused=True,
        )

    def stage(self, in_maps):
        concat = [
            np.concatenate([np.asarray(in_maps[c][n]) for c in range(self.n_cores)], axis=0)
            for n in self.in_names
        ]
        zeros = [np.zeros((self.n_cores * z.shape[0], *z.shape[1:]), z.dtype)
                 for z in self.zero_outs]
        return concat + zeros

    def run(self, staged):
        out = self.fn(*staged)
        jax.block_until_ready(out)
        return out

    def unpack(self, out_arrs):
        return [
            {
                name: np.asarray(out_arrs[i]).reshape(
                    self.n_cores, *self.out_avals[i].shape)[c]
                for i, name in enumerate(self.out_names)
            }
            for c in range(self.n_cores)
        ]


# ---- public entry point -----------------------------------------------------
_CACHE = {}


def kernel(**inputs):
    """Full-input GCN encoder on 8 NeuronCores; returns [1000, 64] float32."""
    inputs = {k: np.asarray(v) for k, v in inputs.items()}
    P = preprocess(inputs["edge_index"], inputs["batch"])
    key = tuple(tuple(P[k].tolist())
                for k in ("SOu", "SAu", "SBu", "SA1u", "SB1u", "SPu"))
    if key not in _CACHE:
        S = {k: P[k] for k in ("SOu", "SAu", "SBu", "SA1u", "SB1u", "SPu")}
        nc = build(S, debug=False)
        _CACHE[key] = SpmdRunner(nc, 8)
    r = _CACHE[key]
    in_maps = make_in_maps(inputs, P)
    staged = r.stage(in_maps)
    res = r.unpack(r.run(staged))
    return np.ascontiguousarray(
        np.concatenate([res[c]["out_slice"] for c in range(NC)], axis=0),
        dtype=np.float32)



# revision 3
# speedup vs baseline: 1.0477x; 1.0477x over previous
"""Trainium2 Bass kernel for nn_LinkerEncoder — v3.

Structure vs v2:
- Tables between layers are fp16 PAIR-PACKED: row = two consecutive
  positions' 64-wide vectors (256B = min gather elem). AllGather ships half
  the bytes of v2 (6.4MB -> 2x ~3.2MB chunks).
- Node classes = (chunk, parity): position chunk 0 = blocks 0..24 (3200
  pos), chunk 1 = blocks 25..48 (3050). Host greedily balances classes per
  dest so the per-block per-class slot maxes stay tight.
- L2/L3 gather streams are flat per chunk: per block [E cols][O cols], one
  gather table per chunk; fold reads the wanted 64-wide half via strided
  views (parity known at build time).
- Each AllGather is split into 2 chunk collectives: chunk-0 fires mid-way
  through the producing layer and overlaps its tail; the consuming layer's
  chunk-0 pass overlaps the chunk-1 collective (pre-pass into an
  accumulator, like v2's O-pre-pass).
- L1 keeps the v2 A1/B1 int16-window streams over the f32 xdp table.
"""
import numpy as np

N = 50000
NC = 8
SHARD = N // NC          # 6250
NBLK = (SHARD + 127) // 128   # 49
BASE_B = 17234
B_PAD_IDX = N + 1 - BASE_B
CLS_OF_CORE = np.array([0, 0, 0, 1, 1, 2, 2, 2], np.int32)
G = 1000
GBLK = 8
GPER = G // GBLK

CHUNK_POS = [(0, 3200), (3200, 6250)]
CHUNK_BLK = [(0, 25), (25, 49)]
NPAIRS = [1600, 1525]
S_KEYS = ("SE", "SO", "SA1", "SB1", "SPu")


def wrap16(stream):
    L = len(stream)
    w = np.asarray(stream, np.int32).reshape(L // 16, 16).T.astype(np.int16)
    return np.tile(w, (8, 1))


def preprocess(edge_index, batch):
    src = np.asarray(edge_index[0], np.int64)
    dst = np.asarray(edge_index[1], np.int64)
    batch = np.asarray(batch, np.int64)

    indeg = np.bincount(dst, minlength=N).astype(np.int64) + 1
    dinv = 1.0 / np.sqrt(indeg.astype(np.float64))

    rank = np.argsort(-indeg, kind="stable")
    core_of = np.empty(N, np.int32)
    core_of[rank] = np.arange(N, dtype=np.int32) % NC

    order_d = np.argsort(dst, kind="stable")
    s_sorted = src[order_d]
    d_sorted = dst[order_d]
    d_starts = np.searchsorted(d_sorted, np.arange(N))
    d_ends = np.searchsorted(d_sorted, np.arange(N) + 1)

    order_s = np.argsort(src, kind="stable")
    d_by_s = dst[order_s]
    s_starts = np.searchsorted(src[order_s], np.arange(N))
    s_ends = np.searchsorted(src[order_s], np.arange(N) + 1)

    # ---- class assignment: K=4 (chunk x parity), greedy balance per dest
    K = 4
    cnt = np.zeros((N, K), np.int32)
    cap = np.zeros((NC, K), np.int64)
    for t in range(2):
        cap[:, 2 * t] = NPAIRS[t]
        cap[:, 2 * t + 1] = NPAIRS[t]
    cls = np.empty(N, np.int32)
    outdeg = (s_ends - s_starts) + 1
    proc = np.argsort(-outdeg, kind="stable")
    BIG = 1 << 30
    for v in proc:
        c = core_of[v]
        ds = np.concatenate([d_by_s[s_starts[v]:s_ends[v]], [v]])
        loads = cnt[ds]
        mx = loads.max(axis=1, keepdims=True)
        score = (loads >= mx).sum(axis=0) * 1000 + loads.sum(axis=0)
        score = np.where(cap[c] > 0, score, BIG)
        k = int(np.argmin(score))
        cls[v] = k
        cap[c, k] -= 1
        cnt[ds, k] += 1

    # exact per-dest per-class counts (greedy's cnt drops multi-edge dups)
    cnt = np.zeros((N, K), np.int32)
    for k in range(K):
        cnt[:, k] = np.bincount(dst[cls[src] == k], minlength=N)
    cnt[np.arange(N), cls] += 1  # self loop

    # ---- L1 window balance (flat counts incl self)
    own_cls = CLS_OF_CORE[core_of]
    cls1 = CLS_OF_CORE[core_of[src]]

    def balance(sel_dst, sel_cls, extra_a, extra_l, extra_b):
        fa = np.bincount(sel_dst, weights=(sel_cls == 0).astype(np.float64),
                         minlength=N).astype(np.int64) + extra_a
        fl = np.bincount(sel_dst, weights=(sel_cls == 1).astype(np.float64),
                         minlength=N).astype(np.int64) + extra_l
        fb = np.bincount(sel_dst, weights=(sel_cls == 2).astype(np.float64),
                         minlength=N).astype(np.int64) + extra_b
        x = np.clip((fb - fa + fl + 1) // 2, 0, fl)
        return fa + x, fb + fl - x, x

    a1_cnt, b1_cnt, xflex1 = balance(
        dst, cls1, (own_cls == 0).astype(np.int64),
        (own_cls == 1).astype(np.int64), (own_cls == 2).astype(np.int64))

    # ---- position assignment
    mxK = cnt.max(axis=1)
    mx1 = np.maximum(a1_cnt, b1_cnt)
    pos_of = np.empty(N, np.int64)
    for c in range(NC):
        for t in range(2):
            base = CHUNK_POS[t][0]
            for par in range(2):
                k = 2 * t + par
                nodes = np.where((core_of == c) & (cls == k))[0]
                nodes = nodes[np.lexsort((mx1[nodes], mxK[nodes]))]
                pos_of[nodes] = SHARD * c + base + 2 * np.arange(len(nodes)) + par
    node_at = np.empty(N, np.int64)
    node_at[pos_of] = np.arange(N)

    local_of = pos_of % SHARD
    blk_of = local_of // 128
    SE = np.zeros((2, NBLK), np.int64)
    SO = np.zeros((2, NBLK), np.int64)
    SA1 = np.zeros(NBLK, np.int64)
    SB1 = np.zeros(NBLK, np.int64)
    for k in range(NBLK):
        sel = blk_of == k
        for t in range(2):
            SE[t, k] = max(int(cnt[sel, 2 * t].max()), 1)
            SO[t, k] = max(int(cnt[sel, 2 * t + 1].max()), 1)
        SA1[k] = max(int(a1_cnt[sel].max()), 1)
        SB1[k] = max(int(b1_cnt[sel].max()), 1)

    # ---- streams
    lenT = [int((SE[t] + SO[t]).sum()) * 128 for t in range(2)]
    streamsT = [np.zeros((NC, lenT[t]), np.int32) for t in range(2)]
    lenA1 = int(SA1.sum()) * 128
    lenB1 = int(SB1.sum()) * 128
    streamsA1 = np.zeros((NC, lenA1), np.int32)
    streamsB1 = np.full((NC, lenB1), B_PAD_IDX, np.int32)

    chunk_of_local = (local_of >= CHUNK_POS[1][0]).astype(np.int64)
    pair_of = np.empty(N, np.int64)
    for t in range(2):
        selt = chunk_of_local == t
        pair_of[selt] = (local_of[selt] - CHUNK_POS[t][0]) // 2
    par_of = local_of % 2
    trow = 1 + core_of * np.array(NPAIRS)[chunk_of_local] + pair_of

    for c in range(NC):
        offT = [0, 0]
        offA1 = offB1 = 0
        for k in range(NBLK):
            lo, hi = 128 * k, min(128 * (k + 1), SHARD)
            gE = [np.zeros((int(SE[t][k]), 128), np.int32) for t in range(2)]
            gO = [np.zeros((int(SO[t][k]), 128), np.int32) for t in range(2)]
            gridA1 = np.zeros((int(SA1[k]), 128), np.int32)
            gridB1 = np.full((int(SB1[k]), 128), B_PAD_IDX, np.int32)
            for lane in range(hi - lo):
                d = node_at[SHARD * c + lo + lane]
                edge_srcs = s_sorted[d_starts[d]:d_ends[d]]
                srcs = np.concatenate([edge_srcs, [d]])
                fE = [0, 0]
                fO = [0, 0]
                for s in srcs:
                    t = int(chunk_of_local[s])
                    if par_of[s] == 0:
                        gE[t][fE[t], lane] = trow[s]
                        fE[t] += 1
                    else:
                        gO[t][fO[t], lane] = trow[s]
                        fO[t] += 1
                aps = pos_of[srcs]
                acls = np.concatenate([CLS_OF_CORE[core_of[edge_srcs]],
                                       [CLS_OF_CORE[c]]])
                flex1 = aps[acls == 1]
                x1 = int(xflex1[d])
                pa1 = np.concatenate([aps[acls == 0], flex1[:x1]]) + 1
                pb1 = np.concatenate([aps[acls == 2], flex1[x1:]]) + 1 - BASE_B
                gridA1[: len(pa1), lane] = pa1
                gridB1[: len(pb1), lane] = pb1
            for t in range(2):
                bg = np.concatenate([gE[t], gO[t]], axis=0)
                streamsT[t][c, offT[t]:offT[t] + bg.size] = bg.ravel()
                offT[t] += bg.size
            streamsA1[c, offA1:offA1 + gridA1.size] = gridA1.ravel()
            streamsB1[c, offB1:offB1 + gridB1.size] = gridB1.ravel()
            offA1 += gridA1.size
            offB1 += gridB1.size

    # ---- pooling tables
    SP = np.zeros((NC, GBLK), np.int64)
    members = {}
    nodes_of_core = [node_at[SHARD * c:SHARD * (c + 1)] for c in range(NC)]
    for c in range(NC):
        g_of_local = batch[nodes_of_core[c]]
        for gb in range(GBLK):
            cnts = np.bincount(
                g_of_local[(g_of_local >= GPER * gb) & (g_of_local < GPER * (gb + 1))] - GPER * gb,
                minlength=GPER)
            SP[c, gb] = max(cnts.max(), 1)
        members[c] = g_of_local
    SPu = SP.max(axis=0)
    lenP = int(SPu.sum()) * 128
    streamsP = np.zeros((NC, lenP), np.int32)
    for c in range(NC):
        g_of_local = members[c]
        off = 0
        for gb in range(GBLK):
            grid = np.zeros((int(SPu[gb]), 128), np.int32)
            for gl in range(GPER):
                locs = np.where(g_of_local == GPER * gb + gl)[0] + 1
                grid[: len(locs), gl] = locs
            streamsP[c, off:off + grid.size] = grid.ravel()
            off += grid.size

    cnts = np.bincount(batch, minlength=G).astype(np.float64)
    inv_cnt = (1.0 / np.maximum(cnts, 1.0)).astype(np.float32)

    return dict(
        core_of=core_of, pos_of=pos_of, node_at=node_at,
        dinv=dinv.astype(np.float32),
        SE=SE, SO=SO, SA1=SA1, SB1=SB1, SPu=SPu,
        streamsT=streamsT, streamsA1=streamsA1, streamsB1=streamsB1,
        streamsP=streamsP, inv_cnt=inv_cnt,
    )


# ==== host IO staging ====

def make_in_maps(inputs, P):
    node_at = P["node_at"]
    dinv = P["dinv"]
    x = np.asarray(inputs["x"], np.float32)
    xd = x * dinv[:, None]
    xp = xd[node_at]
    xdp = np.zeros((N + 2, 64), np.float32)
    xdp[1:N + 1, :37] = xp

    dinv_pos = dinv[node_at].astype(np.float32)

    def rep(v, width):
        return np.tile(np.asarray(v, np.float32)[None, :], (128, 1))

    common = dict(
        xdp=xdp,
        W1=np.asarray(inputs["W1"], np.float32),
        W2=np.asarray(inputs["W2"], np.float32),
        W3=np.asarray(inputs["W3"], np.float32),
        Wf1=np.asarray(inputs["Wf1"], np.float32),
        Wf2=np.asarray(inputs["Wf2"], np.float32),
        b1r=rep(inputs["b1"], 64), b2r=rep(inputs["b2"], 128),
        b3r=rep(inputs["b3"], 64), bf1r=rep(inputs["bf1"], 128),
        bf2r=rep(inputs["bf2"], 64), gammar=rep(inputs["gamma"], 64),
        betar=rep(inputs["beta"], 64),
    )

    in_maps = []
    for c in range(NC):
        dvo = np.zeros((128, NBLK), np.float32)
        own = dinv_pos[SHARD * c:SHARD * (c + 1)]
        for k in range(NBLK):
            rows = min(128, SHARD - 128 * k)
            dvo[:rows, k] = own[128 * k:128 * k + rows]
        invc = np.zeros((128, 1), np.float32)
        invc[:GPER, 0] = P["inv_cnt"][GPER * c:GPER * (c + 1)]
        m = dict(common)
        m["dinv_own"] = dvo
        m["invc_own"] = invc
        m["idxT0"] = wrap16(P["streamsT"][0][c])
        m["idxT1"] = wrap16(P["streamsT"][1][c])
        m["idxA1"] = wrap16(P["streamsA1"][c])
        m["idxB1"] = wrap16(P["streamsB1"][c])
        m["idxP"] = wrap16(P["streamsP"][c])
        in_maps.append(m)
    return in_maps


# ==== bass kernel ====
from contextlib import ExitStack

import concourse.bass as bass
import concourse.bacc as bacc
import concourse.mybir as mybir
from concourse.tile import TileContext
from concourse.masks import make_identity

F32 = mybir.dt.float32
F16 = mybir.dt.float16
I16 = mybir.dt.int16
AX = mybir.AxisListType
AF = mybir.ActivationFunctionType
OP = mybir.AluOpType

D_IN, D1, D2, D3 = 37, 64, 128, 64
GW = 8
SUPER_COLS = 64


def make_supers(SAu, SBu, flush_blocks=()):
    """Group consecutive blocks with combined cols <= SUPER_COLS; force a
    group boundary at each block in flush_blocks."""
    supers = []
    k0 = 0
    na = nb = 0
    for k in range(NBLK):
        if k > k0 and (na + SAu[k] > SUPER_COLS or nb + SBu[k] > SUPER_COLS
                       or k - k0 >= 16 or k in flush_blocks):
            supers.append((k0, k))
            k0, na, nb = k, 0, 0
        na += SAu[k]
        nb += SBu[k]
    supers.append((k0, NBLK))
    if supers[-1][1] - supers[-1][0] > 1:
        k0, k1 = supers[-1]
        supers[-1] = (k0, k1 - 1)
        supers.append((k1 - 1, k1))
    return supers


def build(S, debug=False):
    SE = [list(map(int, S["SE"][t])) for t in range(2)]
    SO = [list(map(int, S["SO"][t])) for t in range(2)]
    SA1 = list(map(int, S["SA1"]))
    SB1 = list(map(int, S["SB1"]))
    SPu = list(map(int, S["SPu"]))
    lenT = [128 * sum(SE[t][k] + SO[t][k] for k in range(NBLK)) for t in range(2)]
    lenA1, lenB1, lenP = 128 * sum(SA1), 128 * sum(SB1), 128 * sum(SPu)
    supers1 = make_supers(SA1, SB1)
    ST = [[SE[t][k] + SO[t][k] for k in range(NBLK)] for t in range(2)]
    supersT = [make_supers(ST[t], ST[t], flush_blocks=(25,)) for t in range(2)]

    nc = bacc.Bacc()

    # ---------------- IO ----------------
    xdp = nc.dram_tensor("xdp", [N + 2, 64], F32, kind="ExternalInput")
    dinv_own = nc.dram_tensor("dinv_own", [128, NBLK], F32, kind="ExternalInput")
    invc_own = nc.dram_tensor("invc_own", [128, 1], F32, kind="ExternalInput")
    idxT0 = nc.dram_tensor("idxT0", [128, lenT[0] // 16], I16, kind="ExternalInput")
    idxT1 = nc.dram_tensor("idxT1", [128, lenT[1] // 16], I16, kind="ExternalInput")
    idxA1 = nc.dram_tensor("idxA1", [128, lenA1 // 16], I16, kind="ExternalInput")
    idxB1 = nc.dram_tensor("idxB1", [128, lenB1 // 16], I16, kind="ExternalInput")
    idxP = nc.dram_tensor("idxP", [128, lenP // 16], I16, kind="ExternalInput")
    W1 = nc.dram_tensor("W1", [D_IN, D1], F32, kind="ExternalInput")
    W2 = nc.dram_tensor("W2", [D1, D2], F32, kind="ExternalInput")
    W3 = nc.dram_tensor("W3", [D2, D3], F32, kind="ExternalInput")
    Wf1 = nc.dram_tensor("Wf1", [64, 128], F32, kind="ExternalInput")
    Wf2 = nc.dram_tensor("Wf2", [128, 64], F32, kind="ExternalInput")
    b1r = nc.dram_tensor("b1r", [128, D1], F32, kind="ExternalInput")
    b2r = nc.dram_tensor("b2r", [128, D2], F32, kind="ExternalInput")
    b3r = nc.dram_tensor("b3r", [128, D3], F32, kind="ExternalInput")
    bf1r = nc.dram_tensor("bf1r", [128, 128], F32, kind="ExternalInput")
    bf2r = nc.dram_tensor("bf2r", [128, 64], F32, kind="ExternalInput")
    gammar = nc.dram_tensor("gammar", [128, 64], F32, kind="ExternalInput")
    betar = nc.dram_tensor("betar", [128, 64], F32, kind="ExternalInput")

    out_slice = nc.dram_tensor("out_slice", [GPER, 64], F32, kind="ExternalOutput")

    # internal DRAM: pair tables per chunk
    h1own = [nc.dram_tensor(f"h1own{t}", [1 + NPAIRS[t], 128], F16)
             for t in range(2)]
    T1 = [nc.dram_tensor(f"T1_{t}", [1 + NC * NPAIRS[t], 128], F16,
                         addr_space="Shared") for t in range(2)]
    hW3own = [nc.dram_tensor(f"hW3own{t}", [1 + NPAIRS[t], 128], F16)
              for t in range(2)]
    T3 = [nc.dram_tensor(f"T3_{t}", [1 + NC * NPAIRS[t], 128], F16,
                         addr_space="Shared") for t in range(2)]
    h3ot = nc.dram_tensor("h3ot", [1 + SHARD, D3], F32)
    poolin = nc.dram_tensor("poolin", [G, 64], F32)
    rs_out = nc.dram_tensor("rs_out", [GPER, 64], F32)

    dbg = {}
    if debug:
        for name, shape, dt in [
                ("d_h1own0", [1 + NPAIRS[0], 128], F16),
                ("d_h1own1", [1 + NPAIRS[1], 128], F16),
                ("d_T1_0", [1 + NC * NPAIRS[0], 128], F16),
                ("d_hW3own0", [1 + NPAIRS[0], 128], F16),
                ("d_h3", [SHARD, D3], F32),
                ("d_poolin", [G, 64], F32), ("d_rs", [GPER, 64], F32)]:
            dbg[name] = nc.dram_tensor(name, shape, dt, kind="ExternalOutput")

    rg = [list(range(NC))]

    with TileContext(nc) as tc, ExitStack() as ctx:
        const = ctx.enter_context(tc.tile_pool(name="const", bufs=1))
        idxp = ctx.enter_context(tc.tile_pool(name="idxp", bufs=1))

        ident = const.tile([128, 128], F32, tag="ident")
        make_identity(nc, ident[:])
        zt = const.tile([128, 128], F32, tag="zt")
        nc.vector.memset(zt[:], 0.0)
        zth = const.tile([128, 128], F16, tag="zth")
        nc.vector.memset(zth[:], 0.0)
        epss = const.tile([128, 1], F32, tag="epss")
        nc.vector.memset(epss[:], 1e-5)

        def csb(t, p0, p1, tag, dt=F32):
            tl = const.tile([p0, p1], dt, tag=tag)
            nc.sync.dma_start(out=tl[:], in_=t[:])
            return tl

        W1s = csb(W1, D_IN, D1, "W1s")
        W2s = csb(W2, D1, D2, "W2s")
        W3s = csb(W3, D2, D3, "W3s")
        Wf1s = csb(Wf1, 64, 128, "Wf1s")
        Wf2s = csb(Wf2, 128, 64, "Wf2s")
        b1s = csb(b1r, 128, D1, "b1s")
        b2s = csb(b2r, 128, D2, "b2s")
        b3s = csb(b3r, 128, D3, "b3s")
        bf1s = csb(bf1r, 128, 128, "bf1s")
        bf2s = csb(bf2r, 128, 64, "bf2s")
        gams = csb(gammar, 128, 64, "gams")
        bets = csb(betar, 128, 64, "bets")
        dvo = csb(dinv_own, 128, NBLK, "dvo")
        ics = csb(invc_own, 128, 1, "ics")

        idxT_s = []
        for t in range(2):
            tl = idxp.tile([128, lenT[t] // 16], I16, tag=f"it{t}")
            nc.sync.dma_start(out=tl[:], in_=[idxT0, idxT1][t][:])
            idxT_s.append(tl)
        idxA1_s = idxp.tile([128, lenA1 // 16], I16, tag="ia1")
        nc.sync.dma_start(out=idxA1_s[:], in_=idxA1[:])
        idxB1_s = idxp.tile([128, lenB1 // 16], I16, tag="ib1")
        nc.sync.dma_start(out=idxB1_s[:], in_=idxB1[:])
        idxP_s = idxp.tile([128, lenP // 16], I16, tag="ip")
        nc.sync.dma_start(out=idxP_s[:], in_=idxP[:])

        # guard rows
        for t in range(2):
            nc.scalar.dma_start(out=h1own[t][0:1, :], in_=zth[0:1, :])
            nc.scalar.dma_start(out=T1[t][0:1, :], in_=zth[0:1, :])
            nc.scalar.dma_start(out=hW3own[t][0:1, :], in_=zth[0:1, :])
            nc.scalar.dma_start(out=T3[t][0:1, :], in_=zth[0:1, :])
        nc.scalar.dma_start(out=h3ot[0:1, :], in_=zt[0:1, 0:D3])

        # ---- pair-table store sink -------------------------------------
        def make_pair_store_sink(dsts, pname):
            """Collects per-block [128, 64] f16 tiles; flushes groups of up
            to 8 blocks into the chunk pair tables. Block groups never
            straddle the chunk boundary (callers force a flush at block 25).
            """
            state = {"tile": None, "k0": None, "n": 0}
            pool = ctx.enter_context(tc.tile_pool(name=pname, bufs=2))

            def flush():
                tl, k0_, n_ = state["tile"], state["k0"], state["n"]
                if tl is None or n_ == 0:
                    return
                t = 0 if k0_ < CHUNK_BLK[1][0] else 1
                dst = dsts[t]
                pr0 = 64 * (k0_ - CHUNK_BLK[t][0])
                rows_total = min(SHARD, 128 * (k0_ + n_)) - 128 * k0_
                full = n_ - 1 if rows_total < 128 * n_ else n_
                if full > 0:
                    nc.scalar.dma_start(
                        out=dst[1 + pr0:1 + pr0 + 64 * full, :].rearrange(
                            "(j q) (par f) -> (q par) j f", q=64, par=2),
                        in_=tl[:, :full * 64].rearrange("p (j f) -> p j f", j=full))
                if full < n_:
                    rl = rows_total - 128 * full          # 106 for last block
                    npair = rl // 2
                    r0 = 1 + pr0 + 64 * full
                    nc.scalar.dma_start(
                        out=dst[r0:r0 + npair, :].rearrange(
                            "q (par f) -> (q par) f", par=2),
                        in_=tl[:rl, full * 64:(full + 1) * 64])
                state["tile"], state["k0"], state["n"] = None, None, 0

            def sink_store(k, rows, res):
                if state["tile"] is None:
                    state["tile"] = pool.tile([128, 8 * 64], F16, tag="stg",
                                              name=pname + "_stg")
                    state["k0"], state["n"] = k, 0
                j = state["n"]
                nc.scalar.activation(
                    state["tile"][:, j * 64:(j + 1) * 64], res[:], AF.Copy)
                state["n"] += 1
                if state["n"] == 8 or k == CHUNK_BLK[0][1] - 1:
                    flush()

            return sink_store, flush

        # ---- plain-row store sink (h3ot) -------------------------------
        def make_store_sink(dst, fout, dt, pname, row_off=0):
            state = {"tile": None, "k0": None, "n": 0}
            pool = ctx.enter_context(tc.tile_pool(name=pname, bufs=2))

            def flush():
                t, k0_, n_ = state["tile"], state["k0"], state["n"]
                if t is None or n_ == 0:
                    return
                row0 = row_off + 128 * k0_
                nrows = min(SHARD, 128 * (k0_ + n_)) - 128 * k0_
                full = n_ - 1 if nrows < 128 * n_ else n_
                if full > 0:
                    nc.scalar.dma_start(
                        out=dst[row0:row0 + 128 * full, :].rearrange(
                            "(j p) f -> p j f", p=128),
                        in_=t[:, :full * fout].rearrange("p (j f) -> p j f", j=full))
                if full < n_:
                    rl = nrows - 128 * full
                    nc.scalar.dma_start(
                        out=dst[row0 + 128 * full:row0 + nrows, :],
                        in_=t[:rl, full * fout:(full + 1) * fout])
                state["tile"], state["k0"], state["n"] = None, None, 0

            def sink_store(k, rows, res):
                if state["tile"] is None:
                    state["tile"] = pool.tile([128, 8 * fout], dt, tag="stg",
                                              name=pname + "_stg")
                    state["k0"], state["n"] = k, 0
                j = state["n"]
                nc.scalar.activation(
                    state["tile"][:, j * fout:(j + 1) * fout], res[:], AF.Copy)
                state["n"] += 1
                if state["n"] == 8:
                    flush()

            return sink_store, flush

        # ---- L1 gather driver (A1/B1 f32 windows, like v2 main pass) ----
        def fold_plain(gt, elem, c0, s, upto=1):
            while s > upto:
                h = min(s // 2, s - upto)
                nc.vector.tensor_tensor(
                    out=gt[:, c0 * elem:(c0 + h) * elem],
                    in0=gt[:, c0 * elem:(c0 + h) * elem],
                    in1=gt[:, (c0 + s - h) * elem:(c0 + s) * elem],
                    op=OP.add)
                s -= h

        def l1_layer(sink, post_block=None):
            elem = 64
            with tc.tile_pool(name="gta", bufs=3) as gta_pool, \
                 tc.tile_pool(name="gtb", bufs=3) as gtb_pool, \
                 tc.tile_pool(name="hacc", bufs=3) as hacc_pool:
                offA = offB = 0
                for (k0, k1) in supers1:
                    nA = sum(SA1[k0:k1])
                    nB = sum(SB1[k0:k1])
                    gtA = gta_pool.tile([128, SUPER_COLS * elem], F32, tag="gtA",
                                        name="gtA")
                    done = 0
                    while done < nA:
                        w = min(GW, nA - done)
                        nc.gpsimd.dma_gather(
                            out_ap=gtA[:, done * elem:(done + w) * elem].rearrange(
                                "p (c f) -> p c f", c=w),
                            in_ap=xdp[:],
                            idxs_ap=idxA1_s[:, 8 * (offA + done):8 * (offA + done + w)],
                            num_idxs=128 * w, num_idxs_reg=128 * w,
                            elem_size=elem)
                        done += w
                    gtB = gtb_pool.tile([128, SUPER_COLS * elem], F32, tag="gtB",
                                        name="gtB")
                    done = 0
                    while done < nB:
                        w = min(GW, nB - done)
                        nc.gpsimd.dma_gather(
                            out_ap=gtB[:, done * elem:(done + w) * elem].rearrange(
                                "p (c f) -> p c f", c=w),
                            in_ap=xdp[BASE_B:, :],
                            idxs_ap=idxB1_s[:, 8 * (offB + done):8 * (offB + done + w)],
                            num_idxs=128 * w, num_idxs_reg=128 * w,
                            elem_size=elem)
                        done += w
                    cA = cB = 0
                    for k in range(k0, k1):
                        sa, sb_ = SA1[k], SB1[k]
                        rows = min(128, SHARD - 128 * k)
                        acc = hacc_pool.tile([128, elem], F32, tag="acc")
                        fold_plain(gtA, elem, cA, sa, upto=2)
                        if sa >= 2:
                            nc.vector.tensor_tensor(
                                out=acc[:],
                                in0=gtA[:, cA * elem:(cA + 1) * elem],
                                in1=gtA[:, (cA + 1) * elem:(cA + 2) * elem],
                                op=OP.add)
                        else:
                            nc.vector.tensor_copy(
                                out=acc[:], in_=gtA[:, cA * elem:(cA + 1) * elem])
                        fold_plain(gtB, elem, cB, sb_, upto=1)
                        nc.vector.tensor_tensor(
                            out=acc[:], in0=acc[:],
                            in1=gtB[:, cB * elem:(cB + 1) * elem], op=OP.add)
                        cA += sa
                        cB += sb_
                        sink(k, rows, acc)
                        if post_block is not None:
                            post_block(k)
                    offA += nA
                    offB += nB

        # ---- pair-table gather layer (two passes over chunk tables) ----
        def pair_layer(tabs, idx_tiles, sink, post_block=None):
            """tabs: [T_0, T_1] dram pair tables; sink(k, rows, acc_f32_64)."""
            with tc.tile_pool(name="accv", bufs=1) as accv_pool, \
                 tc.tile_pool(name="hacc", bufs=3) as hacc_pool:
                accV = accv_pool.tile([128, NBLK * 64], F32, tag="accV")
                for t in range(2):
                    with tc.tile_pool(name=f"gt{t}", bufs=3) as gt_pool:
                        off = 0
                        for (k0, k1) in supersT[t]:
                            ncols = sum(ST[t][k0:k1])
                            gt = gt_pool.tile([128, SUPER_COLS * 128], F16,
                                              tag="gt", name=f"gt{t}")
                            done = 0
                            while done < ncols:
                                w = min(GW, ncols - done)
                                nc.gpsimd.dma_gather(
                                    out_ap=gt[:, done * 128:(done + w) * 128].rearrange(
                                        "p (c f) -> p c f", c=w),
                                    in_ap=tabs[t][:],
                                    idxs_ap=idx_tiles[t][:, 8 * (off + done):8 * (off + done + w)],
                                    num_idxs=128 * w, num_idxs_reg=128 * w,
                                    elem_size=128)
                                done += w
                            c0 = 0
                            for k in range(k0, k1):
                                nE, nO = SE[t][k], SO[t][k]
                                rows = min(128, SHARD - 128 * k)
                                g3 = gt[:].rearrange("p (c f) -> p c f", f=128)

                                def half(base, n, h0):
                                    return g3[:, base:base + n, h0:h0 + 64]

                                # fold E cols down to 1 (tree)
                                def fold_half(base, n, h0):
                                    while n > 1:
                                        h = n // 2
                                        nc.vector.tensor_tensor(
                                            out=half(base, h, h0),
                                            in0=half(base, h, h0),
                                            in1=half(base + n - h, h, h0),
                                            op=OP.add)
                                        n -= h
                                fold_half(c0, nE, 0)
                                fold_half(c0 + nE, nO, 64)
                                if t == 0:
                                    nc.vector.tensor_tensor(
                                        out=accV[:, k * 64:(k + 1) * 64],
                                        in0=half(c0, 1, 0),
                                        in1=half(c0 + nE, 1, 64), op=OP.add)
                                else:
                                    acc = hacc_pool.tile([128, 64], F32, tag="acc")
                                    nc.vector.tensor_tensor(
                                        out=acc[:], in0=half(c0, 1, 0),
                                        in1=half(c0 + nE, 1, 64), op=OP.add)
                                    nc.vector.tensor_tensor(
                                        out=acc[:], in0=acc[:],
                                        in1=accV[:, k * 64:(k + 1) * 64], op=OP.add)
                                    sink(k, rows, acc)
                                    if post_block is not None:
                                        post_block(k)
                                c0 += nE + nO
                            off += ncols

        # ================= L1 =================
        store1, flush1 = make_pair_store_sink(h1own, "st1")
        with tc.tile_pool(name="tp1", bufs=2, space="PSUM") as tp1, \
             tc.tile_pool(name="mm0", bufs=2, space="PSUM") as mm0, \
             tc.tile_pool(name="hT1", bufs=2) as hT1p, \
             tc.tile_pool(name="h1b", bufs=2) as h1bp:
            def sink1(k, rows, agg):
                # h1 = relu((dinv_i * agg) @ W1 + b1); table = dinv_i * h1
                nc.scalar.activation(agg[:], agg[:], AF.Copy,
                                     scale=dvo[:, k:k + 1])
                tp0 = tp1.tile([64, 128], F32, tag="tp0", name="tp0")
                nc.tensor.transpose(out=tp0[:], in_=agg[:], identity=ident[:])
                aT = hT1p.tile([64, 128], F32, tag="aT", name="aT")
                nc.scalar.activation(aT[:], tp0[:], AF.Copy)
                ps0 = mm0.tile([128, D1], F32, tag="ps0", name="ps0")
                nc.tensor.matmul(out=ps0[:], lhsT=aT[:D_IN, :], rhs=W1s[:],
                                 start=True, stop=True)
                h = h1bp.tile([128, D1], F32, tag="h1t", name="h1t")
                nc.vector.tensor_tensor(out=h[:], in0=ps0[:], in1=b1s[:], op=OP.add)
                nc.scalar.activation(h[:], h[:], AF.Relu)
                nc.vector.tensor_tensor(
                    out=h[:], in0=h[:],
                    in1=dvo[:, k:k + 1].to_broadcast([128, D1]), op=OP.mult)
                store1(k, rows, h)

            def post1(k):
                if k == CHUNK_BLK[0][1] - 1:
                    flush1()
                    nc.gpsimd.collective_compute(
                        "AllGather", OP.bypass, replica_groups=rg,
                        ins=[h1own[0][1:, :]], outs=[T1[0][1:, :]])
            l1_layer(sink1, post_block=post1)
            flush1()
        nc.gpsimd.collective_compute(
            "AllGather", OP.bypass, replica_groups=rg,
            ins=[h1own[1][1:, :]], outs=[T1[1][1:, :]])

        # ================= L2 =================
        store2, flush2 = make_pair_store_sink(hW3own, "st2")
        with tc.tile_pool(name="tp2", bufs=2, space="PSUM") as tp2, \
             tc.tile_pool(name="mm2", bufs=2, space="PSUM") as mm2, \
             tc.tile_pool(name="mm3", bufs=2, space="PSUM") as mm3, \
             tc.tile_pool(name="hT2", bufs=2) as hT2p, \
             tc.tile_pool(name="h2b", bufs=2) as h2bp:
            def sink2(k, rows, acc):
                # h2 = relu((dinv_i*acc) @ W2 + b2); store (dinv_i*h2)@W3 fp16
                nc.scalar.activation(acc[:], acc[:], AF.Copy,
                                     scale=dvo[:, k:k + 1])
                tp0 = tp2.tile([64, 128], F32, tag="tp0", name="tp0")
                nc.tensor.transpose(out=tp0[:], in_=acc[:], identity=ident[:])
                aT = hT2p.tile([64, 128], F32, tag="aT", name="aT")
                nc.scalar.activation(aT[:], tp0[:], AF.Copy)
                ps = mm2.tile([128, D2], F32, tag="ps", name="ps")
                nc.tensor.matmul(out=ps[:], lhsT=aT[:], rhs=W2s[:],
                                 start=True, stop=True)
                h = h2bp.tile([128, D2], F32, tag="h2t", name="h2t")
                nc.vector.tensor_tensor(out=h[:], in0=ps[:], in1=b2s[:], op=OP.add)
                nc.scalar.activation(h[:], h[:], AF.Relu)
                nc.vector.tensor_tensor(
                    out=h[:], in0=h[:],
                    in1=dvo[:, k:k + 1].to_broadcast([128, D2]), op=OP.mult)
                tp = tp2.tile([D2, 128], F32, tag="tp", name="tp")
                nc.tensor.transpose(out=tp[:], in_=h[:], identity=ident[:])
                hT = hT2p.tile([D2, 128], F32, tag="hT", name="hT")
                nc.vector.tensor_copy(out=hT[:], in_=tp[:])
                ps3 = mm3.tile([128, D3], F32, tag="ps3", name="ps3")
                nc.tensor.matmul(out=ps3[:], lhsT=hT[:], rhs=W3s[:],
                                 start=True, stop=True)
                store2(k, rows, ps3)

            def post2(k):
                if k == CHUNK_BLK[0][1] - 1:
                    flush2()
                    nc.gpsimd.collective_compute(
                        "AllGather", OP.bypass, replica_groups=rg,
                        ins=[hW3own[0][1:, :]], outs=[T3[0][1:, :]])
            pair_layer(T1, idxT_s, sink2, post_block=post2)
            flush2()
        nc.gpsimd.collective_compute(
            "AllGather", OP.bypass, replica_groups=rg,
            ins=[hW3own[1][1:, :]], outs=[T3[1][1:, :]])

        # ================= L3 =================
        storeh3, flushh3 = make_store_sink(h3ot, D3, F32, "sth3", row_off=1)

        def sink3(k, rows, acc):
            nc.scalar.activation(acc[:], acc[:], AF.Copy,
                                 scale=dvo[:, k:k + 1])
            nc.vector.tensor_tensor(out=acc[:], in0=acc[:], in1=b3s[:], op=OP.add)
            nc.scalar.activation(acc[:], acc[:], AF.Relu)
            storeh3(k, rows, acc)
        pair_layer(T3, idxT_s, sink3)
        flushh3()

        # ================= pooling =================
        with tc.tile_pool(name="gp", bufs=1) as gp, \
             tc.tile_pool(name="pstg", bufs=1) as pstg:
            stg = pstg.tile([128, GBLK * 64], F32, tag="pstg")
            spT = sum(SPu)
            gt = gp.tile([128, spT * 64], F32, tag="gtp")
            done = 0
            while done < spT:
                w = min(GW, spT - done)
                nc.gpsimd.dma_gather(
                    out_ap=gt[:, done * 64:(done + w) * 64].rearrange(
                        "p (c f) -> p c f", c=w),
                    in_ap=h3ot[:],
                    idxs_ap=idxP_s[:, 8 * done:8 * (done + w)],
                    num_idxs=128 * w, num_idxs_reg=128 * w, elem_size=64)
                done += w
            offP = 0
            for gb in range(GBLK):
                sp = SPu[gb]
                fold_plain(gt, 64, offP, sp, upto=2)
                nc.vector.tensor_tensor(
                    out=stg[:, 64 * gb:64 * (gb + 1)],
                    in0=gt[:, offP * 64:(offP + 1) * 64],
                    in1=gt[:, (offP + 1) * 64:(offP + 2) * 64], op=OP.add)
                offP += sp
            nc.scalar.dma_start(
                out=poolin[:, :].rearrange("(j p) f -> p j f", p=GPER),
                in_=stg[:GPER, :].rearrange("p (j f) -> p j f", j=GBLK))

        nc.gpsimd.collective_compute(
            "ReduceScatter", OP.add, replica_groups=rg,
            ins=[poolin[:]], outs=[rs_out[:]])

        # ================= MLP + LayerNorm =================
        with tc.tile_pool(name="mlp", bufs=1) as mlp, \
             tc.tile_pool(name="mps", bufs=2, space="PSUM") as mps:
            gtl = mlp.tile([128, 64], F32, tag="g0")
            nc.vector.memset(gtl[:], 0.0)
            nc.sync.dma_start(out=gtl[:GPER, :], in_=rs_out[:])
            nc.vector.tensor_tensor(
                out=gtl[:], in0=gtl[:],
                in1=ics[:].to_broadcast([128, 64]), op=OP.mult)
            tp = mps.tile([64, 128], F32, tag="t1")
            nc.tensor.transpose(out=tp[:], in_=gtl[:], identity=ident[:])
            gT = mlp.tile([64, 128], F32, tag="gT")
            nc.vector.tensor_copy(out=gT[:], in_=tp[:])
            p1 = mps.tile([128, 128], F32, tag="p1")
            nc.tensor.matmul(out=p1[:], lhsT=gT[:], rhs=Wf1s[:],
                             start=True, stop=True)
            g1t = mlp.tile([128, 128], F32, tag="g1t")
            nc.vector.tensor_tensor(out=g1t[:], in0=p1[:], in1=bf1s[:], op=OP.add)
            nc.scalar.activation(g1t[:], g1t[:], AF.Relu)
            tp2_ = mps.tile([128, 128], F32, tag="t2")
            nc.tensor.transpose(out=tp2_[:], in_=g1t[:], identity=ident[:])
            g1T = mlp.tile([128, 128], F32, tag="g1T")
            nc.vector.tensor_copy(out=g1T[:], in_=tp2_[:])
            p2 = mps.tile([128, 64], F32, tag="p2")
            nc.tensor.matmul(out=p2[:], lhsT=g1T[:], rhs=Wf2s[:],
                             start=True, stop=True)
            g2t = mlp.tile([128, 64], F32, tag="g2t")
            nc.vector.tensor_tensor(out=g2t[:], in0=p2[:], in1=bf2s[:], op=OP.add)
            nc.scalar.activation(g2t[:], g2t[:], AF.Relu)
            mu = mlp.tile([128, 1], F32, tag="mu")
            nc.vector.reduce_sum(mu[:], g2t[:], axis=AX.X)
            nc.vector.tensor_scalar_mul(mu[:], in0=mu[:], scalar1=1.0 / 64)
            xm = mlp.tile([128, 64], F32, tag="xm")
            nc.vector.tensor_tensor(out=xm[:], in0=g2t[:],
                                    in1=mu[:].to_broadcast([128, 64]),
                                    op=OP.subtract)
            sq = mlp.tile([128, 64], F32, tag="sq")
            nc.vector.tensor_tensor(out=sq[:], in0=xm[:], in1=xm[:], op=OP.mult)
            var = mlp.tile([128, 1], F32, tag="var")
            nc.vector.reduce_sum(var[:], sq[:], axis=AX.X)
            rstd = mlp.tile([128, 1], F32, tag="rstd")
            nc.vector.tensor_scalar_mul(var[:], in0=var[:], scalar1=1.0 / 64)
            nc.vector.tensor_tensor(out=var[:], in0=var[:], in1=epss[:],
                                    op=OP.add)
            nc.scalar.activation(rstd[:], var[:], AF.Sqrt)
            nc.vector.reciprocal(rstd[:], rstd[:])
            nc.vector.tensor_tensor(out=xm[:], in0=xm[:],
                                    in1=rstd[:].to_broadcast([128, 64]),
                                    op=OP.mult)
            nc.vector.tensor_tensor(out=xm[:], in0=xm[:], in1=gams[:], op=OP.mult)
            nc.vector.tensor_tensor(out=xm[:], in0=xm[:], in1=bets[:], op=OP.add)
            nc.sync.dma_start(out=out_slice[:, :], in_=xm[:GPER, :])

        if debug:
            with tc.tile_pool(name="dbg", bufs=2) as dp:
                def dump(srct, dstt, nrows, width, dt=F32):
                    for c in range((nrows + 127) // 128):
                        rows = min(128, nrows - 128 * c)
                        tl = dp.tile([128, width], dt, tag="dt")
                        nc.sync.dma_start(out=tl[:rows, :],
                                          in_=srct[128 * c:128 * c + rows, :])
                        nc.sync.dma_start(out=dstt[128 * c:128 * c + rows, :],
                                          in_=tl[:rows, :])
                dump(h1own[0], dbg["d_h1own0"], 1 + NPAIRS[0], 128, F16)
                dump(h1own[1], dbg["d_h1own1"], 1 + NPAIRS[1], 128, F16)
                dump(T1[0], dbg["d_T1_0"], 1 + NC * NPAIRS[0], 128, F16)
                dump(hW3own[0], dbg["d_hW3own0"], 1 + NPAIRS[0], 128, F16)
                dump(h3ot[1:, :], dbg["d_h3"], SHARD, D3)
                dump(poolin, dbg["d_poolin"], G, 64)
                dump(rs_out, dbg["d_rs"], GPER, 64)

    nc.compile()
    nc.finalize()
    return nc


# ==== SPMD runner (same as v2) ====
import jax
from jax.sharding import Mesh, PartitionSpec
from jax.experimental.shard_map import shard_map

from concourse import bass2jax


class SpmdRunner:
    def __init__(self, nc, n_cores=8):
        bass2jax.install_neuronx_cc_hook()
        self.nc = nc
        self.n_cores = n_cores
        partition_name = nc.partition_id_tensor.name if nc.partition_id_tensor else None
        in_names, out_names, out_avals, zero_outs = [], [], [], []
        for alloc in nc.m.functions[0].allocations:
            if not isinstance(alloc, mybir.MemoryLocationSet):
                continue
            name = alloc.memorylocations[0].name
            if alloc.kind == "ExternalInput":
                if name != partition_name:
                    in_names.append(name)
            elif alloc.kind == "ExternalOutput":
                shape = tuple(alloc.tensor_shape)
                dtype = mybir.dt.np(alloc.dtype)
                out_names.append(name)
                out_avals.append(jax.core.ShapedArray(shape, dtype))
                zero_outs.append(np.zeros(shape, dtype))
        self.in_names = list(in_names)
        self.out_names = out_names
        self.out_avals = out_avals
        self.zero_outs = zero_outs
        n_params = len(in_names)
        n_outs = len(out_avals)
        all_in_names = in_names + out_names + ([partition_name] if partition_name else [])
        self.n_params = n_params

        def _body(*args):
            operands = list(args)
            if partition_name is not None:
                operands.append(bass2jax.partition_id_tensor())
            outs = bass2jax._bass_exec_p.bind(
                *operands,
                out_avals=tuple(out_avals),
                in_names=tuple(all_in_names),
                out_names=tuple(out_names),
                lowering_input_output_aliases=(),
                sim_require_finite=True,
                sim_require_nnan=True,
                nc=nc,
            )
            return tuple(outs)

        try:
            devices = jax.devices("axon")[:n_cores]
        except RuntimeError:
            devices = jax.devices()[:n_cores]
        mesh = Mesh(np.asarray(devices), ("core",))
        in_specs = (PartitionSpec("core"),) * (n_params + n_outs)
        out_specs = (PartitionSpec("core"),) * n_outs
        self.fn = jax.jit(
            shard_map(_body, mesh=mesh, in_specs=in_specs, out_specs=out_specs,
                      check_rep=False),
            keep_unused=True,
        )

    def stage(self, in_maps):
        concat = [
            np.concatenate([np.asarray(in_maps[c][n]) for c in range(self.n_cores)], axis=0)
            for n in self.in_names
        ]
        zeros = [np.zeros((self.n_cores * z.shape[0], *z.shape[1:]), z.dtype)
                 for z in self.zero_outs]
        return concat + zeros

    def run(self, staged):
        out = self.fn(*staged)
        jax.block_until_ready(out)
        return out

    def unpack(self, out_arrs):
        return [
            {
                name: np.asarray(out_arrs[i]).reshape(
                    self.n_cores, *self.out_avals[i].shape)[c]
                for i, name in enumerate(self.out_names)
            }
            for c in range(self.n_cores)
        ]


_CACHE = {}


def kernel(**inputs):
    inputs = {k: np.asarray(v) for k, v in inputs.items()}
    P = preprocess(inputs["edge_index"], inputs["batch"])
    key = (tuple(map(tuple, P["SE"])), tuple(map(tuple, P["SO"])),
           tuple(P["SA1"].tolist()), tuple(P["SB1"].tolist()),
           tuple(P["SPu"].tolist()))
    if key not in _CACHE:
        S = {k: P[k] for k in ("SE", "SO", "SA1", "SB1", "SPu")}
        nc = build(S, debug=False)
        _CACHE[key] = SpmdRunner(nc, 8)
    r = _CACHE[key]
    in_maps = make_in_maps(inputs, P)
    staged = r.stage(in_maps)
    res = r.unpack(r.run(staged))
    return np.ascontiguousarray(
        np.concatenate([res[c]["out_slice"] for c in range(NC)], axis=0),
        dtype=np.float32)


# revision 4
# speedup vs baseline: 1.0529x; 1.0050x over previous
"""Trainium2 Bass kernel for nn_LinkerEncoder — v3.

Structure vs v2:
- Tables between layers are fp16 PAIR-PACKED: row = two consecutive
  positions' 64-wide vectors (256B = min gather elem). AllGather ships half
  the bytes of v2 (6.4MB -> 2x ~3.2MB chunks).
- Node classes = (chunk, parity): position chunk 0 = blocks 0..24 (3200
  pos), chunk 1 = blocks 25..48 (3050). Host greedily balances classes per
  dest so the per-block per-class slot maxes stay tight.
- L2/L3 gather streams are flat per chunk: per block [E cols][O cols], one
  gather table per chunk; fold reads the wanted 64-wide half via strided
  views (parity known at build time).
- Each AllGather is split into 2 chunk collectives: chunk-0 fires mid-way
  through the producing layer and overlaps its tail; the consuming layer's
  chunk-0 pass overlaps the chunk-1 collective (pre-pass into an
  accumulator, like v2's O-pre-pass).
- L1 keeps the v2 A1/B1 int16-window streams over the f32 xdp table.
"""
import numpy as np

N = 50000
NC = 8
SHARD = N // NC          # 6250
NBLK = (SHARD + 127) // 128   # 49
BASE_B = 17234
B_PAD_IDX = N + 1 - BASE_B
CLS_OF_CORE = np.array([0, 0, 0, 1, 1, 2, 2, 2], np.int32)
G = 1000
GBLK = 8
GPER = G // GBLK

CHUNK_POS = [(0, 3200), (3200, 6250)]
CHUNK_BLK = [(0, 25), (25, 49)]
RQ = 4                       # positions packed per 256B fp8 table row
NQUADS = [800, 763]          # ceil(chunk_size / 4); last quad of chunk 1 is half
S_KEYS = ("SR", "SA1", "SB1", "SPu")


def wrap16(stream):
    L = len(stream)
    w = np.asarray(stream, np.int32).reshape(L // 16, 16).T.astype(np.int16)
    return np.tile(w, (8, 1))


def preprocess(edge_index, batch):
    src = np.asarray(edge_index[0], np.int64)
    dst = np.asarray(edge_index[1], np.int64)
    batch = np.asarray(batch, np.int64)

    indeg = np.bincount(dst, minlength=N).astype(np.int64) + 1
    dinv = 1.0 / np.sqrt(indeg.astype(np.float64))

    rank = np.argsort(-indeg, kind="stable")
    core_of = np.empty(N, np.int32)
    core_of[rank] = np.arange(N, dtype=np.int32) % NC

    order_d = np.argsort(dst, kind="stable")
    s_sorted = src[order_d]
    d_sorted = dst[order_d]
    d_starts = np.searchsorted(d_sorted, np.arange(N))
    d_ends = np.searchsorted(d_sorted, np.arange(N) + 1)

    order_s = np.argsort(src, kind="stable")
    d_by_s = dst[order_s]
    s_starts = np.searchsorted(src[order_s], np.arange(N))
    s_ends = np.searchsorted(src[order_s], np.arange(N) + 1)

    # ---- class assignment: K=8 (chunk x quad-slot), greedy balance per dest
    K = 8
    cnt = np.zeros((N, K), np.int32)
    cap = np.zeros((NC, K), np.int64)
    for t in range(2):
        size = CHUNK_POS[t][1] - CHUNK_POS[t][0]
        for r in range(RQ):
            # positions base+4q+r exist while 4q+r < size
            cap[:, RQ * t + r] = (size - r + RQ - 1) // RQ
    cls = np.empty(N, np.int32)
    outdeg = (s_ends - s_starts) + 1
    proc = np.argsort(-outdeg, kind="stable")
    BIG = 1 << 30
    for v in proc:
        c = core_of[v]
        ds = np.concatenate([d_by_s[s_starts[v]:s_ends[v]], [v]])
        loads = cnt[ds]
        mx = loads.max(axis=1, keepdims=True)
        score = (loads >= mx).sum(axis=0) * 1000 + loads.sum(axis=0)
        score = np.where(cap[c] > 0, score, BIG)
        k = int(np.argmin(score))
        cls[v] = k
        cap[c, k] -= 1
        cnt[ds, k] += 1

    # exact per-dest per-class counts (greedy's cnt drops multi-edge dups)
    cnt = np.zeros((N, K), np.int32)
    for k in range(K):
        cnt[:, k] = np.bincount(dst[cls[src] == k], minlength=N)
    cnt[np.arange(N), cls] += 1  # self loop

    # ---- L1 window balance (flat counts incl self)
    own_cls = CLS_OF_CORE[core_of]
    cls1 = CLS_OF_CORE[core_of[src]]

    def balance(sel_dst, sel_cls, extra_a, extra_l, extra_b):
        fa = np.bincount(sel_dst, weights=(sel_cls == 0).astype(np.float64),
                         minlength=N).astype(np.int64) + extra_a
        fl = np.bincount(sel_dst, weights=(sel_cls == 1).astype(np.float64),
                         minlength=N).astype(np.int64) + extra_l
        fb = np.bincount(sel_dst, weights=(sel_cls == 2).astype(np.float64),
                         minlength=N).astype(np.int64) + extra_b
        x = np.clip((fb - fa + fl + 1) // 2, 0, fl)
        return fa + x, fb + fl - x, x

    a1_cnt, b1_cnt, xflex1 = balance(
        dst, cls1, (own_cls == 0).astype(np.int64),
        (own_cls == 1).astype(np.int64), (own_cls == 2).astype(np.int64))

    # ---- position assignment
    mxK = cnt.max(axis=1)
    mx1 = np.maximum(a1_cnt, b1_cnt)
    pos_of = np.empty(N, np.int64)
    for c in range(NC):
        for t in range(2):
            base = CHUNK_POS[t][0]
            for r in range(RQ):
                k = RQ * t + r
                nodes = np.where((core_of == c) & (cls == k))[0]
                nodes = nodes[np.lexsort((mx1[nodes], mxK[nodes]))]
                pos_of[nodes] = SHARD * c + base + RQ * np.arange(len(nodes)) + r
    node_at = np.empty(N, np.int64)
    node_at[pos_of] = np.arange(N)

    local_of = pos_of % SHARD
    blk_of = local_of // 128
    SR = np.zeros((2, RQ, NBLK), np.int64)
    SA1 = np.zeros(NBLK, np.int64)
    SB1 = np.zeros(NBLK, np.int64)
    for k in range(NBLK):
        sel = blk_of == k
        for t in range(2):
            for r in range(RQ):
                SR[t, r, k] = max(int(cnt[sel, RQ * t + r].max()), 1)
        SA1[k] = max(int(a1_cnt[sel].max()), 1)
        SB1[k] = max(int(b1_cnt[sel].max()), 1)

    # ---- streams
    lenT = [int(SR[t].sum()) * 128 for t in range(2)]
    streamsT = [np.zeros((NC, lenT[t]), np.int32) for t in range(2)]
    lenA1 = int(SA1.sum()) * 128
    lenB1 = int(SB1.sum()) * 128
    streamsA1 = np.zeros((NC, lenA1), np.int32)
    streamsB1 = np.full((NC, lenB1), B_PAD_IDX, np.int32)

    chunk_of_local = (local_of >= CHUNK_POS[1][0]).astype(np.int64)
    quad_of = np.empty(N, np.int64)
    for t in range(2):
        selt = chunk_of_local == t
        quad_of[selt] = (local_of[selt] - CHUNK_POS[t][0]) // RQ
    r_of = (local_of - np.array([CHUNK_POS[t][0] for t in range(2)])[chunk_of_local]) % RQ
    trow = 1 + core_of * np.array(NQUADS)[chunk_of_local] + quad_of

    for c in range(NC):
        offT = [0, 0]
        offA1 = offB1 = 0
        for k in range(NBLK):
            lo, hi = 128 * k, min(128 * (k + 1), SHARD)
            gR = [[np.zeros((int(SR[t][r][k]), 128), np.int32) for r in range(RQ)]
                  for t in range(2)]
            gridA1 = np.zeros((int(SA1[k]), 128), np.int32)
            gridB1 = np.full((int(SB1[k]), 128), B_PAD_IDX, np.int32)
            for lane in range(hi - lo):
                d = node_at[SHARD * c + lo + lane]
                edge_srcs = s_sorted[d_starts[d]:d_ends[d]]
                srcs = np.concatenate([edge_srcs, [d]])
                fR = [[0] * RQ for _ in range(2)]
                for s in srcs:
                    t = int(chunk_of_local[s])
                    r = int(r_of[s])
                    gR[t][r][fR[t][r], lane] = trow[s]
                    fR[t][r] += 1
                aps = pos_of[srcs]
                acls = np.concatenate([CLS_OF_CORE[core_of[edge_srcs]],
                                       [CLS_OF_CORE[c]]])
                flex1 = aps[acls == 1]
                x1 = int(xflex1[d])
                pa1 = np.concatenate([aps[acls == 0], flex1[:x1]]) + 1
                pb1 = np.concatenate([aps[acls == 2], flex1[x1:]]) + 1 - BASE_B
                gridA1[: len(pa1), lane] = pa1
                gridB1[: len(pb1), lane] = pb1
            for t in range(2):
                bg = np.concatenate(gR[t], axis=0)
                streamsT[t][c, offT[t]:offT[t] + bg.size] = bg.ravel()
                offT[t] += bg.size
            streamsA1[c, offA1:offA1 + gridA1.size] = gridA1.ravel()
            streamsB1[c, offB1:offB1 + gridB1.size] = gridB1.ravel()
            offA1 += gridA1.size
            offB1 += gridB1.size

    # ---- pooling tables
    SP = np.zeros((NC, GBLK), np.int64)
    members = {}
    nodes_of_core = [node_at[SHARD * c:SHARD * (c + 1)] for c in range(NC)]
    for c in range(NC):
        g_of_local = batch[nodes_of_core[c]]
        for gb in range(GBLK):
            cnts = np.bincount(
                g_of_local[(g_of_local >= GPER * gb) & (g_of_local < GPER * (gb + 1))] - GPER * gb,
                minlength=GPER)
            SP[c, gb] = max(cnts.max(), 1)
        members[c] = g_of_local
    SPu = SP.max(axis=0)
    lenP = int(SPu.sum()) * 128
    streamsP = np.zeros((NC, lenP), np.int32)
    for c in range(NC):
        g_of_local = members[c]
        off = 0
        for gb in range(GBLK):
            grid = np.zeros((int(SPu[gb]), 128), np.int32)
            for gl in range(GPER):
                locs = np.where(g_of_local == GPER * gb + gl)[0] + 1
                grid[: len(locs), gl] = locs
            streamsP[c, off:off + grid.size] = grid.ravel()
            off += grid.size

    cnts = np.bincount(batch, minlength=G).astype(np.float64)
    inv_cnt = (1.0 / np.maximum(cnts, 1.0)).astype(np.float32)

    return dict(
        core_of=core_of, pos_of=pos_of, node_at=node_at,
        dinv=dinv.astype(np.float32),
        SR=SR, SA1=SA1, SB1=SB1, SPu=SPu,
        streamsT=streamsT, streamsA1=streamsA1, streamsB1=streamsB1,
        streamsP=streamsP, inv_cnt=inv_cnt,
    )


# ==== host IO staging ====

def make_in_maps(inputs, P):
    node_at = P["node_at"]
    dinv = P["dinv"]
    x = np.asarray(inputs["x"], np.float32)
    xd = x * dinv[:, None]
    xp = xd[node_at]
    xdp = np.zeros((N + 2, 64), np.float32)
    xdp[1:N + 1, :37] = xp

    dinv_pos = dinv[node_at].astype(np.float32)

    def rep(v, width):
        return np.tile(np.asarray(v, np.float32)[None, :], (128, 1))

    common = dict(
        xdp=xdp,
        W1=np.asarray(inputs["W1"], np.float32),
        W2=np.asarray(inputs["W2"], np.float32),
        W3=np.asarray(inputs["W3"], np.float32),
        Wf1=np.asarray(inputs["Wf1"], np.float32),
        Wf2=np.asarray(inputs["Wf2"], np.float32),
        b1r=rep(inputs["b1"], 64), b2r=rep(inputs["b2"], 128),
        b3r=rep(inputs["b3"], 64), bf1r=rep(inputs["bf1"], 128),
        bf2r=rep(inputs["bf2"], 64), gammar=rep(inputs["gamma"], 64),
        betar=rep(inputs["beta"], 64),
    )

    in_maps = []
    for c in range(NC):
        dvo = np.zeros((128, NBLK), np.float32)
        own = dinv_pos[SHARD * c:SHARD * (c + 1)]
        for k in range(NBLK):
            rows = min(128, SHARD - 128 * k)
            dvo[:rows, k] = own[128 * k:128 * k + rows]
        invc = np.zeros((128, 1), np.float32)
        invc[:GPER, 0] = P["inv_cnt"][GPER * c:GPER * (c + 1)]
        m = dict(common)
        m["dinv_own"] = dvo
        m["invc_own"] = invc
        m["idxT0"] = wrap16(P["streamsT"][0][c])
        m["idxT1"] = wrap16(P["streamsT"][1][c])
        m["idxA1"] = wrap16(P["streamsA1"][c])
        m["idxB1"] = wrap16(P["streamsB1"][c])
        m["idxP"] = wrap16(P["streamsP"][c])
        in_maps.append(m)
    return in_maps


# ==== bass kernel ====
from contextlib import ExitStack

import concourse.bass as bass
import concourse.bacc as bacc
import concourse.mybir as mybir
from concourse.tile import TileContext
from concourse.masks import make_identity

F32 = mybir.dt.float32
F16 = mybir.dt.float16
F8 = mybir.dt.float8e4
I16 = mybir.dt.int16
AX = mybir.AxisListType
AF = mybir.ActivationFunctionType
OP = mybir.AluOpType

D_IN, D1, D2, D3 = 37, 64, 128, 64
GW = 8
SUPER_COLS = 64


def make_supers(SAu, SBu, flush_blocks=()):
    """Group consecutive blocks with combined cols <= SUPER_COLS; force a
    group boundary at each block in flush_blocks."""
    supers = []
    k0 = 0
    na = nb = 0
    for k in range(NBLK):
        if k > k0 and (na + SAu[k] > SUPER_COLS or nb + SBu[k] > SUPER_COLS
                       or k - k0 >= 16 or k in flush_blocks):
            supers.append((k0, k))
            k0, na, nb = k, 0, 0
        na += SAu[k]
        nb += SBu[k]
    supers.append((k0, NBLK))
    if supers[-1][1] - supers[-1][0] > 1:
        k0, k1 = supers[-1]
        supers[-1] = (k0, k1 - 1)
        supers.append((k1 - 1, k1))
    return supers


def build(S, debug=False):
    SR = [[list(map(int, S["SR"][t][r])) for r in range(RQ)] for t in range(2)]
    SA1 = list(map(int, S["SA1"]))
    SB1 = list(map(int, S["SB1"]))
    SPu = list(map(int, S["SPu"]))
    ST = [[sum(SR[t][r][k] for r in range(RQ)) for k in range(NBLK)]
          for t in range(2)]
    lenT = [128 * sum(ST[t]) for t in range(2)]
    lenA1, lenB1, lenP = 128 * sum(SA1), 128 * sum(SB1), 128 * sum(SPu)
    supers1 = make_supers(SA1, SB1, flush_blocks=(CHUNK_BLK[1][0],))
    supersT = [make_supers(ST[t], ST[t], flush_blocks=(CHUNK_BLK[1][0],))
               for t in range(2)]

    nc = bacc.Bacc()

    # ---------------- IO ----------------
    xdp = nc.dram_tensor("xdp", [N + 2, 64], F32, kind="ExternalInput")
    dinv_own = nc.dram_tensor("dinv_own", [128, NBLK], F32, kind="ExternalInput")
    invc_own = nc.dram_tensor("invc_own", [128, 1], F32, kind="ExternalInput")
    idxT0 = nc.dram_tensor("idxT0", [128, lenT[0] // 16], I16, kind="ExternalInput")
    idxT1 = nc.dram_tensor("idxT1", [128, lenT[1] // 16], I16, kind="ExternalInput")
    idxA1 = nc.dram_tensor("idxA1", [128, lenA1 // 16], I16, kind="ExternalInput")
    idxB1 = nc.dram_tensor("idxB1", [128, lenB1 // 16], I16, kind="ExternalInput")
    idxP = nc.dram_tensor("idxP", [128, lenP // 16], I16, kind="ExternalInput")
    W1 = nc.dram_tensor("W1", [D_IN, D1], F32, kind="ExternalInput")
    W2 = nc.dram_tensor("W2", [D1, D2], F32, kind="ExternalInput")
    W3 = nc.dram_tensor("W3", [D2, D3], F32, kind="ExternalInput")
    Wf1 = nc.dram_tensor("Wf1", [64, 128], F32, kind="ExternalInput")
    Wf2 = nc.dram_tensor("Wf2", [128, 64], F32, kind="ExternalInput")
    b1r = nc.dram_tensor("b1r", [128, D1], F32, kind="ExternalInput")
    b2r = nc.dram_tensor("b2r", [128, D2], F32, kind="ExternalInput")
    b3r = nc.dram_tensor("b3r", [128, D3], F32, kind="ExternalInput")
    bf1r = nc.dram_tensor("bf1r", [128, 128], F32, kind="ExternalInput")
    bf2r = nc.dram_tensor("bf2r", [128, 64], F32, kind="ExternalInput")
    gammar = nc.dram_tensor("gammar", [128, 64], F32, kind="ExternalInput")
    betar = nc.dram_tensor("betar", [128, 64], F32, kind="ExternalInput")

    out_slice = nc.dram_tensor("out_slice", [GPER, 64], F32, kind="ExternalOutput")

    # internal DRAM: fp8 quad tables per chunk
    h1own = [nc.dram_tensor(f"h1own{t}", [1 + NQUADS[t], 256], F8)
             for t in range(2)]
    T1 = [nc.dram_tensor(f"T1_{t}", [1 + NC * NQUADS[t], 256], F8,
                         addr_space="Shared") for t in range(2)]
    hW3own = [nc.dram_tensor(f"hW3own{t}", [1 + NQUADS[t], 256], F8)
              for t in range(2)]
    T3 = [nc.dram_tensor(f"T3_{t}", [1 + NC * NQUADS[t], 256], F8,
                         addr_space="Shared") for t in range(2)]
    h3ot = nc.dram_tensor("h3ot", [1 + SHARD, D3], F32)
    poolin = nc.dram_tensor("poolin", [G, 64], F32)
    rs_out = nc.dram_tensor("rs_out", [GPER, 64], F32)

    dbg = {}
    if debug:
        for name, shape, dt in [
                ("d_h1own0", [1 + NQUADS[0], 256], F8),
                ("d_h1own1", [1 + NQUADS[1], 256], F8),
                ("d_T1_0", [1 + NC * NQUADS[0], 256], F8),
                ("d_hW3own0", [1 + NQUADS[0], 256], F8),
                ("d_h3", [SHARD, D3], F32),
                ("d_poolin", [G, 64], F32), ("d_rs", [GPER, 64], F32)]:
            dbg[name] = nc.dram_tensor(name, shape, dt, kind="ExternalOutput")

    rg = [list(range(NC))]

    with TileContext(nc) as tc, ExitStack() as ctx:
        const = ctx.enter_context(tc.tile_pool(name="const", bufs=1))
        idxp = ctx.enter_context(tc.tile_pool(name="idxp", bufs=1))

        ident = const.tile([128, 128], F32, tag="ident")
        make_identity(nc, ident[:])
        zt = const.tile([128, 128], F32, tag="zt")
        nc.vector.memset(zt[:], 0.0)
        zth = const.tile([128, 128], F16, tag="zth")
        nc.vector.memset(zth[:], 0.0)
        zf8 = const.tile([128, 256], F8, tag="zf8")
        nc.vector.memset(zf8[:], 0.0)
        epss = const.tile([128, 1], F32, tag="epss")
        nc.vector.memset(epss[:], 1e-5)

        def csb(t, p0, p1, tag, dt=F32):
            tl = const.tile([p0, p1], dt, tag=tag)
            nc.sync.dma_start(out=tl[:], in_=t[:])
            return tl

        W1s = csb(W1, D_IN, D1, "W1s")
        W2s = csb(W2, D1, D2, "W2s")
        W3s = csb(W3, D2, D3, "W3s")
        Wf1s = csb(Wf1, 64, 128, "Wf1s")
        Wf2s = csb(Wf2, 128, 64, "Wf2s")
        b1s = csb(b1r, 128, D1, "b1s")
        b2s = csb(b2r, 128, D2, "b2s")
        b3s = csb(b3r, 128, D3, "b3s")
        bf1s = csb(bf1r, 128, 128, "bf1s")
        bf2s = csb(bf2r, 128, 64, "bf2s")
        gams = csb(gammar, 128, 64, "gams")
        bets = csb(betar, 128, 64, "bets")
        dvo = csb(dinv_own, 128, NBLK, "dvo")
        ics = csb(invc_own, 128, 1, "ics")

        idxT_s = []
        for t in range(2):
            tl = idxp.tile([128, lenT[t] // 16], I16, tag=f"it{t}")
            nc.sync.dma_start(out=tl[:], in_=[idxT0, idxT1][t][:])
            idxT_s.append(tl)
        idxA1_s = idxp.tile([128, lenA1 // 16], I16, tag="ia1")
        nc.sync.dma_start(out=idxA1_s[:], in_=idxA1[:])
        idxB1_s = idxp.tile([128, lenB1 // 16], I16, tag="ib1")
        nc.sync.dma_start(out=idxB1_s[:], in_=idxB1[:])
        idxP_s = idxp.tile([128, lenP // 16], I16, tag="ip")
        nc.sync.dma_start(out=idxP_s[:], in_=idxP[:])

        # guard rows
        for t in range(2):
            nc.scalar.dma_start(out=h1own[t][0:1, :], in_=zf8[0:1, :])
            nc.scalar.dma_start(out=T1[t][0:1, :], in_=zf8[0:1, :])
            nc.scalar.dma_start(out=hW3own[t][0:1, :], in_=zf8[0:1, :])
            nc.scalar.dma_start(out=T3[t][0:1, :], in_=zf8[0:1, :])
        nc.scalar.dma_start(out=h3ot[0:1, :], in_=zt[0:1, 0:D3])

        # ---- fp8 quad-table store sink ---------------------------------
        def make_pair_store_sink(dsts, pname):
            """Collects per-block [128, 64] tiles (stored as f8); flushes
            groups of up to 8 blocks into the chunk quad tables. Block
            groups never straddle the chunk boundary."""
            state = {"tile": None, "k0": None, "n": 0}
            pool = ctx.enter_context(tc.tile_pool(name=pname, bufs=2))

            def flush():
                tl, k0_, n_ = state["tile"], state["k0"], state["n"]
                if tl is None or n_ == 0:
                    return
                t = 0 if k0_ < CHUNK_BLK[1][0] else 1
                dst = dsts[t]
                qr0 = 32 * (k0_ - CHUNK_BLK[t][0])
                rows_total = min(SHARD, 128 * (k0_ + n_)) - 128 * k0_
                full = n_ - 1 if rows_total < 128 * n_ else n_
                if full > 0:
                    nc.scalar.dma_start(
                        out=dst[1 + qr0:1 + qr0 + 32 * full, :].rearrange(
                            "(j q) (r f) -> (q r) j f", q=32, r=4),
                        in_=tl[:, :full * 64].rearrange("p (j f) -> p j f", j=full))
                if full < n_:
                    rl = rows_total - 128 * full          # 106 for last block
                    nq = rl // 4                          # 26 full quads
                    r0 = 1 + qr0 + 32 * full
                    nc.scalar.dma_start(
                        out=dst[r0:r0 + nq, :].rearrange(
                            "q (r f) -> (q r) f", r=4),
                        in_=tl[:4 * nq, full * 64:(full + 1) * 64])
                    rem = rl - 4 * nq                     # 2 leftover rows
                    if rem:
                        nc.scalar.dma_start(
                            out=dst[r0 + nq:r0 + nq + 1, 0:rem * 64].rearrange(
                                "q (r f) -> (q r) f", r=rem),
                            in_=tl[4 * nq:rl, full * 64:(full + 1) * 64])
                state["tile"], state["k0"], state["n"] = None, None, 0

            def sink_store(k, rows, res):
                if state["tile"] is None:
                    state["tile"] = pool.tile([128, 8 * 64], F8, tag="stg",
                                              name=pname + "_stg")
                    state["k0"], state["n"] = k, 0
                j = state["n"]
                nc.scalar.activation(
                    state["tile"][:, j * 64:(j + 1) * 64], res[:], AF.Copy)
                state["n"] += 1
                if state["n"] == 8 or k == CHUNK_BLK[0][1] - 1:
                    flush()

            return sink_store, flush

        # ---- plain-row store sink (h3ot) -------------------------------
        def make_store_sink(dst, fout, dt, pname, row_off=0):
            state = {"tile": None, "k0": None, "n": 0}
            pool = ctx.enter_context(tc.tile_pool(name=pname, bufs=2))

            def flush():
                t, k0_, n_ = state["tile"], state["k0"], state["n"]
                if t is None or n_ == 0:
                    return
                row0 = row_off + 128 * k0_
                nrows = min(SHARD, 128 * (k0_ + n_)) - 128 * k0_
                full = n_ - 1 if nrows < 128 * n_ else n_
                if full > 0:
                    nc.scalar.dma_start(
                        out=dst[row0:row0 + 128 * full, :].rearrange(
                            "(j p) f -> p j f", p=128),
                        in_=t[:, :full * fout].rearrange("p (j f) -> p j f", j=full))
                if full < n_:
                    rl = nrows - 128 * full
                    nc.scalar.dma_start(
                        out=dst[row0 + 128 * full:row0 + nrows, :],
                        in_=t[:rl, full * fout:(full + 1) * fout])
                state["tile"], state["k0"], state["n"] = None, None, 0

            def sink_store(k, rows, res):
                if state["tile"] is None:
                    state["tile"] = pool.tile([128, 8 * fout], dt, tag="stg",
                                              name=pname + "_stg")
                    state["k0"], state["n"] = k, 0
                j = state["n"]
                nc.scalar.activation(
                    state["tile"][:, j * fout:(j + 1) * fout], res[:], AF.Copy)
                state["n"] += 1
                if state["n"] == 8:
                    flush()

            return sink_store, flush

        # ---- L1 gather driver (A1/B1 f32 windows, like v2 main pass) ----
        def fold_plain(gt, elem, c0, s, upto=1):
            while s > upto:
                h = min(s // 2, s - upto)
                nc.vector.tensor_tensor(
                    out=gt[:, c0 * elem:(c0 + h) * elem],
                    in0=gt[:, c0 * elem:(c0 + h) * elem],
                    in1=gt[:, (c0 + s - h) * elem:(c0 + s) * elem],
                    op=OP.add)
                s -= h

        def l1_layer(sink, mid_hook=None, park_ms=0.0):
            elem = 64
            with tc.tile_pool(name="gta", bufs=3) as gta_pool, \
                 tc.tile_pool(name="gtb", bufs=3) as gtb_pool, \
                 tc.tile_pool(name="hacc", bufs=3) as hacc_pool, \
                 ExitStack() as lctx:
                offA = offB = 0
                for (k0, k1) in supers1:
                    if k0 == CHUNK_BLK[1][0] and mid_hook is not None:
                        mid_hook()
                        if park_ms > 0:
                            lctx.enter_context(tc.tile_wait_until(park_ms))
                    nA = sum(SA1[k0:k1])
                    nB = sum(SB1[k0:k1])
                    gtA = gta_pool.tile([128, SUPER_COLS * elem], F32, tag="gtA",
                                        name="gtA")
                    done = 0
                    while done < nA:
                        w = min(GW, nA - done)
                        nc.gpsimd.dma_gather(
                            out_ap=gtA[:, done * elem:(done + w) * elem].rearrange(
                                "p (c f) -> p c f", c=w),
                            in_ap=xdp[:],
                            idxs_ap=idxA1_s[:, 8 * (offA + done):8 * (offA + done + w)],
                            num_idxs=128 * w, num_idxs_reg=128 * w,
                            elem_size=elem)
                        done += w
                    gtB = gtb_pool.tile([128, SUPER_COLS * elem], F32, tag="gtB",
                                        name="gtB")
                    done = 0
                    while done < nB:
                        w = min(GW, nB - done)
                        nc.gpsimd.dma_gather(
                            out_ap=gtB[:, done * elem:(done + w) * elem].rearrange(
                                "p (c f) -> p c f", c=w),
                            in_ap=xdp[BASE_B:, :],
                            idxs_ap=idxB1_s[:, 8 * (offB + done):8 * (offB + done + w)],
                            num_idxs=128 * w, num_idxs_reg=128 * w,
                            elem_size=elem)
                        done += w
                    cA = cB = 0
                    for k in range(k0, k1):
                        sa, sb_ = SA1[k], SB1[k]
                        rows = min(128, SHARD - 128 * k)
                        acc = hacc_pool.tile([128, elem], F32, tag="acc")
                        fold_plain(gtA, elem, cA, sa, upto=2)
                        if sa >= 2:
                            nc.vector.tensor_tensor(
                                out=acc[:],
                                in0=gtA[:, cA * elem:(cA + 1) * elem],
                                in1=gtA[:, (cA + 1) * elem:(cA + 2) * elem],
                                op=OP.add)
                        else:
                            nc.vector.tensor_copy(
                                out=acc[:], in_=gtA[:, cA * elem:(cA + 1) * elem])
                        fold_plain(gtB, elem, cB, sb_, upto=1)
                        nc.vector.tensor_tensor(
                            out=acc[:], in0=acc[:],
                            in1=gtB[:, cB * elem:(cB + 1) * elem], op=OP.add)
                        cA += sa
                        cB += sb_
                        sink(k, rows, acc)
                    offA += nA
                    offB += nB

        # ---- fp8 quad-table gather layer (two passes over chunk tables) ----
        def pair_layer(tabs, idx_tiles, sink, mid_hook=None, park_ms=0.0):
            """tabs: [T_0, T_1] dram quad tables; sink(k, rows, acc_f32_64).
            Fold: sequential accumulation into an f32 target so fp8 never
            holds a partial sum."""
            with tc.tile_pool(name="accv", bufs=1) as accv_pool, \
                 tc.tile_pool(name="hacc", bufs=3) as hacc_pool:
                accV = accv_pool.tile([128, NBLK * 64], F32, tag="accV")
                for t in range(2):
                    with tc.tile_pool(name=f"gt{t}", bufs=3) as gt_pool, \
                         ExitStack() as lctx:
                        off = 0
                        for (k0, k1) in supersT[t]:
                            if (t == 1 and k0 == CHUNK_BLK[1][0]
                                    and mid_hook is not None):
                                mid_hook()
                                if park_ms > 0:
                                    lctx.enter_context(
                                        tc.tile_wait_until(park_ms))
                            ncols = sum(ST[t][k0:k1])
                            gt = gt_pool.tile([128, SUPER_COLS * 256], F8,
                                              tag="gt", name=f"gt{t}")
                            done = 0
                            while done < ncols:
                                w = min(GW, ncols - done)
                                nc.gpsimd.dma_gather(
                                    out_ap=gt[:, done * 256:(done + w) * 256].rearrange(
                                        "p (c f) -> p c f", c=w),
                                    in_ap=tabs[t][:],
                                    idxs_ap=idx_tiles[t][:, 8 * (off + done):8 * (off + done + w)],
                                    num_idxs=128 * w, num_idxs_reg=128 * w,
                                    elem_size=256)
                                done += w
                            c0 = 0
                            g3 = gt[:].rearrange("p (c f) -> p c f", f=256)
                            for k in range(k0, k1):
                                rows = min(128, SHARD - 128 * k)
                                cols = []
                                cc = c0
                                for r in range(RQ):
                                    for i in range(SR[t][r][k]):
                                        cols.append((cc + i, r * 64))
                                    cc += SR[t][r][k]
                                if t == 0:
                                    target = accV[:, k * 64:(k + 1) * 64]
                                else:
                                    acc = hacc_pool.tile([128, 64], F32, tag="acc")
                                    target = acc[:]
                                (i0, o0), (i1, o1) = cols[0], cols[1]
                                nc.vector.tensor_tensor(
                                    out=target,
                                    in0=g3[:, i0:i0 + 1, o0:o0 + 64],
                                    in1=g3[:, i1:i1 + 1, o1:o1 + 64], op=OP.add)
                                for (ci, o) in cols[2:]:
                                    nc.vector.tensor_tensor(
                                        out=target, in0=target,
                                        in1=g3[:, ci:ci + 1, o:o + 64], op=OP.add)
                                if t == 1:
                                    nc.vector.tensor_tensor(
                                        out=target, in0=target,
                                        in1=accV[:, k * 64:(k + 1) * 64], op=OP.add)
                                    sink(k, rows, acc)
                                c0 = cc
                            off += ncols

        # ================= L1 =================
        store1, flush1 = make_pair_store_sink(h1own, "st1")
        with tc.tile_pool(name="tp1", bufs=2, space="PSUM") as tp1, \
             tc.tile_pool(name="mm0", bufs=2, space="PSUM") as mm0, \
             tc.tile_pool(name="hT1", bufs=2) as hT1p, \
             tc.tile_pool(name="h1b", bufs=2) as h1bp:
            def sink1(k, rows, agg):
                # h1 = relu((dinv_i * agg) @ W1 + b1); table = dinv_i * h1
                nc.scalar.activation(agg[:], agg[:], AF.Copy,
                                     scale=dvo[:, k:k + 1])
                tp0 = tp1.tile([64, 128], F32, tag="tp0", name="tp0")
                nc.tensor.transpose(out=tp0[:], in_=agg[:], identity=ident[:])
                aT = hT1p.tile([64, 128], F32, tag="aT", name="aT")
                nc.scalar.activation(aT[:], tp0[:], AF.Copy)
                ps0 = mm0.tile([128, D1], F32, tag="ps0", name="ps0")
                nc.tensor.matmul(out=ps0[:], lhsT=aT[:D_IN, :], rhs=W1s[:],
                                 start=True, stop=True)
                h = h1bp.tile([128, D1], F32, tag="h1t", name="h1t")
                nc.vector.tensor_tensor(out=h[:], in0=ps0[:], in1=b1s[:], op=OP.add)
                nc.scalar.activation(h[:], h[:], AF.Relu)
                nc.vector.tensor_tensor(
                    out=h[:], in0=h[:],
                    in1=dvo[:, k:k + 1].to_broadcast([128, D1]), op=OP.mult)
                store1(k, rows, h)

            def mid1():
                flush1()
                with tc.high_priority():
                    nc.gpsimd.collective_compute(
                        "AllGather", OP.bypass, replica_groups=rg,
                        ins=[h1own[0][1:, :]], outs=[T1[0][1:, :]])
            l1_layer(sink1, mid_hook=mid1, park_ms=0.068)
            flush1()
        with tc.high_priority():
            nc.gpsimd.collective_compute(
                "AllGather", OP.bypass, replica_groups=rg,
                ins=[h1own[1][1:, :]], outs=[T1[1][1:, :]])

        # ================= L2 =================
        store2, flush2 = make_pair_store_sink(hW3own, "st2")
        with tc.tile_pool(name="tp2", bufs=2, space="PSUM") as tp2, \
             tc.tile_pool(name="mm2", bufs=2, space="PSUM") as mm2, \
             tc.tile_pool(name="mm3", bufs=2, space="PSUM") as mm3, \
             tc.tile_pool(name="hT2", bufs=2) as hT2p, \
             tc.tile_pool(name="h2b", bufs=2) as h2bp:
            def sink2(k, rows, acc):
                # h2 = relu((dinv_i*acc) @ W2 + b2); store (dinv_i*h2)@W3 fp16
                nc.scalar.activation(acc[:], acc[:], AF.Copy,
                                     scale=dvo[:, k:k + 1])
                tp0 = tp2.tile([64, 128], F32, tag="tp0", name="tp0")
                nc.tensor.transpose(out=tp0[:], in_=acc[:], identity=ident[:])
                aT = hT2p.tile([64, 128], F32, tag="aT", name="aT")
                nc.scalar.activation(aT[:], tp0[:], AF.Copy)
                ps = mm2.tile([128, D2], F32, tag="ps", name="ps")
                nc.tensor.matmul(out=ps[:], lhsT=aT[:], rhs=W2s[:],
                                 start=True, stop=True)
                h = h2bp.tile([128, D2], F32, tag="h2t", name="h2t")
                nc.vector.tensor_tensor(out=h[:], in0=ps[:], in1=b2s[:], op=OP.add)
                nc.scalar.activation(h[:], h[:], AF.Relu)
                nc.vector.tensor_tensor(
                    out=h[:], in0=h[:],
                    in1=dvo[:, k:k + 1].to_broadcast([128, D2]), op=OP.mult)
                tp = tp2.tile([D2, 128], F32, tag="tp", name="tp")
                nc.tensor.transpose(out=tp[:], in_=h[:], identity=ident[:])
                hT = hT2p.tile([D2, 128], F32, tag="hT", name="hT")
                nc.vector.tensor_copy(out=hT[:], in_=tp[:])
                ps3 = mm3.tile([128, D3], F32, tag="ps3", name="ps3")
                nc.tensor.matmul(out=ps3[:], lhsT=hT[:], rhs=W3s[:],
                                 start=True, stop=True)
                store2(k, rows, ps3)

            def mid2():
                flush2()
                with tc.high_priority():
                    nc.gpsimd.collective_compute(
                        "AllGather", OP.bypass, replica_groups=rg,
                        ins=[hW3own[0][1:, :]], outs=[T3[0][1:, :]])
            pair_layer(T1, idxT_s, sink2, mid_hook=mid2, park_ms=0.30)
            flush2()
        with tc.high_priority():
            nc.gpsimd.collective_compute(
                "AllGather", OP.bypass, replica_groups=rg,
                ins=[hW3own[1][1:, :]], outs=[T3[1][1:, :]])

        # ================= L3 =================
        storeh3, flushh3 = make_store_sink(h3ot, D3, F32, "sth3", row_off=1)

        def sink3(k, rows, acc):
            nc.scalar.activation(acc[:], acc[:], AF.Copy,
                                 scale=dvo[:, k:k + 1])
            nc.vector.tensor_tensor(out=acc[:], in0=acc[:], in1=b3s[:], op=OP.add)
            nc.scalar.activation(acc[:], acc[:], AF.Relu)
            storeh3(k, rows, acc)
        pair_layer(T3, idxT_s, sink3)
        flushh3()

        # ================= pooling =================
        with tc.tile_pool(name="gp", bufs=1) as gp, \
             tc.tile_pool(name="pstg", bufs=1) as pstg:
            stg = pstg.tile([128, GBLK * 64], F32, tag="pstg")
            spT = sum(SPu)
            gt = gp.tile([128, spT * 64], F32, tag="gtp")
            done = 0
            while done < spT:
                w = min(GW, spT - done)
                nc.gpsimd.dma_gather(
                    out_ap=gt[:, done * 64:(done + w) * 64].rearrange(
                        "p (c f) -> p c f", c=w),
                    in_ap=h3ot[:],
                    idxs_ap=idxP_s[:, 8 * done:8 * (done + w)],
                    num_idxs=128 * w, num_idxs_reg=128 * w, elem_size=64)
                done += w
            offP = 0
            for gb in range(GBLK):
                sp = SPu[gb]
                fold_plain(gt, 64, offP, sp, upto=2)
                nc.vector.tensor_tensor(
                    out=stg[:, 64 * gb:64 * (gb + 1)],
                    in0=gt[:, offP * 64:(offP + 1) * 64],
                    in1=gt[:, (offP + 1) * 64:(offP + 2) * 64], op=OP.add)
                offP += sp
            nc.scalar.dma_start(
                out=poolin[:, :].rearrange("(j p) f -> p j f", p=GPER),
                in_=stg[:GPER, :].rearrange("p (j f) -> p j f", j=GBLK))

        nc.gpsimd.collective_compute(
            "ReduceScatter", OP.add, replica_groups=rg,
            ins=[poolin[:]], outs=[rs_out[:]])

        # ================= MLP + LayerNorm =================
        with tc.tile_pool(name="mlp", bufs=1) as mlp, \
             tc.tile_pool(name="mps", bufs=2, space="PSUM") as mps:
            gtl = mlp.tile([128, 64], F32, tag="g0")
            nc.vector.memset(gtl[:], 0.0)
            nc.sync.dma_start(out=gtl[:GPER, :], in_=rs_out[:])
            nc.vector.tensor_tensor(
                out=gtl[:], in0=gtl[:],
                in1=ics[:].to_broadcast([128, 64]), op=OP.mult)
            tp = mps.tile([64, 128], F32, tag="t1")
            nc.tensor.transpose(out=tp[:], in_=gtl[:], identity=ident[:])
            gT = mlp.tile([64, 128], F32, tag="gT")
            nc.vector.tensor_copy(out=gT[:], in_=tp[:])
            p1 = mps.tile([128, 128], F32, tag="p1")
            nc.tensor.matmul(out=p1[:], lhsT=gT[:], rhs=Wf1s[:],
                             start=True, stop=True)
            g1t = mlp.tile([128, 128], F32, tag="g1t")
            nc.vector.tensor_tensor(out=g1t[:], in0=p1[:], in1=bf1s[:], op=OP.add)
            nc.scalar.activation(g1t[:], g1t[:], AF.Relu)
            tp2_ = mps.tile([128, 128], F32, tag="t2")
            nc.tensor.transpose(out=tp2_[:], in_=g1t[:], identity=ident[:])
            g1T = mlp.tile([128, 128], F32, tag="g1T")
            nc.vector.tensor_copy(out=g1T[:], in_=tp2_[:])
            p2 = mps.tile([128, 64], F32, tag="p2")
            nc.tensor.matmul(out=p2[:], lhsT=g1T[:], rhs=Wf2s[:],
                             start=True, stop=True)
            g2t = mlp.tile([128, 64], F32, tag="g2t")
            nc.vector.tensor_tensor(out=g2t[:], in0=p2[:], in1=bf2s[:], op=OP.add)
            nc.scalar.activation(g2t[:], g2t[:], AF.Relu)
            mu = mlp.tile([128, 1], F32, tag="mu")
            nc.vector.reduce_sum(mu[:], g2t[:], axis=AX.X)
            nc.vector.tensor_scalar_mul(mu[:], in0=mu[:], scalar1=1.0 / 64)
            xm = mlp.tile([128, 64], F32, tag="xm")
            nc.vector.tensor_tensor(out=xm[:], in0=g2t[:],
                                    in1=mu[:].to_broadcast([128, 64]),
                                    op=OP.subtract)
            sq = mlp.tile([128, 64], F32, tag="sq")
            nc.vector.tensor_tensor(out=sq[:], in0=xm[:], in1=xm[:], op=OP.mult)
            var = mlp.tile([128, 1], F32, tag="var")
            nc.vector.reduce_sum(var[:], sq[:], axis=AX.X)
            rstd = mlp.tile([128, 1], F32, tag="rstd")
            nc.vector.tensor_scalar_mul(var[:], in0=var[:], scalar1=1.0 / 64)
            nc.vector.tensor_tensor(out=var[:], in0=var[:], in1=epss[:],
                                    op=OP.add)
            nc.scalar.activation(rstd[:], var[:], AF.Sqrt)
            nc.vector.reciprocal(rstd[:], rstd[:])
            nc.vector.tensor_tensor(out=xm[:], in0=xm[:],
                                    in1=rstd[:].to_broadcast([128, 64]),
                                    op=OP.mult)
            nc.vector.tensor_tensor(out=xm[:], in0=xm[:], in1=gams[:], op=OP.mult)
            nc.vector.tensor_tensor(out=xm[:], in0=xm[:], in1=bets[:], op=OP.add)
            nc.sync.dma_start(out=out_slice[:, :], in_=xm[:GPER, :])

        if debug:
            with tc.tile_pool(name="dbg", bufs=2) as dp:
                def dump(srct, dstt, nrows, width, dt=F32):
                    for c in range((nrows + 127) // 128):
                        rows = min(128, nrows - 128 * c)
                        tl = dp.tile([128, width], dt, tag="dt")
                        nc.sync.dma_start(out=tl[:rows, :],
                                          in_=srct[128 * c:128 * c + rows, :])
                        nc.sync.dma_start(out=dstt[128 * c:128 * c + rows, :],
                                          in_=tl[:rows, :])
                dump(h1own[0], dbg["d_h1own0"], 1 + NPAIRS[0], 128, F16)
                dump(h1own[1], dbg["d_h1own1"], 1 + NPAIRS[1], 128, F16)
                dump(T1[0], dbg["d_T1_0"], 1 + NC * NPAIRS[0], 128, F16)
                dump(hW3own[0], dbg["d_hW3own0"], 1 + NPAIRS[0], 128, F16)
                dump(h3ot[1:, :], dbg["d_h3"], SHARD, D3)
                dump(poolin, dbg["d_poolin"], G, 64)
                dump(rs_out, dbg["d_rs"], GPER, 64)

    nc.compile()
    nc.finalize()
    return nc


# ==== SPMD runner (same as v2) ====
import jax
from jax.sharding import Mesh, PartitionSpec
from jax.experimental.shard_map import shard_map

from concourse import bass2jax


class SpmdRunner:
    def __init__(self, nc, n_cores=8):
        bass2jax.install_neuronx_cc_hook()
        self.nc = nc
        self.n_cores = n_cores
        partition_name = nc.partition_id_tensor.name if nc.partition_id_tensor else None
        in_names, out_names, out_avals, zero_outs = [], [], [], []
        for alloc in nc.m.functions[0].allocations:
            if not isinstance(alloc, mybir.MemoryLocationSet):
                continue
            name = alloc.memorylocations[0].name
            if alloc.kind == "ExternalInput":
                if name != partition_name:
                    in_names.append(name)
            elif alloc.kind == "ExternalOutput":
                shape = tuple(alloc.tensor_shape)
                dtype = mybir.dt.np(alloc.dtype)
                out_names.append(name)
                out_avals.append(jax.core.ShapedArray(shape, dtype))
                zero_outs.append(np.zeros(shape, dtype))
        self.in_names = list(in_names)
        self.out_names = out_names
        self.out_avals = out_avals
        self.zero_outs = zero_outs
        n_params = len(in_names)
        n_outs = len(out_avals)
        all_in_names = in_names + out_names + ([partition_name] if partition_name else [])
        self.n_params = n_params

        def _body(*args):
            operands = list(args)
            if partition_name is not None:
                operands.append(bass2jax.partition_id_tensor())
            outs = bass2jax._bass_exec_p.bind(
                *operands,
                out_avals=tuple(out_avals),
                in_names=tuple(all_in_names),
                out_names=tuple(out_names),
                lowering_input_output_aliases=(),
                sim_require_finite=True,
                sim_require_nnan=True,
                nc=nc,
            )
            return tuple(outs)

        try:
            devices = jax.devices("axon")[:n_cores]
        except RuntimeError:
            devices = jax.devices()[:n_cores]
        mesh = Mesh(np.asarray(devices), ("core",))
        in_specs = (PartitionSpec("core"),) * (n_params + n_outs)
        out_specs = (PartitionSpec("core"),) * n_outs
        self.fn = jax.jit(
            shard_map(_body, mesh=mesh, in_specs=in_specs, out_specs=out_specs,
                      check_rep=False),
            keep_unused=True,
        )

    def stage(self, in_maps):
        concat = [
            np.concatenate([np.asarray(in_maps[c][n]) for c in range(self.n_cores)], axis=0)
            for n in self.in_names
        ]
        zeros = [np.zeros((self.n_cores * z.shape[0], *z.shape[1:]), z.dtype)
                 for z in self.zero_outs]
        return concat + zeros

    def run(self, staged):
        out = self.fn(*staged)
        jax.block_until_ready(out)
        return out

    def unpack(self, out_arrs):
        return [
            {
                name: np.asarray(out_arrs[i]).reshape(
                    self.n_cores, *self.out_avals[i].shape)[c]
                for i, name in enumerate(self.out_names)
            }
            for c in range(self.n_cores)
        ]


_CACHE = {}


def kernel(**inputs):
    inputs = {k: np.asarray(v) for k, v in inputs.items()}
    P = preprocess(inputs["edge_index"], inputs["batch"])
    key = (tuple(map(tuple, P["SR"].reshape(2 * RQ, -1))),
           tuple(P["SA1"].tolist()), tuple(P["SB1"].tolist()),
           tuple(P["SPu"].tolist()))
    if key not in _CACHE:
        S = {k: P[k] for k in S_KEYS}
        nc = build(S, debug=False)
        _CACHE[key] = SpmdRunner(nc, 8)
    r = _CACHE[key]
    in_maps = make_in_maps(inputs, P)
    staged = r.stage(in_maps)
    res = r.unpack(r.run(staged))
    return np.ascontiguousarray(
        np.concatenate([res[c]["out_slice"] for c in range(NC)], axis=0),
        dtype=np.float32)


# revision 6
# speedup vs baseline: 1.0712x; 1.0173x over previous
"""Trainium2 Bass kernel for nn_LinkerEncoder (3-layer GCN + mean-pool +
MLP + LayerNorm), SPMD over 8 NeuronCores.

Structure (v4: fp8 quad tables + chunked overlapped AllGathers):
- Inter-layer tables (dinv-scaled h1, (dinv*h2)@W3) are fp8-e4m3, QUAD-
  PACKED: one 256B table row holds four consecutive positions' 64-wide
  vectors (256B = the dma_gather minimum element). Folds select the wanted
  64-wide slice via strided views and accumulate in f32; end-to-end rel
  err 4.5e-4 (gate is 2e-2).
- Node classes = (position chunk x quad slot), K=8: chunk 0 = blocks 0..24
  (3200 positions), chunk 1 = blocks 25..48 (3050). The host greedily
  assigns each node a class to balance per-dest class counts, then
  count-sorts within (core, class) so per-block slot maxes stay tight.
- Each AllGather is split into two chunk collectives (~1.6MB each): the
  chunk-0 collective fires right after the producing layer sinks block 24
  (tc.tile_wait_until parks the rest of the layer in the scheduler's
  virtual clock so the collective lands early in the Pool stream); the
  consuming layer's chunk-0 pass (accumulated into accV) overlaps the
  chunk-1 collective.
- L1 gathers the f32 xdp table (x pre-scaled by dinv, 37->64 padded) via
  the two int16 index windows (A/B) and applies W1 after aggregation.
- Sinks fuse the dest dinv scale through the matmuls (per-row scale
  commutes) and into activation scale (relu(s*x)=s*relu(x), s>0); the f8
  downcast store is fused into the final activation.
- Pooling tail: local pool-gather of h3 -> per-graph partial sums ->
  ReduceScatter(add) -> per-core 125-graph MLP + LayerNorm -> host
  concatenates the 8 output slices.
"""
import numpy as np

N = 50000
NC = 8
SHARD = N // NC          # 6250
NBLK = (SHARD + 127) // 128   # 49
BASE_B = 17234
B_PAD_IDX = N + 1 - BASE_B
CLS_OF_CORE = np.array([0, 0, 0, 1, 1, 2, 2, 2], np.int32)
G = 1000
GBLK = 8
GPER = G // GBLK

CHUNK_POS = [(0, 3200), (3200, 6250)]
CHUNK_BLK = [(0, 25), (25, 49)]
RQ = 4                       # positions packed per 256B fp8 table row
NQUADS = [800, 763]          # ceil(chunk_size / 4); last quad of chunk 1 is half
S_KEYS = ("SR", "SA1", "SB1", "SPu")


def wrap16(stream):
    L = len(stream)
    w = np.asarray(stream, np.int32).reshape(L // 16, 16).T.astype(np.int16)
    return np.tile(w, (8, 1))


def preprocess(edge_index, batch):
    src = np.asarray(edge_index[0], np.int64)
    dst = np.asarray(edge_index[1], np.int64)
    batch = np.asarray(batch, np.int64)

    indeg = np.bincount(dst, minlength=N).astype(np.int64) + 1
    dinv = 1.0 / np.sqrt(indeg.astype(np.float64))

    rank = np.argsort(-indeg, kind="stable")
    core_of = np.empty(N, np.int32)
    core_of[rank] = np.arange(N, dtype=np.int32) % NC

    order_d = np.argsort(dst, kind="stable")
    s_sorted = src[order_d]
    d_sorted = dst[order_d]
    d_starts = np.searchsorted(d_sorted, np.arange(N))
    d_ends = np.searchsorted(d_sorted, np.arange(N) + 1)

    order_s = np.argsort(src, kind="stable")
    d_by_s = dst[order_s]
    s_starts = np.searchsorted(src[order_s], np.arange(N))
    s_ends = np.searchsorted(src[order_s], np.arange(N) + 1)

    # ---- class assignment: K=8 (chunk x quad-slot), greedy balance per dest
    K = 8
    cnt = np.zeros((N, K), np.int32)
    cap = np.zeros((NC, K), np.int64)
    for t in range(2):
        size = CHUNK_POS[t][1] - CHUNK_POS[t][0]
        for r in range(RQ):
            # positions base+4q+r exist while 4q+r < size
            cap[:, RQ * t + r] = (size - r + RQ - 1) // RQ
    cls = np.empty(N, np.int32)
    outdeg = (s_ends - s_starts) + 1
    proc = np.argsort(-outdeg, kind="stable")
    BIG = 1 << 30
    for v in proc:
        c = core_of[v]
        ds = np.concatenate([d_by_s[s_starts[v]:s_ends[v]], [v]])
        loads = cnt[ds]
        mx = loads.max(axis=1, keepdims=True)
        score = (loads >= mx).sum(axis=0) * 1000 + loads.sum(axis=0)
        score = np.where(cap[c] > 0, score, BIG)
        k = int(np.argmin(score))
        cls[v] = k
        cap[c, k] -= 1
        cnt[ds, k] += 1

    # exact per-dest per-class counts (greedy's cnt drops multi-edge dups)
    cnt = np.zeros((N, K), np.int32)
    for k in range(K):
        cnt[:, k] = np.bincount(dst[cls[src] == k], minlength=N)
    cnt[np.arange(N), cls] += 1  # self loop

    # ---- L1 window balance (flat counts incl self)
    own_cls = CLS_OF_CORE[core_of]
    cls1 = CLS_OF_CORE[core_of[src]]

    def balance(sel_dst, sel_cls, extra_a, extra_l, extra_b):
        fa = np.bincount(sel_dst, weights=(sel_cls == 0).astype(np.float64),
                         minlength=N).astype(np.int64) + extra_a
        fl = np.bincount(sel_dst, weights=(sel_cls == 1).astype(np.float64),
                         minlength=N).astype(np.int64) + extra_l
        fb = np.bincount(sel_dst, weights=(sel_cls == 2).astype(np.float64),
                         minlength=N).astype(np.int64) + extra_b
        x = np.clip((fb - fa + fl + 1) // 2, 0, fl)
        return fa + x, fb + fl - x, x

    a1_cnt, b1_cnt, xflex1 = balance(
        dst, cls1, (own_cls == 0).astype(np.int64),
        (own_cls == 1).astype(np.int64), (own_cls == 2).astype(np.int64))

    # ---- position assignment
    mxK = cnt.max(axis=1)
    mx1 = np.maximum(a1_cnt, b1_cnt)
    pos_of = np.empty(N, np.int64)
    for c in range(NC):
        for t in range(2):
            base = CHUNK_POS[t][0]
            for r in range(RQ):
                k = RQ * t + r
                nodes = np.where((core_of == c) & (cls == k))[0]
                nodes = nodes[np.lexsort((mx1[nodes], mxK[nodes]))]
                pos_of[nodes] = SHARD * c + base + RQ * np.arange(len(nodes)) + r
    node_at = np.empty(N, np.int64)
    node_at[pos_of] = np.arange(N)

    local_of = pos_of % SHARD
    blk_of = local_of // 128
    SR = np.zeros((2, RQ, NBLK), np.int64)
    SA1 = np.zeros(NBLK, np.int64)
    SB1 = np.zeros(NBLK, np.int64)
    for k in range(NBLK):
        sel = blk_of == k
        for t in range(2):
            for r in range(RQ):
                SR[t, r, k] = max(int(cnt[sel, RQ * t + r].max()), 1)
        SA1[k] = max(int(a1_cnt[sel].max()), 1)
        SB1[k] = max(int(b1_cnt[sel].max()), 1)

    # ---- streams
    lenT = [int(SR[t].sum()) * 128 for t in range(2)]
    streamsT = [np.zeros((NC, lenT[t]), np.int32) for t in range(2)]
    lenA1 = int(SA1.sum()) * 128
    lenB1 = int(SB1.sum()) * 128
    streamsA1 = np.zeros((NC, lenA1), np.int32)
    streamsB1 = np.full((NC, lenB1), B_PAD_IDX, np.int32)

    chunk_of_local = (local_of >= CHUNK_POS[1][0]).astype(np.int64)
    quad_of = np.empty(N, np.int64)
    for t in range(2):
        selt = chunk_of_local == t
        quad_of[selt] = (local_of[selt] - CHUNK_POS[t][0]) // RQ
    r_of = (local_of - np.array([CHUNK_POS[t][0] for t in range(2)])[chunk_of_local]) % RQ
    trow = 1 + core_of * np.array(NQUADS)[chunk_of_local] + quad_of

    for c in range(NC):
        offT = [0, 0]
        offA1 = offB1 = 0
        for k in range(NBLK):
            lo, hi = 128 * k, min(128 * (k + 1), SHARD)
            gR = [[np.zeros((int(SR[t][r][k]), 128), np.int32) for r in range(RQ)]
                  for t in range(2)]
            gridA1 = np.zeros((int(SA1[k]), 128), np.int32)
            gridB1 = np.full((int(SB1[k]), 128), B_PAD_IDX, np.int32)
            for lane in range(hi - lo):
                d = node_at[SHARD * c + lo + lane]
                edge_srcs = s_sorted[d_starts[d]:d_ends[d]]
                srcs = np.concatenate([edge_srcs, [d]])
                fR = [[0] * RQ for _ in range(2)]
                for s in srcs:
                    t = int(chunk_of_local[s])
                    r = int(r_of[s])
                    gR[t][r][fR[t][r], lane] = trow[s]
                    fR[t][r] += 1
                aps = pos_of[srcs]
                acls = np.concatenate([CLS_OF_CORE[core_of[edge_srcs]],
                                       [CLS_OF_CORE[c]]])
                flex1 = aps[acls == 1]
                x1 = int(xflex1[d])
                pa1 = np.concatenate([aps[acls == 0], flex1[:x1]]) + 1
                pb1 = np.concatenate([aps[acls == 2], flex1[x1:]]) + 1 - BASE_B
                gridA1[: len(pa1), lane] = pa1
                gridB1[: len(pb1), lane] = pb1
            for t in range(2):
                bg = np.concatenate(gR[t], axis=0)
                streamsT[t][c, offT[t]:offT[t] + bg.size] = bg.ravel()
                offT[t] += bg.size
            streamsA1[c, offA1:offA1 + gridA1.size] = gridA1.ravel()
            streamsB1[c, offB1:offB1 + gridB1.size] = gridB1.ravel()
            offA1 += gridA1.size
            offB1 += gridB1.size

    # ---- pooling tables
    SP = np.zeros((NC, GBLK), np.int64)
    members = {}
    nodes_of_core = [node_at[SHARD * c:SHARD * (c + 1)] for c in range(NC)]
    for c in range(NC):
        g_of_local = batch[nodes_of_core[c]]
        for gb in range(GBLK):
            cnts = np.bincount(
                g_of_local[(g_of_local >= GPER * gb) & (g_of_local < GPER * (gb + 1))] - GPER * gb,
                minlength=GPER)
            SP[c, gb] = max(cnts.max(), 1)
        members[c] = g_of_local
    SPu = SP.max(axis=0)
    lenP = int(SPu.sum()) * 128
    streamsP = np.zeros((NC, lenP), np.int32)
    for c in range(NC):
        g_of_local = members[c]
        off = 0
        for gb in range(GBLK):
            grid = np.zeros((int(SPu[gb]), 128), np.int32)
            for gl in range(GPER):
                locs = np.where(g_of_local == GPER * gb + gl)[0] + 1
                grid[: len(locs), gl] = locs
            streamsP[c, off:off + grid.size] = grid.ravel()
            off += grid.size

    cnts = np.bincount(batch, minlength=G).astype(np.float64)
    inv_cnt = (1.0 / np.maximum(cnts, 1.0)).astype(np.float32)

    return dict(
        core_of=core_of, pos_of=pos_of, node_at=node_at,
        dinv=dinv.astype(np.float32),
        SR=SR, SA1=SA1, SB1=SB1, SPu=SPu,
        streamsT=streamsT, streamsA1=streamsA1, streamsB1=streamsB1,
        streamsP=streamsP, inv_cnt=inv_cnt,
    )


# ==== host IO staging ====

def make_in_maps(inputs, P):
    node_at = P["node_at"]
    dinv = P["dinv"]
    x = np.asarray(inputs["x"], np.float32)
    xd = x * dinv[:, None]
    xp = xd[node_at]
    xdp = np.zeros((N + 2, 64), np.float32)
    xdp[1:N + 1, :37] = xp

    dinv_pos = dinv[node_at].astype(np.float32)

    def rep(v, width):
        return np.tile(np.asarray(v, np.float32)[None, :], (128, 1))

    common = dict(
        xdp=xdp,
        W1=np.asarray(inputs["W1"], np.float32),
        W2=np.asarray(inputs["W2"], np.float32),
        W3=np.asarray(inputs["W3"], np.float32),
        Wf1=np.asarray(inputs["Wf1"], np.float32),
        Wf2=np.asarray(inputs["Wf2"], np.float32),
        b1r=rep(inputs["b1"], 64), b2r=rep(inputs["b2"], 128),
        b3r=rep(inputs["b3"], 64), bf1r=rep(inputs["bf1"], 128),
        bf2r=rep(inputs["bf2"], 64), gammar=rep(inputs["gamma"], 64),
        betar=rep(inputs["beta"], 64),
    )

    in_maps = []
    for c in range(NC):
        dvo = np.zeros((128, NBLK), np.float32)
        own = dinv_pos[SHARD * c:SHARD * (c + 1)]
        for k in range(NBLK):
            rows = min(128, SHARD - 128 * k)
            dvo[:rows, k] = own[128 * k:128 * k + rows]
        invc = np.zeros((128, 1), np.float32)
        invc[:GPER, 0] = P["inv_cnt"][GPER * c:GPER * (c + 1)]
        m = dict(common)
        m["dinv_own"] = dvo
        m["invc_own"] = invc
        m["idxT0"] = wrap16(P["streamsT"][0][c])
        m["idxT1"] = wrap16(P["streamsT"][1][c])
        m["idxA1"] = wrap16(P["streamsA1"][c])
        m["idxB1"] = wrap16(P["streamsB1"][c])
        m["idxP"] = wrap16(P["streamsP"][c])
        in_maps.append(m)
    return in_maps


# ==== bass kernel ====
from contextlib import ExitStack

import concourse.bass as bass
import concourse.bacc as bacc
import concourse.mybir as mybir
from concourse.tile import TileContext
from concourse.masks import make_identity

F32 = mybir.dt.float32
F16 = mybir.dt.float16
F8 = mybir.dt.float8e4
I16 = mybir.dt.int16
AX = mybir.AxisListType
AF = mybir.ActivationFunctionType
OP = mybir.AluOpType

D_IN, D1, D2, D3 = 37, 64, 128, 64
GW = 8
SUPER_COLS = 64


def make_supers(SAu, SBu, flush_blocks=()):
    """Group consecutive blocks with combined cols <= SUPER_COLS; force a
    group boundary at each block in flush_blocks."""
    supers = []
    k0 = 0
    na = nb = 0
    for k in range(NBLK):
        if k > k0 and (na + SAu[k] > SUPER_COLS or nb + SBu[k] > SUPER_COLS
                       or k - k0 >= 16 or k in flush_blocks):
            supers.append((k0, k))
            k0, na, nb = k, 0, 0
        na += SAu[k]
        nb += SBu[k]
    supers.append((k0, NBLK))
    if supers[-1][1] - supers[-1][0] > 1:
        k0, k1 = supers[-1]
        supers[-1] = (k0, k1 - 1)
        supers.append((k1 - 1, k1))
    return supers


def build(S, debug=False):
    SR = [[list(map(int, S["SR"][t][r])) for r in range(RQ)] for t in range(2)]
    SA1 = list(map(int, S["SA1"]))
    SB1 = list(map(int, S["SB1"]))
    SPu = list(map(int, S["SPu"]))
    ST = [[sum(SR[t][r][k] for r in range(RQ)) for k in range(NBLK)]
          for t in range(2)]
    lenT = [128 * sum(ST[t]) for t in range(2)]
    lenA1, lenB1, lenP = 128 * sum(SA1), 128 * sum(SB1), 128 * sum(SPu)
    supers1 = make_supers(SA1, SB1, flush_blocks=(CHUNK_BLK[1][0],))
    supersT = [make_supers(ST[t], ST[t], flush_blocks=(CHUNK_BLK[1][0],))
               for t in range(2)]

    nc = bacc.Bacc()

    # ---------------- IO ----------------
    xdp = nc.dram_tensor("xdp", [N + 2, 64], F32, kind="ExternalInput")
    dinv_own = nc.dram_tensor("dinv_own", [128, NBLK], F32, kind="ExternalInput")
    invc_own = nc.dram_tensor("invc_own", [128, 1], F32, kind="ExternalInput")
    idxT0 = nc.dram_tensor("idxT0", [128, lenT[0] // 16], I16, kind="ExternalInput")
    idxT1 = nc.dram_tensor("idxT1", [128, lenT[1] // 16], I16, kind="ExternalInput")
    idxA1 = nc.dram_tensor("idxA1", [128, lenA1 // 16], I16, kind="ExternalInput")
    idxB1 = nc.dram_tensor("idxB1", [128, lenB1 // 16], I16, kind="ExternalInput")
    idxP = nc.dram_tensor("idxP", [128, lenP // 16], I16, kind="ExternalInput")
    W1 = nc.dram_tensor("W1", [D_IN, D1], F32, kind="ExternalInput")
    W2 = nc.dram_tensor("W2", [D1, D2], F32, kind="ExternalInput")
    W3 = nc.dram_tensor("W3", [D2, D3], F32, kind="ExternalInput")
    Wf1 = nc.dram_tensor("Wf1", [64, 128], F32, kind="ExternalInput")
    Wf2 = nc.dram_tensor("Wf2", [128, 64], F32, kind="ExternalInput")
    b1r = nc.dram_tensor("b1r", [128, D1], F32, kind="ExternalInput")
    b2r = nc.dram_tensor("b2r", [128, D2], F32, kind="ExternalInput")
    b3r = nc.dram_tensor("b3r", [128, D3], F32, kind="ExternalInput")
    bf1r = nc.dram_tensor("bf1r", [128, 128], F32, kind="ExternalInput")
    bf2r = nc.dram_tensor("bf2r", [128, 64], F32, kind="ExternalInput")
    gammar = nc.dram_tensor("gammar", [128, 64], F32, kind="ExternalInput")
    betar = nc.dram_tensor("betar", [128, 64], F32, kind="ExternalInput")

    out_slice = nc.dram_tensor("out_slice", [GPER, 64], F32, kind="ExternalOutput")

    # internal DRAM: fp8 quad tables per chunk
    h1own = [nc.dram_tensor(f"h1own{t}", [1 + NQUADS[t], 256], F8)
             for t in range(2)]
    T1 = [nc.dram_tensor(f"T1_{t}", [1 + NC * NQUADS[t], 256], F8,
                         addr_space="Shared") for t in range(2)]
    hW3own = [nc.dram_tensor(f"hW3own{t}", [1 + NQUADS[t], 256], F8)
              for t in range(2)]
    T3 = [nc.dram_tensor(f"T3_{t}", [1 + NC * NQUADS[t], 256], F8,
                         addr_space="Shared") for t in range(2)]
    h3ot = nc.dram_tensor("h3ot", [1 + SHARD, D3], F32)
    poolin = nc.dram_tensor("poolin", [G, 64], F32)
    rs_out = nc.dram_tensor("rs_out", [GPER, 64], F32)

    dbg = {}
    if debug:
        for name, shape, dt in [
                ("d_h1own0", [1 + NQUADS[0], 256], F8),
                ("d_h1own1", [1 + NQUADS[1], 256], F8),
                ("d_T1_0", [1 + NC * NQUADS[0], 256], F8),
                ("d_hW3own0", [1 + NQUADS[0], 256], F8),
                ("d_h3", [SHARD, D3], F32),
                ("d_poolin", [G, 64], F32), ("d_rs", [GPER, 64], F32)]:
            dbg[name] = nc.dram_tensor(name, shape, dt, kind="ExternalOutput")

    rg = [list(range(NC))]

    with TileContext(nc) as tc, ExitStack() as ctx:
        const = ctx.enter_context(tc.tile_pool(name="const", bufs=1))
        idxp = ctx.enter_context(tc.tile_pool(name="idxp", bufs=1))

        ident = const.tile([128, 128], F32, tag="ident")
        make_identity(nc, ident[:])
        zt = const.tile([128, 128], F32, tag="zt")
        nc.vector.memset(zt[:], 0.0)
        zth = const.tile([128, 128], F16, tag="zth")
        nc.vector.memset(zth[:], 0.0)
        zf8 = const.tile([128, 256], F8, tag="zf8")
        nc.vector.memset(zf8[:], 0.0)
        epss = const.tile([128, 1], F32, tag="epss")
        nc.vector.memset(epss[:], 1e-5)

        def csb(t, p0, p1, tag, dt=F32):
            tl = const.tile([p0, p1], dt, tag=tag)
            nc.sync.dma_start(out=tl[:], in_=t[:])
            return tl

        W1s = csb(W1, D_IN, D1, "W1s")
        W2s = csb(W2, D1, D2, "W2s")
        W3s = csb(W3, D2, D3, "W3s")
        Wf1s = csb(Wf1, 64, 128, "Wf1s")
        Wf2s = csb(Wf2, 128, 64, "Wf2s")
        b1s = csb(b1r, 128, D1, "b1s")
        b2s = csb(b2r, 128, D2, "b2s")
        b3s = csb(b3r, 128, D3, "b3s")
        bf1s = csb(bf1r, 128, 128, "bf1s")
        bf2s = csb(bf2r, 128, 64, "bf2s")
        gams = csb(gammar, 128, 64, "gams")
        bets = csb(betar, 128, 64, "bets")
        dvo = csb(dinv_own, 128, NBLK, "dvo")
        ics = csb(invc_own, 128, 1, "ics")

        idxT_s = []
        for t in range(2):
            tl = idxp.tile([128, lenT[t] // 16], I16, tag=f"it{t}")
            nc.sync.dma_start(out=tl[:], in_=[idxT0, idxT1][t][:])
            idxT_s.append(tl)
        idxA1_s = idxp.tile([128, lenA1 // 16], I16, tag="ia1")
        nc.sync.dma_start(out=idxA1_s[:], in_=idxA1[:])
        idxB1_s = idxp.tile([128, lenB1 // 16], I16, tag="ib1")
        nc.sync.dma_start(out=idxB1_s[:], in_=idxB1[:])
        idxP_s = idxp.tile([128, lenP // 16], I16, tag="ip")
        nc.sync.dma_start(out=idxP_s[:], in_=idxP[:])

        # guard rows
        for t in range(2):
            nc.scalar.dma_start(out=h1own[t][0:1, :], in_=zf8[0:1, :])
            nc.scalar.dma_start(out=T1[t][0:1, :], in_=zf8[0:1, :])
            nc.scalar.dma_start(out=hW3own[t][0:1, :], in_=zf8[0:1, :])
            nc.scalar.dma_start(out=T3[t][0:1, :], in_=zf8[0:1, :])
        nc.scalar.dma_start(out=h3ot[0:1, :], in_=zt[0:1, 0:D3])

        # ---- fp8 quad-table store sink ---------------------------------
        def make_pair_store_sink(dsts, pname):
            """Collects per-block [128, 64] tiles (stored as f8); flushes
            groups of up to 8 blocks into the chunk quad tables. Block
            groups never straddle the chunk boundary."""
            state = {"tile": None, "k0": None, "n": 0}
            pool = ctx.enter_context(tc.tile_pool(name=pname, bufs=2))

            def flush():
                tl, k0_, n_ = state["tile"], state["k0"], state["n"]
                if tl is None or n_ == 0:
                    return
                t = 0 if k0_ < CHUNK_BLK[1][0] else 1
                dst = dsts[t]
                qr0 = 32 * (k0_ - CHUNK_BLK[t][0])
                rows_total = min(SHARD, 128 * (k0_ + n_)) - 128 * k0_
                full = n_ - 1 if rows_total < 128 * n_ else n_
                if full > 0:
                    nc.scalar.dma_start(
                        out=dst[1 + qr0:1 + qr0 + 32 * full, :].rearrange(
                            "(j q) (r f) -> (q r) j f", q=32, r=4),
                        in_=tl[:, :full * 64].rearrange("p (j f) -> p j f", j=full))
                if full < n_:
                    rl = rows_total - 128 * full          # 106 for last block
                    nq = rl // 4                          # 26 full quads
                    r0 = 1 + qr0 + 32 * full
                    nc.scalar.dma_start(
                        out=dst[r0:r0 + nq, :].rearrange(
                            "q (r f) -> (q r) f", r=4),
                        in_=tl[:4 * nq, full * 64:(full + 1) * 64])
                    rem = rl - 4 * nq                     # 2 leftover rows
                    if rem:
                        nc.scalar.dma_start(
                            out=dst[r0 + nq:r0 + nq + 1, 0:rem * 64].rearrange(
                                "q (r f) -> (q r) f", r=rem),
                            in_=tl[4 * nq:rl, full * 64:(full + 1) * 64])
                state["tile"], state["k0"], state["n"] = None, None, 0

            def sink_store(k, rows, res, func=AF.Copy, scale=1.0):
                if state["tile"] is None:
                    state["tile"] = pool.tile([128, 8 * 64], F8, tag="stg",
                                              name=pname + "_stg")
                    state["k0"], state["n"] = k, 0
                j = state["n"]
                nc.scalar.activation(
                    state["tile"][:, j * 64:(j + 1) * 64], res[:], func,
                    scale=scale)
                state["n"] += 1
                if state["n"] == 8 or k == CHUNK_BLK[0][1] - 1:
                    flush()

            return sink_store, flush

        # ---- plain-row store sink (h3ot) -------------------------------
        def make_store_sink(dst, fout, dt, pname, row_off=0):
            state = {"tile": None, "k0": None, "n": 0}
            pool = ctx.enter_context(tc.tile_pool(name=pname, bufs=2))

            def flush():
                t, k0_, n_ = state["tile"], state["k0"], state["n"]
                if t is None or n_ == 0:
                    return
                row0 = row_off + 128 * k0_
                nrows = min(SHARD, 128 * (k0_ + n_)) - 128 * k0_
                full = n_ - 1 if nrows < 128 * n_ else n_
                if full > 0:
                    nc.scalar.dma_start(
                        out=dst[row0:row0 + 128 * full, :].rearrange(
                            "(j p) f -> p j f", p=128),
                        in_=t[:, :full * fout].rearrange("p (j f) -> p j f", j=full))
                if full < n_:
                    rl = nrows - 128 * full
                    nc.scalar.dma_start(
                        out=dst[row0 + 128 * full:row0 + nrows, :],
                        in_=t[:rl, full * fout:(full + 1) * fout])
                state["tile"], state["k0"], state["n"] = None, None, 0

            def sink_store(k, rows, res, func=AF.Copy, scale=1.0):
                if state["tile"] is None:
                    state["tile"] = pool.tile([128, 8 * fout], dt, tag="stg",
                                              name=pname + "_stg")
                    state["k0"], state["n"] = k, 0
                j = state["n"]
                nc.scalar.activation(
                    state["tile"][:, j * fout:(j + 1) * fout], res[:], func,
                    scale=scale)
                state["n"] += 1
                if state["n"] == 8:
                    flush()

            return sink_store, flush

        # ---- L1 gather driver (A1/B1 f32 windows, like v2 main pass) ----
        def fold_plain(gt, elem, c0, s, upto=1):
            while s > upto:
                h = min(s // 2, s - upto)
                nc.vector.tensor_tensor(
                    out=gt[:, c0 * elem:(c0 + h) * elem],
                    in0=gt[:, c0 * elem:(c0 + h) * elem],
                    in1=gt[:, (c0 + s - h) * elem:(c0 + s) * elem],
                    op=OP.add)
                s -= h

        def l1_layer(sink, mid_hook=None, park_ms=0.0):
            elem = 64
            with tc.tile_pool(name="gta", bufs=3) as gta_pool, \
                 tc.tile_pool(name="gtb", bufs=3) as gtb_pool, \
                 tc.tile_pool(name="hacc", bufs=4) as hacc_pool, \
                 ExitStack() as lctx:
                offA = offB = 0
                for (k0, k1) in supers1:
                    if k0 == CHUNK_BLK[1][0] and mid_hook is not None:
                        mid_hook()
                        if park_ms > 0:
                            lctx.enter_context(tc.tile_wait_until(park_ms))
                    nA = sum(SA1[k0:k1])
                    nB = sum(SB1[k0:k1])
                    gtA = gta_pool.tile([128, SUPER_COLS * elem], F32, tag="gtA",
                                        name="gtA")
                    done = 0
                    while done < nA:
                        w = min(GW, nA - done)
                        nc.gpsimd.dma_gather(
                            out_ap=gtA[:, done * elem:(done + w) * elem].rearrange(
                                "p (c f) -> p c f", c=w),
                            in_ap=xdp[:],
                            idxs_ap=idxA1_s[:, 8 * (offA + done):8 * (offA + done + w)],
                            num_idxs=128 * w, num_idxs_reg=128 * w,
                            elem_size=elem)
                        done += w
                    gtB = gtb_pool.tile([128, SUPER_COLS * elem], F32, tag="gtB",
                                        name="gtB")
                    done = 0
                    while done < nB:
                        w = min(GW, nB - done)
                        nc.gpsimd.dma_gather(
                            out_ap=gtB[:, done * elem:(done + w) * elem].rearrange(
                                "p (c f) -> p c f", c=w),
                            in_ap=xdp[BASE_B:, :],
                            idxs_ap=idxB1_s[:, 8 * (offB + done):8 * (offB + done + w)],
                            num_idxs=128 * w, num_idxs_reg=128 * w,
                            elem_size=elem)
                        done += w
                    cA = cB = 0
                    for k in range(k0, k1):
                        sa, sb_ = SA1[k], SB1[k]
                        rows = min(128, SHARD - 128 * k)
                        acc = hacc_pool.tile([128, elem], F32, tag="acc")
                        fold_plain(gtA, elem, cA, sa, upto=2)
                        if sa >= 2:
                            nc.vector.tensor_tensor(
                                out=acc[:],
                                in0=gtA[:, cA * elem:(cA + 1) * elem],
                                in1=gtA[:, (cA + 1) * elem:(cA + 2) * elem],
                                op=OP.add)
                        else:
                            nc.vector.tensor_copy(
                                out=acc[:], in_=gtA[:, cA * elem:(cA + 1) * elem])
                        fold_plain(gtB, elem, cB, sb_, upto=1)
                        nc.vector.tensor_tensor(
                            out=acc[:], in0=acc[:],
                            in1=gtB[:, cB * elem:(cB + 1) * elem], op=OP.add)
                        cA += sa
                        cB += sb_
                        sink(k, rows, acc)
                    offA += nA
                    offB += nB

        # ---- fp8 quad-table gather layer (two passes over chunk tables) ----
        def pair_layer(tabs, idx_tiles, sink, mid_hook=None, park_ms=0.0):
            """tabs: [T_0, T_1] dram quad tables; sink(k, rows, acc_f32_64).
            Fold: sequential accumulation into an f32 target so fp8 never
            holds a partial sum."""
            with tc.tile_pool(name="accv", bufs=1) as accv_pool, \
                 tc.tile_pool(name="hacc", bufs=4) as hacc_pool:
                accV = accv_pool.tile([128, NBLK * 64], F32, tag="accV")
                for t in range(2):
                    with tc.tile_pool(name=f"gt{t}", bufs=3) as gt_pool, \
                         ExitStack() as lctx:
                        off = 0
                        for (k0, k1) in supersT[t]:
                            if (t == 1 and k0 == CHUNK_BLK[1][0]
                                    and mid_hook is not None):
                                mid_hook()
                                if park_ms > 0:
                                    lctx.enter_context(
                                        tc.tile_wait_until(park_ms))
                            ncols = sum(ST[t][k0:k1])
                            gt = gt_pool.tile([128, SUPER_COLS * 256], F8,
                                              tag="gt", name=f"gt{t}")
                            done = 0
                            while done < ncols:
                                w = min(GW, ncols - done)
                                nc.gpsimd.dma_gather(
                                    out_ap=gt[:, done * 256:(done + w) * 256].rearrange(
                                        "p (c f) -> p c f", c=w),
                                    in_ap=tabs[t][:],
                                    idxs_ap=idx_tiles[t][:, 8 * (off + done):8 * (off + done + w)],
                                    num_idxs=128 * w, num_idxs_reg=128 * w,
                                    elem_size=256)
                                done += w
                            c0 = 0
                            g3 = gt[:].rearrange("p (c f) -> p c f", f=256)
                            for k in range(k0, k1):
                                rows = min(128, SHARD - 128 * k)
                                cols = []
                                cc = c0
                                for r in range(RQ):
                                    for i in range(SR[t][r][k]):
                                        cols.append((cc + i, r * 64))
                                    cc += SR[t][r][k]
                                if t == 0:
                                    target = accV[:, k * 64:(k + 1) * 64]
                                else:
                                    acc = hacc_pool.tile([128, 64], F32, tag="acc")
                                    target = acc[:]
                                (i0, o0), (i1, o1) = cols[0], cols[1]
                                nc.vector.tensor_tensor(
                                    out=target,
                                    in0=g3[:, i0:i0 + 1, o0:o0 + 64],
                                    in1=g3[:, i1:i1 + 1, o1:o1 + 64], op=OP.add)
                                for (ci, o) in cols[2:]:
                                    nc.vector.tensor_tensor(
                                        out=target, in0=target,
                                        in1=g3[:, ci:ci + 1, o:o + 64], op=OP.add)
                                if t == 1:
                                    nc.vector.tensor_tensor(
                                        out=target, in0=target,
                                        in1=accV[:, k * 64:(k + 1) * 64], op=OP.add)
                                    sink(k, rows, acc)
                                c0 = cc
                            off += ncols

        # ================= L1 =================
        store1, flush1 = make_pair_store_sink(h1own, "st1")
        with tc.tile_pool(name="tp1", bufs=2, space="PSUM") as tp1, \
             tc.tile_pool(name="mm0", bufs=2, space="PSUM") as mm0, \
             tc.tile_pool(name="hT1", bufs=4) as hT1p, \
             tc.tile_pool(name="h1b", bufs=4) as h1bp:
            def sink1(k, rows, agg):
                # table = dvo * relu(dvo*(agg@W1) + b1); dvo passes through
                # the matmul (per-row scale), relu(s*x)=s*relu(x) for s>0.
                tp0 = tp1.tile([64, 128], F32, tag="tp0", name="tp0")
                nc.tensor.transpose(out=tp0[:], in_=agg[:], identity=ident[:])
                aT = hT1p.tile([64, 128], F32, tag="aT", name="aT")
                nc.scalar.activation(aT[:], tp0[:], AF.Copy)
                ps0 = mm0.tile([128, D1], F32, tag="ps0", name="ps0")
                nc.tensor.matmul(out=ps0[:], lhsT=aT[:D_IN, :], rhs=W1s[:],
                                 start=True, stop=True)
                h = h1bp.tile([128, D1], F32, tag="h1t", name="h1t")
                nc.vector.tensor_tensor(
                    out=h[:], in0=ps0[:],
                    in1=dvo[:, k:k + 1].to_broadcast([128, D1]), op=OP.mult)
                nc.vector.tensor_tensor(out=h[:], in0=h[:], in1=b1s[:], op=OP.add)
                store1(k, rows, h, func=AF.Relu, scale=dvo[:, k:k + 1])

            def mid1():
                flush1()
                with tc.high_priority():
                    nc.gpsimd.collective_compute(
                        "AllGather", OP.bypass, replica_groups=rg,
                        ins=[h1own[0][1:, :]], outs=[T1[0][1:, :]])
            l1_layer(sink1, mid_hook=mid1, park_ms=0.068)
            flush1()
        with tc.high_priority():
            nc.gpsimd.collective_compute(
                "AllGather", OP.bypass, replica_groups=rg,
                ins=[h1own[1][1:, :]], outs=[T1[1][1:, :]])

        # ================= L2 =================
        store2, flush2 = make_pair_store_sink(hW3own, "st2")
        with tc.tile_pool(name="tp2", bufs=2, space="PSUM") as tp2, \
             tc.tile_pool(name="mm2", bufs=2, space="PSUM") as mm2, \
             tc.tile_pool(name="mm3", bufs=2, space="PSUM") as mm3, \
             tc.tile_pool(name="hT2", bufs=4) as hT2p, \
             tc.tile_pool(name="h2b", bufs=4) as h2bp:
            def sink2(k, rows, acc):
                # h2' = dvo*relu(dvo*(acc@W2) + b2) = relu(dvo*(dvo*(acc@W2)+b2));
                # store f8((h2' @ W3))
                tp0 = tp2.tile([64, 128], F32, tag="tp0", name="tp0")
                nc.tensor.transpose(out=tp0[:], in_=acc[:], identity=ident[:])
                aT = hT2p.tile([64, 128], F32, tag="aT", name="aT")
                nc.scalar.activation(aT[:], tp0[:], AF.Copy)
                ps = mm2.tile([128, D2], F32, tag="ps", name="ps")
                nc.tensor.matmul(out=ps[:], lhsT=aT[:], rhs=W2s[:],
                                 start=True, stop=True)
                h = h2bp.tile([128, D2], F32, tag="h2t", name="h2t")
                nc.vector.tensor_tensor(
                    out=h[:], in0=ps[:],
                    in1=dvo[:, k:k + 1].to_broadcast([128, D2]), op=OP.mult)
                nc.vector.tensor_tensor(out=h[:], in0=h[:], in1=b2s[:], op=OP.add)
                nc.scalar.activation(h[:], h[:], AF.Relu,
                                     scale=dvo[:, k:k + 1])
                tp = tp2.tile([D2, 128], F32, tag="tp", name="tp")
                nc.tensor.transpose(out=tp[:], in_=h[:], identity=ident[:])
                hT = hT2p.tile([D2, 128], F32, tag="hT", name="hT")
                nc.vector.tensor_copy(out=hT[:], in_=tp[:])
                ps3 = mm3.tile([128, D3], F32, tag="ps3", name="ps3")
                nc.tensor.matmul(out=ps3[:], lhsT=hT[:], rhs=W3s[:],
                                 start=True, stop=True)
                store2(k, rows, ps3)

            def mid2():
                flush2()
                with tc.high_priority():
                    nc.gpsimd.collective_compute(
                        "AllGather", OP.bypass, replica_groups=rg,
                        ins=[hW3own[0][1:, :]], outs=[T3[0][1:, :]])
            pair_layer(T1, idxT_s, sink2, mid_hook=mid2, park_ms=0.30)
            flush2()
        with tc.high_priority():
            nc.gpsimd.collective_compute(
                "AllGather", OP.bypass, replica_groups=rg,
                ins=[hW3own[1][1:, :]], outs=[T3[1][1:, :]])

        # ================= L3 =================
        storeh3, flushh3 = make_store_sink(h3ot, D3, F32, "sth3", row_off=1)

        def sink3(k, rows, acc):
            nc.vector.tensor_tensor(
                out=acc[:], in0=acc[:],
                in1=dvo[:, k:k + 1].to_broadcast([128, D3]), op=OP.mult)
            nc.vector.tensor_tensor(out=acc[:], in0=acc[:], in1=b3s[:], op=OP.add)
            storeh3(k, rows, acc, func=AF.Relu)
        pair_layer(T3, idxT_s, sink3)
        flushh3()

        # ================= pooling =================
        with tc.tile_pool(name="gp", bufs=1) as gp, \
             tc.tile_pool(name="pstg", bufs=1) as pstg:
            stg = pstg.tile([128, GBLK * 64], F32, tag="pstg")
            spT = sum(SPu)
            gt = gp.tile([128, spT * 64], F32, tag="gtp")
            done = 0
            while done < spT:
                w = min(GW, spT - done)
                nc.gpsimd.dma_gather(
                    out_ap=gt[:, done * 64:(done + w) * 64].rearrange(
                        "p (c f) -> p c f", c=w),
                    in_ap=h3ot[:],
                    idxs_ap=idxP_s[:, 8 * done:8 * (done + w)],
                    num_idxs=128 * w, num_idxs_reg=128 * w, elem_size=64)
                done += w
            offP = 0
            for gb in range(GBLK):
                sp = SPu[gb]
                fold_plain(gt, 64, offP, sp, upto=2)
                nc.vector.tensor_tensor(
                    out=stg[:, 64 * gb:64 * (gb + 1)],
                    in0=gt[:, offP * 64:(offP + 1) * 64],
                    in1=gt[:, (offP + 1) * 64:(offP + 2) * 64], op=OP.add)
                offP += sp
            nc.scalar.dma_start(
                out=poolin[:, :].rearrange("(j p) f -> p j f", p=GPER),
                in_=stg[:GPER, :].rearrange("p (j f) -> p j f", j=GBLK))

        nc.gpsimd.collective_compute(
            "ReduceScatter", OP.add, replica_groups=rg,
            ins=[poolin[:]], outs=[rs_out[:]])

        # ================= MLP + LayerNorm =================
        with tc.tile_pool(name="mlp", bufs=1) as mlp, \
             tc.tile_pool(name="mps", bufs=2, space="PSUM") as mps:
            gtl = mlp.tile([128, 64], F32, tag="g0")
            nc.vector.memset(gtl[:], 0.0)
            nc.sync.dma_start(out=gtl[:GPER, :], in_=rs_out[:])
            nc.vector.tensor_tensor(
                out=gtl[:], in0=gtl[:],
                in1=ics[:].to_broadcast([128, 64]), op=OP.mult)
            tp = mps.tile([64, 128], F32, tag="t1")
            nc.tensor.transpose(out=tp[:], in_=gtl[:], identity=ident[:])
            gT = mlp.tile([64, 128], F32, tag="gT")
            nc.vector.tensor_copy(out=gT[:], in_=tp[:])
            p1 = mps.tile([128, 128], F32, tag="p1")
            nc.tensor.matmul(out=p1[:], lhsT=gT[:], rhs=Wf1s[:],
                             start=True, stop=True)
            g1t = mlp.tile([128, 128], F32, tag="g1t")
            nc.vector.tensor_tensor(out=g1t[:], in0=p1[:], in1=bf1s[:], op=OP.add)
            nc.scalar.activation(g1t[:], g1t[:], AF.Relu)
            tp2_ = mps.tile([128, 128], F32, tag="t2")
            nc.tensor.transpose(out=tp2_[:], in_=g1t[:], identity=ident[:])
            g1T = mlp.tile([128, 128], F32, tag="g1T")
            nc.vector.tensor_copy(out=g1T[:], in_=tp2_[:])
            p2 = mps.tile([128, 64], F32, tag="p2")
            nc.tensor.matmul(out=p2[:], lhsT=g1T[:], rhs=Wf2s[:],
                             start=True, stop=True)
            g2t = mlp.tile([128, 64], F32, tag="g2t")
            nc.vector.tensor_tensor(out=g2t[:], in0=p2[:], in1=bf2s[:], op=OP.add)
            nc.scalar.activation(g2t[:], g2t[:], AF.Relu)
            mu = mlp.tile([128, 1], F32, tag="mu")
            nc.vector.reduce_sum(mu[:], g2t[:], axis=AX.X)
            nc.vector.tensor_scalar_mul(mu[:], in0=mu[:], scalar1=1.0 / 64)
            xm = mlp.tile([128, 64], F32, tag="xm")
            nc.vector.tensor_tensor(out=xm[:], in0=g2t[:],
                                    in1=mu[:].to_broadcast([128, 64]),
                                    op=OP.subtract)
            sq = mlp.tile([128, 64], F32, tag="sq")
            nc.vector.tensor_tensor(out=sq[:], in0=xm[:], in1=xm[:], op=OP.mult)
            var = mlp.tile([128, 1], F32, tag="var")
            nc.vector.reduce_sum(var[:], sq[:], axis=AX.X)
            rstd = mlp.tile([128, 1], F32, tag="rstd")
            nc.vector.tensor_scalar_mul(var[:], in0=var[:], scalar1=1.0 / 64)
            nc.vector.tensor_tensor(out=var[:], in0=var[:], in1=epss[:],
                                    op=OP.add)
            nc.scalar.activation(rstd[:], var[:], AF.Sqrt)
            nc.vector.reciprocal(rstd[:], rstd[:])
            nc.vector.tensor_tensor(out=xm[:], in0=xm[:],
                                    in1=rstd[:].to_broadcast([128, 64]),
                                    op=OP.mult)
            nc.vector.tensor_tensor(out=xm[:], in0=xm[:], in1=gams[:], op=OP.mult)
            nc.vector.tensor_tensor(out=xm[:], in0=xm[:], in1=bets[:], op=OP.add)
            nc.sync.dma_start(out=out_slice[:, :], in_=xm[:GPER, :])

        if debug:
            with tc.tile_pool(name="dbg", bufs=2) as dp:
                def dump(srct, dstt, nrows, width, dt=F32):
                    for c in range((nrows + 127) // 128):
                        rows = min(128, nrows - 128 * c)
                        tl = dp.tile([128, width], dt, tag="dt")
                        nc.sync.dma_start(out=tl[:rows, :],
                                          in_=srct[128 * c:128 * c + rows, :])
                        nc.sync.dma_start(out=dstt[128 * c:128 * c + rows, :],
                                          in_=tl[:rows, :])
                dump(h1own[0], dbg["d_h1own0"], 1 + NPAIRS[0], 128, F16)
                dump(h1own[1], dbg["d_h1own1"], 1 + NPAIRS[1], 128, F16)
                dump(T1[0], dbg["d_T1_0"], 1 + NC * NPAIRS[0], 128, F16)
                dump(hW3own[0], dbg["d_hW3own0"], 1 + NPAIRS[0], 128, F16)
                dump(h3ot[1:, :], dbg["d_h3"], SHARD, D3)
                dump(poolin, dbg["d_poolin"], G, 64)
                dump(rs_out, dbg["d_rs"], GPER, 64)

    nc.compile()
    nc.finalize()
    return nc


# ==== SPMD runner (same as v2) ====
import jax
from jax.sharding import Mesh, PartitionSpec
from jax.experimental.shard_map import shard_map

from concourse import bass2jax


class SpmdRunner:
    def __init__(self, nc, n_cores=8):
        bass2jax.install_neuronx_cc_hook()
        self.nc = nc
        self.n_cores = n_cores
        partition_name = nc.partition_id_tensor.name if nc.partition_id_tensor else None
        in_names, out_names, out_avals, zero_outs = [], [], [], []
        for alloc in nc.m.functions[0].allocations:
            if not isinstance(alloc, mybir.MemoryLocationSet):
                continue
            name = alloc.memorylocations[0].name
            if alloc.kind == "ExternalInput":
                if name != partition_name:
                    in_names.append(name)
            elif alloc.kind == "ExternalOutput":
                shape = tuple(alloc.tensor_shape)
                dtype = mybir.dt.np(alloc.dtype)
                out_names.append(name)
                out_avals.append(jax.core.ShapedArray(shape, dtype))
                zero_outs.append(np.zeros(shape, dtype))
        self.in_names = list(in_names)
        self.out_names = out_names
        self.out_avals = out_avals
        self.zero_outs = zero_outs
        n_params = len(in_names)
        n_outs = len(out_avals)
        all_in_names = in_names + out_names + ([partition_name] if partition_name else [])
        self.n_params = n_params

        def _body(*args):
            operands = list(args)
            if partition_name is not None:
                operands.append(bass2jax.partition_id_tensor())
            outs = bass2jax._bass_exec_p.bind(
                *operands,
                out_avals=tuple(out_avals),
                in_names=tuple(all_in_names),
                out_names=tuple(out_names),
                lowering_input_output_aliases=(),
                sim_require_finite=True,
                sim_require_nnan=True,
                nc=nc,
            )
            return tuple(outs)

        try:
            devices = jax.devices("axon")[:n_cores]
        except RuntimeError:
            devices = jax.devices()[:n_cores]
        mesh = Mesh(np.asarray(devices), ("core",))
        in_specs = (PartitionSpec("core"),) * (n_params + n_outs)
        out_specs = (PartitionSpec("core"),) * n_outs
        self.fn = jax.jit(
            shard_map(_body, mesh=mesh, in_specs=in_specs, out_specs=out_specs,
                      check_rep=False),
            keep_unused=True,
        )

    def stage(self, in_maps):
        concat = [
            np.concatenate([np.asarray(in_maps[c][n]) for c in range(self.n_cores)], axis=0)
            for n in self.in_names
        ]
        zeros = [np.zeros((self.n_cores * z.shape[0], *z.shape[1:]), z.dtype)
                 for z in self.zero_outs]
        return concat + zeros

    def run(self, staged):
        out = self.fn(*staged)
        jax.block_until_ready(out)
        return out

    def unpack(self, out_arrs):
        return [
            {
                name: np.asarray(out_arrs[i]).reshape(
                    self.n_cores, *self.out_avals[i].shape)[c]
                for i, name in enumerate(self.out_names)
            }
            for c in range(self.n_cores)
        ]


_CACHE = {}


def kernel(**inputs):
    inputs = {k: np.asarray(v) for k, v in inputs.items()}
    P = preprocess(inputs["edge_index"], inputs["batch"])
    key = (tuple(map(tuple, P["SR"].reshape(2 * RQ, -1))),
           tuple(P["SA1"].tolist()), tuple(P["SB1"].tolist()),
           tuple(P["SPu"].tolist()))
    if key not in _CACHE:
        S = {k: P[k] for k in S_KEYS}
        nc = build(S, debug=False)
        _CACHE[key] = SpmdRunner(nc, 8)
    r = _CACHE[key]
    in_maps = make_in_maps(inputs, P)
    staged = r.stage(in_maps)
    res = r.unpack(r.run(staged))
    return np.ascontiguousarray(
        np.concatenate([res[c]["out_slice"] for c in range(NC)], axis=0),
        dtype=np.float32)


# revision 7
# speedup vs baseline: 1.0807x; 1.0088x over previous
"""Trainium2 Bass kernel for nn_LinkerEncoder — v3.

Structure vs v2:
- Tables between layers are fp16 PAIR-PACKED: row = two consecutive
  positions' 64-wide vectors (256B = min gather elem). AllGather ships half
  the bytes of v2 (6.4MB -> 2x ~3.2MB chunks).
- Node classes = (chunk, parity): position chunk 0 = blocks 0..24 (3200
  pos), chunk 1 = blocks 25..48 (3050). Host greedily balances classes per
  dest so the per-block per-class slot maxes stay tight.
- L2/L3 gather streams are flat per chunk: per block [E cols][O cols], one
  gather table per chunk; fold reads the wanted 64-wide half via strided
  views (parity known at build time).
- Each AllGather is split into 2 chunk collectives: chunk-0 fires mid-way
  through the producing layer and overlaps its tail; the consuming layer's
  chunk-0 pass overlaps the chunk-1 collective (pre-pass into an
  accumulator, like v2's O-pre-pass).
- L1 keeps the v2 A1/B1 int16-window streams over the f32 xdp table.
"""
import numpy as np

N = 50000
NC = 8
SHARD = N // NC          # 6250
NBLK = (SHARD + 127) // 128   # 49
BASE_B = 17234
B_PAD_IDX = N + 1 - BASE_B
CLS_OF_CORE = np.array([0, 0, 0, 1, 1, 2, 2, 2], np.int32)
G = 1000
GBLK = 8
GPER = G // GBLK

CHUNK_POS = [(0, 3200), (3200, 6250)]
CHUNK_BLK = [(0, 25), (25, 49)]
RQ = 4                       # positions packed per 256B fp8 table row
NQUADS = [800, 763]          # ceil(chunk_size / 4); last quad of chunk 1 is half
S_KEYS = ("SR", "SA1", "SB1", "SPu")


def wrap16(stream):
    L = len(stream)
    w = np.asarray(stream, np.int32).reshape(L // 16, 16).T.astype(np.int16)
    return np.tile(w, (8, 1))


def preprocess(edge_index, batch):
    src = np.asarray(edge_index[0], np.int64)
    dst = np.asarray(edge_index[1], np.int64)
    batch = np.asarray(batch, np.int64)

    indeg = np.bincount(dst, minlength=N).astype(np.int64) + 1
    dinv = 1.0 / np.sqrt(indeg.astype(np.float64))

    rank = np.argsort(-indeg, kind="stable")
    core_of = np.empty(N, np.int32)
    core_of[rank] = np.arange(N, dtype=np.int32) % NC

    order_d = np.argsort(dst, kind="stable")
    s_sorted = src[order_d]
    d_sorted = dst[order_d]
    d_starts = np.searchsorted(d_sorted, np.arange(N))
    d_ends = np.searchsorted(d_sorted, np.arange(N) + 1)

    order_s = np.argsort(src, kind="stable")
    d_by_s = dst[order_s]
    s_starts = np.searchsorted(src[order_s], np.arange(N))
    s_ends = np.searchsorted(src[order_s], np.arange(N) + 1)

    # ---- class assignment: K=8 (chunk x quad-slot), greedy balance per dest
    K = 8
    cnt = np.zeros((N, K), np.int32)
    cap = np.zeros((NC, K), np.int64)
    for t in range(2):
        size = CHUNK_POS[t][1] - CHUNK_POS[t][0]
        for r in range(RQ):
            # positions base+4q+r exist while 4q+r < size
            cap[:, RQ * t + r] = (size - r + RQ - 1) // RQ
    cls = np.empty(N, np.int32)
    outdeg = (s_ends - s_starts) + 1
    proc = np.argsort(-outdeg, kind="stable")
    BIG = 1 << 30
    for v in proc:
        c = core_of[v]
        ds = np.concatenate([d_by_s[s_starts[v]:s_ends[v]], [v]])
        loads = cnt[ds]
        mx = loads.max(axis=1, keepdims=True)
        score = (loads >= mx).sum(axis=0) * 1000 + loads.sum(axis=0)
        score = np.where(cap[c] > 0, score, BIG)
        k = int(np.argmin(score))
        cls[v] = k
        cap[c, k] -= 1
        cnt[ds, k] += 1

    # exact per-dest per-class counts (greedy's cnt drops multi-edge dups)
    cnt = np.zeros((N, K), np.int32)
    for k in range(K):
        cnt[:, k] = np.bincount(dst[cls[src] == k], minlength=N)
    cnt[np.arange(N), cls] += 1  # self loop

    # ---- L1 window balance (flat counts incl self)
    own_cls = CLS_OF_CORE[core_of]
    cls1 = CLS_OF_CORE[core_of[src]]

    def balance(sel_dst, sel_cls, extra_a, extra_l, extra_b):
        fa = np.bincount(sel_dst, weights=(sel_cls == 0).astype(np.float64),
                         minlength=N).astype(np.int64) + extra_a
        fl = np.bincount(sel_dst, weights=(sel_cls == 1).astype(np.float64),
                         minlength=N).astype(np.int64) + extra_l
        fb = np.bincount(sel_dst, weights=(sel_cls == 2).astype(np.float64),
                         minlength=N).astype(np.int64) + extra_b
        x = np.clip((fb - fa + fl + 1) // 2, 0, fl)
        return fa + x, fb + fl - x, x

    a1_cnt, b1_cnt, xflex1 = balance(
        dst, cls1, (own_cls == 0).astype(np.int64),
        (own_cls == 1).astype(np.int64), (own_cls == 2).astype(np.int64))

    # ---- position assignment
    mxK = cnt.max(axis=1)
    mx1 = np.maximum(a1_cnt, b1_cnt)
    pos_of = np.empty(N, np.int64)
    for c in range(NC):
        for t in range(2):
            base = CHUNK_POS[t][0]
            for r in range(RQ):
                k = RQ * t + r
                nodes = np.where((core_of == c) & (cls == k))[0]
                nodes = nodes[np.lexsort((mx1[nodes], mxK[nodes]))]
                pos_of[nodes] = SHARD * c + base + RQ * np.arange(len(nodes)) + r
    node_at = np.empty(N, np.int64)
    node_at[pos_of] = np.arange(N)

    local_of = pos_of % SHARD
    blk_of = local_of // 128
    SR = np.zeros((2, RQ, NBLK), np.int64)
    SA1 = np.zeros(NBLK, np.int64)
    SB1 = np.zeros(NBLK, np.int64)
    for k in range(NBLK):
        sel = blk_of == k
        for t in range(2):
            for r in range(RQ):
                SR[t, r, k] = max(int(cnt[sel, RQ * t + r].max()), 1)
        SA1[k] = max(int(a1_cnt[sel].max()), 1)
        SB1[k] = max(int(b1_cnt[sel].max()), 1)

    # ---- streams
    lenT = [int(SR[t].sum()) * 128 for t in range(2)]
    streamsT = [np.zeros((NC, lenT[t]), np.int32) for t in range(2)]
    lenA1 = int(SA1.sum()) * 128
    lenB1 = int(SB1.sum()) * 128
    streamsA1 = np.zeros((NC, lenA1), np.int32)
    streamsB1 = np.full((NC, lenB1), B_PAD_IDX, np.int32)

    chunk_of_local = (local_of >= CHUNK_POS[1][0]).astype(np.int64)
    quad_of = np.empty(N, np.int64)
    for t in range(2):
        selt = chunk_of_local == t
        quad_of[selt] = (local_of[selt] - CHUNK_POS[t][0]) // RQ
    r_of = (local_of - np.array([CHUNK_POS[t][0] for t in range(2)])[chunk_of_local]) % RQ
    trow = 1 + core_of * np.array(NQUADS)[chunk_of_local] + quad_of

    for c in range(NC):
        offT = [0, 0]
        offA1 = offB1 = 0
        for k in range(NBLK):
            lo, hi = 128 * k, min(128 * (k + 1), SHARD)
            gR = [[np.zeros((int(SR[t][r][k]), 128), np.int32) for r in range(RQ)]
                  for t in range(2)]
            gridA1 = np.zeros((int(SA1[k]), 128), np.int32)
            gridB1 = np.full((int(SB1[k]), 128), B_PAD_IDX, np.int32)
            for lane in range(hi - lo):
                d = node_at[SHARD * c + lo + lane]
                edge_srcs = s_sorted[d_starts[d]:d_ends[d]]
                srcs = np.concatenate([edge_srcs, [d]])
                fR = [[0] * RQ for _ in range(2)]
                for s in srcs:
                    t = int(chunk_of_local[s])
                    r = int(r_of[s])
                    gR[t][r][fR[t][r], lane] = trow[s]
                    fR[t][r] += 1
                aps = pos_of[srcs]
                acls = np.concatenate([CLS_OF_CORE[core_of[edge_srcs]],
                                       [CLS_OF_CORE[c]]])
                flex1 = aps[acls == 1]
                x1 = int(xflex1[d])
                pa1 = np.concatenate([aps[acls == 0], flex1[:x1]]) + 1
                pb1 = np.concatenate([aps[acls == 2], flex1[x1:]]) + 1 - BASE_B
                gridA1[: len(pa1), lane] = pa1
                gridB1[: len(pb1), lane] = pb1
            for t in range(2):
                bg = np.concatenate(gR[t], axis=0)
                streamsT[t][c, offT[t]:offT[t] + bg.size] = bg.ravel()
                offT[t] += bg.size
            streamsA1[c, offA1:offA1 + gridA1.size] = gridA1.ravel()
            streamsB1[c, offB1:offB1 + gridB1.size] = gridB1.ravel()
            offA1 += gridA1.size
            offB1 += gridB1.size

    # ---- pooling tables
    SP = np.zeros((NC, GBLK), np.int64)
    members = {}
    nodes_of_core = [node_at[SHARD * c:SHARD * (c + 1)] for c in range(NC)]
    for c in range(NC):
        g_of_local = batch[nodes_of_core[c]]
        for gb in range(GBLK):
            cnts = np.bincount(
                g_of_local[(g_of_local >= GPER * gb) & (g_of_local < GPER * (gb + 1))] - GPER * gb,
                minlength=GPER)
            SP[c, gb] = max(cnts.max(), 1)
        members[c] = g_of_local
    SPu = SP.max(axis=0)
    lenP = int(SPu.sum()) * 128
    streamsP = np.zeros((NC, lenP), np.int32)
    for c in range(NC):
        g_of_local = members[c]
        off = 0
        for gb in range(GBLK):
            grid = np.zeros((int(SPu[gb]), 128), np.int32)
            for gl in range(GPER):
                locs = np.where(g_of_local == GPER * gb + gl)[0] + 1
                grid[: len(locs), gl] = locs
            streamsP[c, off:off + grid.size] = grid.ravel()
            off += grid.size

    cnts = np.bincount(batch, minlength=G).astype(np.float64)
    inv_cnt = (1.0 / np.maximum(cnts, 1.0)).astype(np.float32)

    return dict(
        core_of=core_of, pos_of=pos_of, node_at=node_at,
        dinv=dinv.astype(np.float32),
        SR=SR, SA1=SA1, SB1=SB1, SPu=SPu,
        streamsT=streamsT, streamsA1=streamsA1, streamsB1=streamsB1,
        streamsP=streamsP, inv_cnt=inv_cnt,
    )


# ==== host IO staging ====

def make_in_maps(inputs, P):
    node_at = P["node_at"]
    dinv = P["dinv"]
    x = np.asarray(inputs["x"], np.float32)
    xd = x * dinv[:, None]
    xp = xd[node_at]
    xdp = np.zeros((N + 2, 64), np.float32)
    xdp[1:N + 1, :37] = xp

    dinv_pos = dinv[node_at].astype(np.float32)

    def rep(v, width):
        return np.tile(np.asarray(v, np.float32)[None, :], (128, 1))

    common = dict(
        xdp=xdp,
        W1=np.asarray(inputs["W1"], np.float32),
        W2=np.asarray(inputs["W2"], np.float32),
        W3=np.asarray(inputs["W3"], np.float32),
        Wf1=np.asarray(inputs["Wf1"], np.float32),
        Wf2=np.asarray(inputs["Wf2"], np.float32),
        b1r=rep(inputs["b1"], 64), b2r=rep(inputs["b2"], 128),
        b3r=rep(inputs["b3"], 64), bf1r=rep(inputs["bf1"], 128),
        bf2r=rep(inputs["bf2"], 64), gammar=rep(inputs["gamma"], 64),
        betar=rep(inputs["beta"], 64),
    )

    in_maps = []
    for c in range(NC):
        dvo = np.zeros((128, NBLK), np.float32)
        own = dinv_pos[SHARD * c:SHARD * (c + 1)]
        for k in range(NBLK):
            rows = min(128, SHARD - 128 * k)
            dvo[:rows, k] = own[128 * k:128 * k + rows]
        invc = np.zeros((128, 1), np.float32)
        invc[:GPER, 0] = P["inv_cnt"][GPER * c:GPER * (c + 1)]
        m = dict(common)
        m["dinv_own"] = dvo
        m["invc_own"] = invc
        m["idxT0"] = wrap16(P["streamsT"][0][c])
        m["idxT1"] = wrap16(P["streamsT"][1][c])
        m["idxA1"] = wrap16(P["streamsA1"][c])
        m["idxB1"] = wrap16(P["streamsB1"][c])
        m["idxP"] = wrap16(P["streamsP"][c])
        in_maps.append(m)
    return in_maps


# ==== bass kernel ====
from contextlib import ExitStack

import concourse.bass as bass
import concourse.bacc as bacc
import concourse.mybir as mybir
from concourse.tile import TileContext
from concourse.masks import make_identity

F32 = mybir.dt.float32
F16 = mybir.dt.float16
F8 = mybir.dt.float8e4
I16 = mybir.dt.int16
AX = mybir.AxisListType
AF = mybir.ActivationFunctionType
OP = mybir.AluOpType

D_IN, D1, D2, D3 = 37, 64, 128, 64
GW = 8
SUPER_COLS = 64


def make_supers(SAu, SBu, flush_blocks=()):
    """Group consecutive blocks with combined cols <= SUPER_COLS; force a
    group boundary at each block in flush_blocks."""
    supers = []
    k0 = 0
    na = nb = 0
    for k in range(NBLK):
        if k > k0 and (na + SAu[k] > SUPER_COLS or nb + SBu[k] > SUPER_COLS
                       or k - k0 >= 16 or k in flush_blocks):
            supers.append((k0, k))
            k0, na, nb = k, 0, 0
        na += SAu[k]
        nb += SBu[k]
    supers.append((k0, NBLK))
    if supers[-1][1] - supers[-1][0] > 1:
        k0, k1 = supers[-1]
        supers[-1] = (k0, k1 - 1)
        supers.append((k1 - 1, k1))
    return supers


def build(S, debug=False):
    SR = [[list(map(int, S["SR"][t][r])) for r in range(RQ)] for t in range(2)]
    SA1 = list(map(int, S["SA1"]))
    SB1 = list(map(int, S["SB1"]))
    SPu = list(map(int, S["SPu"]))
    ST = [[sum(SR[t][r][k] for r in range(RQ)) for k in range(NBLK)]
          for t in range(2)]
    lenT = [128 * sum(ST[t]) for t in range(2)]
    lenA1, lenB1, lenP = 128 * sum(SA1), 128 * sum(SB1), 128 * sum(SPu)
    supers1 = make_supers(SA1, SB1, flush_blocks=(CHUNK_BLK[1][0],))
    supersT = [make_supers(ST[t], ST[t], flush_blocks=(CHUNK_BLK[1][0],))
               for t in range(2)]

    nc = bacc.Bacc()

    # ---------------- IO ----------------
    xdp = nc.dram_tensor("xdp", [N + 2, 64], F32, kind="ExternalInput")
    dinv_own = nc.dram_tensor("dinv_own", [128, NBLK], F32, kind="ExternalInput")
    invc_own = nc.dram_tensor("invc_own", [128, 1], F32, kind="ExternalInput")
    idxT0 = nc.dram_tensor("idxT0", [128, lenT[0] // 16], I16, kind="ExternalInput")
    idxT1 = nc.dram_tensor("idxT1", [128, lenT[1] // 16], I16, kind="ExternalInput")
    idxA1 = nc.dram_tensor("idxA1", [128, lenA1 // 16], I16, kind="ExternalInput")
    idxB1 = nc.dram_tensor("idxB1", [128, lenB1 // 16], I16, kind="ExternalInput")
    idxP = nc.dram_tensor("idxP", [128, lenP // 16], I16, kind="ExternalInput")
    W1 = nc.dram_tensor("W1", [D_IN, D1], F32, kind="ExternalInput")
    W2 = nc.dram_tensor("W2", [D1, D2], F32, kind="ExternalInput")
    W3 = nc.dram_tensor("W3", [D2, D3], F32, kind="ExternalInput")
    Wf1 = nc.dram_tensor("Wf1", [64, 128], F32, kind="ExternalInput")
    Wf2 = nc.dram_tensor("Wf2", [128, 64], F32, kind="ExternalInput")
    b1r = nc.dram_tensor("b1r", [128, D1], F32, kind="ExternalInput")
    b2r = nc.dram_tensor("b2r", [128, D2], F32, kind="ExternalInput")
    b3r = nc.dram_tensor("b3r", [128, D3], F32, kind="ExternalInput")
    bf1r = nc.dram_tensor("bf1r", [128, 128], F32, kind="ExternalInput")
    bf2r = nc.dram_tensor("bf2r", [128, 64], F32, kind="ExternalInput")
    gammar = nc.dram_tensor("gammar", [128, 64], F32, kind="ExternalInput")
    betar = nc.dram_tensor("betar", [128, 64], F32, kind="ExternalInput")

    out_slice = nc.dram_tensor("out_slice", [GPER, 64], F32, kind="ExternalOutput")

    # internal DRAM: fp8 quad tables per chunk
    h1own = [nc.dram_tensor(f"h1own{t}", [1 + NQUADS[t], 256], F8)
             for t in range(2)]
    T1 = [nc.dram_tensor(f"T1_{t}", [1 + NC * NQUADS[t], 256], F8,
                         addr_space="Shared") for t in range(2)]
    hW3own = [nc.dram_tensor(f"hW3own{t}", [1 + NQUADS[t], 256], F8)
              for t in range(2)]
    T3 = [nc.dram_tensor(f"T3_{t}", [1 + NC * NQUADS[t], 256], F8,
                         addr_space="Shared") for t in range(2)]
    h3ot = nc.dram_tensor("h3ot", [1 + SHARD, D3], F32)
    poolin = nc.dram_tensor("poolin", [G, 64], F32)
    rs_out = nc.dram_tensor("rs_out", [GPER, 64], F32)

    dbg = {}
    if debug:
        for name, shape, dt in [
                ("d_h1own0", [1 + NQUADS[0], 256], F8),
                ("d_h1own1", [1 + NQUADS[1], 256], F8),
                ("d_T1_0", [1 + NC * NQUADS[0], 256], F8),
                ("d_hW3own0", [1 + NQUADS[0], 256], F8),
                ("d_h3", [SHARD, D3], F32),
                ("d_poolin", [G, 64], F32), ("d_rs", [GPER, 64], F32)]:
            dbg[name] = nc.dram_tensor(name, shape, dt, kind="ExternalOutput")

    rg = [list(range(NC))]

    with TileContext(nc) as tc, ExitStack() as ctx:
        const = ctx.enter_context(tc.tile_pool(name="const", bufs=1))
        idxp = ctx.enter_context(tc.tile_pool(name="idxp", bufs=1))

        ident = const.tile([128, 128], F32, tag="ident")
        make_identity(nc, ident[:])
        zt = const.tile([128, 128], F32, tag="zt")
        nc.vector.memset(zt[:], 0.0)
        zth = const.tile([128, 128], F16, tag="zth")
        nc.vector.memset(zth[:], 0.0)
        zf8 = const.tile([128, 256], F8, tag="zf8")
        nc.vector.memset(zf8[:], 0.0)
        epss = const.tile([128, 1], F32, tag="epss")
        nc.vector.memset(epss[:], 1e-5)

        def csb(t, p0, p1, tag, dt=F32):
            tl = const.tile([p0, p1], dt, tag=tag)
            nc.sync.dma_start(out=tl[:], in_=t[:])
            return tl

        W1s = csb(W1, D_IN, D1, "W1s")
        W2s = csb(W2, D1, D2, "W2s")
        W3s = csb(W3, D2, D3, "W3s")
        Wf1s = csb(Wf1, 64, 128, "Wf1s")
        Wf2s = csb(Wf2, 128, 64, "Wf2s")
        b1s = csb(b1r, 128, D1, "b1s")
        b2s = csb(b2r, 128, D2, "b2s")
        b3s = csb(b3r, 128, D3, "b3s")
        bf1s = csb(bf1r, 128, 128, "bf1s")
        bf2s = csb(bf2r, 128, 64, "bf2s")
        gams = csb(gammar, 128, 64, "gams")
        bets = csb(betar, 128, 64, "bets")
        dvo = csb(dinv_own, 128, NBLK, "dvo")
        ics = csb(invc_own, 128, 1, "ics")

        idxT_s = []
        for t in range(2):
            tl = idxp.tile([128, lenT[t] // 16], I16, tag=f"it{t}")
            nc.sync.dma_start(out=tl[:], in_=[idxT0, idxT1][t][:])
            idxT_s.append(tl)
        idxA1_s = idxp.tile([128, lenA1 // 16], I16, tag="ia1")
        nc.sync.dma_start(out=idxA1_s[:], in_=idxA1[:])
        idxB1_s = idxp.tile([128, lenB1 // 16], I16, tag="ib1")
        nc.sync.dma_start(out=idxB1_s[:], in_=idxB1[:])
        idxP_s = idxp.tile([128, lenP // 16], I16, tag="ip")
        nc.sync.dma_start(out=idxP_s[:], in_=idxP[:])

        # guard rows
        for t in range(2):
            nc.scalar.dma_start(out=h1own[t][0:1, :], in_=zf8[0:1, :])
            nc.scalar.dma_start(out=T1[t][0:1, :], in_=zf8[0:1, :])
            nc.scalar.dma_start(out=hW3own[t][0:1, :], in_=zf8[0:1, :])
            nc.scalar.dma_start(out=T3[t][0:1, :], in_=zf8[0:1, :])
        nc.scalar.dma_start(out=h3ot[0:1, :], in_=zt[0:1, 0:D3])

        # ---- fp8 quad-table store sink ---------------------------------
        def make_pair_store_sink(dsts, pname):
            """Collects per-block [128, 64] tiles (stored as f8); flushes
            groups of up to 8 blocks into the chunk quad tables. Block
            groups never straddle the chunk boundary."""
            state = {"tile": None, "k0": None, "n": 0}
            pool = ctx.enter_context(tc.tile_pool(name=pname, bufs=2))

            def flush():
                tl, k0_, n_ = state["tile"], state["k0"], state["n"]
                if tl is None or n_ == 0:
                    return
                t = 0 if k0_ < CHUNK_BLK[1][0] else 1
                dst = dsts[t]
                qr0 = 32 * (k0_ - CHUNK_BLK[t][0])
                rows_total = min(SHARD, 128 * (k0_ + n_)) - 128 * k0_
                full = n_ - 1 if rows_total < 128 * n_ else n_
                if full > 0:
                    nc.scalar.dma_start(
                        out=dst[1 + qr0:1 + qr0 + 32 * full, :].rearrange(
                            "(j q) (r f) -> (q r) j f", q=32, r=4),
                        in_=tl[:, :full * 64].rearrange("p (j f) -> p j f", j=full))
                if full < n_:
                    rl = rows_total - 128 * full          # 106 for last block
                    nq = rl // 4                          # 26 full quads
                    r0 = 1 + qr0 + 32 * full
                    nc.scalar.dma_start(
                        out=dst[r0:r0 + nq, :].rearrange(
                            "q (r f) -> (q r) f", r=4),
                        in_=tl[:4 * nq, full * 64:(full + 1) * 64])
                    rem = rl - 4 * nq                     # 2 leftover rows
                    if rem:
                        nc.scalar.dma_start(
                            out=dst[r0 + nq:r0 + nq + 1, 0:rem * 64].rearrange(
                                "q (r f) -> (q r) f", r=rem),
                            in_=tl[4 * nq:rl, full * 64:(full + 1) * 64])
                state["tile"], state["k0"], state["n"] = None, None, 0

            def sink_store(k, rows, res, func=AF.Copy, scale=1.0):
                if state["tile"] is None:
                    state["tile"] = pool.tile([128, 8 * 64], F8, tag="stg",
                                              name=pname + "_stg")
                    state["k0"], state["n"] = k, 0
                j = state["n"]
                nc.scalar.activation(
                    state["tile"][:, j * 64:(j + 1) * 64], res[:], func,
                    scale=scale)
                state["n"] += 1
                if state["n"] == 8 or k == CHUNK_BLK[0][1] - 1:
                    flush()

            return sink_store, flush

        # ---- plain-row store sink (h3ot) -------------------------------
        def make_store_sink(dst, fout, dt, pname, row_off=0):
            state = {"tile": None, "k0": None, "n": 0}
            pool = ctx.enter_context(tc.tile_pool(name=pname, bufs=2))

            def flush():
                t, k0_, n_ = state["tile"], state["k0"], state["n"]
                if t is None or n_ == 0:
                    return
                row0 = row_off + 128 * k0_
                nrows = min(SHARD, 128 * (k0_ + n_)) - 128 * k0_
                full = n_ - 1 if nrows < 128 * n_ else n_
                if full > 0:
                    nc.scalar.dma_start(
                        out=dst[row0:row0 + 128 * full, :].rearrange(
                            "(j p) f -> p j f", p=128),
                        in_=t[:, :full * fout].rearrange("p (j f) -> p j f", j=full))
                if full < n_:
                    rl = nrows - 128 * full
                    nc.scalar.dma_start(
                        out=dst[row0 + 128 * full:row0 + nrows, :],
                        in_=t[:rl, full * fout:(full + 1) * fout])
                state["tile"], state["k0"], state["n"] = None, None, 0

            def sink_store(k, rows, res, func=AF.Copy, scale=1.0):
                if state["tile"] is None:
                    state["tile"] = pool.tile([128, 8 * fout], dt, tag="stg",
                                              name=pname + "_stg")
                    state["k0"], state["n"] = k, 0
                j = state["n"]
                nc.scalar.activation(
                    state["tile"][:, j * fout:(j + 1) * fout], res[:], func,
                    scale=scale)
                state["n"] += 1
                if state["n"] == 8:
                    flush()

            return sink_store, flush

        # ---- L1 gather driver (A1/B1 f32 windows, like v2 main pass) ----
        def fold_plain(gt, elem, c0, s, upto=1):
            while s > upto:
                h = min(s // 2, s - upto)
                nc.vector.tensor_tensor(
                    out=gt[:, c0 * elem:(c0 + h) * elem],
                    in0=gt[:, c0 * elem:(c0 + h) * elem],
                    in1=gt[:, (c0 + s - h) * elem:(c0 + s) * elem],
                    op=OP.add)
                s -= h

        def l1_layer(sink, mid_hook=None, park_ms=0.0):
            elem = 64
            with tc.tile_pool(name="gta", bufs=3) as gta_pool, \
                 tc.tile_pool(name="gtb", bufs=3) as gtb_pool, \
                 tc.tile_pool(name="hacc", bufs=4) as hacc_pool, \
                 ExitStack() as lctx:
                offA = offB = 0
                for (k0, k1) in supers1:
                    if k0 == CHUNK_BLK[1][0] and mid_hook is not None:
                        mid_hook()
                        if park_ms > 0:
                            lctx.enter_context(tc.tile_wait_until(park_ms))
                    nA = sum(SA1[k0:k1])
                    nB = sum(SB1[k0:k1])
                    gtA = gta_pool.tile([128, SUPER_COLS * elem], F32, tag="gtA",
                                        name="gtA")
                    done = 0
                    while done < nA:
                        w = min(GW, nA - done)
                        nc.gpsimd.dma_gather(
                            out_ap=gtA[:, done * elem:(done + w) * elem].rearrange(
                                "p (c f) -> p c f", c=w),
                            in_ap=xdp[:],
                            idxs_ap=idxA1_s[:, 8 * (offA + done):8 * (offA + done + w)],
                            num_idxs=128 * w, num_idxs_reg=128 * w,
                            elem_size=elem)
                        done += w
                    gtB = gtb_pool.tile([128, SUPER_COLS * elem], F32, tag="gtB",
                                        name="gtB")
                    done = 0
                    while done < nB:
                        w = min(GW, nB - done)
                        nc.gpsimd.dma_gather(
                            out_ap=gtB[:, done * elem:(done + w) * elem].rearrange(
                                "p (c f) -> p c f", c=w),
                            in_ap=xdp[BASE_B:, :],
                            idxs_ap=idxB1_s[:, 8 * (offB + done):8 * (offB + done + w)],
                            num_idxs=128 * w, num_idxs_reg=128 * w,
                            elem_size=elem)
                        done += w
                    cA = cB = 0
                    for k in range(k0, k1):
                        sa, sb_ = SA1[k], SB1[k]
                        rows = min(128, SHARD - 128 * k)
                        acc = hacc_pool.tile([128, elem], F32, tag="acc")
                        fold_plain(gtA, elem, cA, sa, upto=2)
                        if sa >= 2:
                            nc.vector.tensor_tensor(
                                out=acc[:],
                                in0=gtA[:, cA * elem:(cA + 1) * elem],
                                in1=gtA[:, (cA + 1) * elem:(cA + 2) * elem],
                                op=OP.add)
                        else:
                            nc.vector.tensor_copy(
                                out=acc[:], in_=gtA[:, cA * elem:(cA + 1) * elem])
                        fold_plain(gtB, elem, cB, sb_, upto=1)
                        nc.vector.tensor_tensor(
                            out=acc[:], in0=acc[:],
                            in1=gtB[:, cB * elem:(cB + 1) * elem], op=OP.add)
                        cA += sa
                        cB += sb_
                        sink(k, rows, acc)
                    offA += nA
                    offB += nB

        # ---- fp8 quad-table gather layer (two passes over chunk tables) ----
        def pair_layer(tabs, idx_tiles, sink, mid_hook=None, park_ms=0.0):
            """tabs: [T_0, T_1] dram quad tables; sink(k, rows, acc_f32_64).
            Fold: sequential accumulation into an f32 target so fp8 never
            holds a partial sum."""
            with tc.tile_pool(name="accv", bufs=1) as accv_pool, \
                 tc.tile_pool(name="ftmp", bufs=4) as ftmp_pool, \
                 tc.tile_pool(name="hacc", bufs=4) as hacc_pool:
                accV = accv_pool.tile([128, NBLK * 64], F32, tag="accV")
                for t in range(2):
                    with tc.tile_pool(name=f"gt{t}", bufs=3) as gt_pool, \
                         ExitStack() as lctx:
                        off = 0
                        for (k0, k1) in supersT[t]:
                            if (t == 1 and k0 == CHUNK_BLK[1][0]
                                    and mid_hook is not None):
                                mid_hook()
                                if park_ms > 0:
                                    lctx.enter_context(
                                        tc.tile_wait_until(park_ms))
                            ncols = sum(ST[t][k0:k1])
                            gt = gt_pool.tile([128, SUPER_COLS * 256], F8,
                                              tag="gt", name=f"gt{t}")
                            done = 0
                            while done < ncols:
                                w = min(GW, ncols - done)
                                nc.gpsimd.dma_gather(
                                    out_ap=gt[:, done * 256:(done + w) * 256].rearrange(
                                        "p (c f) -> p c f", c=w),
                                    in_ap=tabs[t][:],
                                    idxs_ap=idx_tiles[t][:, 8 * (off + done):8 * (off + done + w)],
                                    num_idxs=128 * w, num_idxs_reg=128 * w,
                                    elem_size=256)
                                done += w
                            c0 = 0
                            g3 = gt[:].rearrange("p (c f) -> p c f", f=256)
                            for k in range(k0, k1):
                                rows = min(128, SHARD - 128 * k)
                                cols = []
                                cc = c0
                                for r in range(RQ):
                                    for i in range(SR[t][r][k]):
                                        cols.append((cc + i, r * 64))
                                    cc += SR[t][r][k]
                                if t == 0:
                                    target = accV[:, k * 64:(k + 1) * 64]
                                else:
                                    acc = hacc_pool.tile([128, 64], F32, tag="acc")
                                    target = acc[:]
                                # pairwise f8+f8 -> f32 temp (independent ops),
                                # then wide f32 tree fold: short DVE chain.
                                npair = len(cols) // 2
                                odd = len(cols) % 2
                                tmp = ftmp_pool.tile([128, max(npair, 2) * 64],
                                                     F32, tag="ftmp")
                                for j in range(npair):
                                    (iA, oA), (iB, oB) = cols[2 * j], cols[2 * j + 1]
                                    nc.vector.tensor_tensor(
                                        out=tmp[:, j * 64:(j + 1) * 64],
                                        in0=g3[:, iA:iA + 1, oA:oA + 64],
                                        in1=g3[:, iB:iB + 1, oB:oB + 64],
                                        op=OP.add)
                                fold_plain(tmp, 64, 0, npair, upto=2)
                                nc.vector.tensor_tensor(
                                    out=target, in0=tmp[:, 0:64],
                                    in1=tmp[:, 64:128], op=OP.add)
                                if odd:
                                    (ci, o) = cols[-1]
                                    nc.vector.tensor_tensor(
                                        out=target, in0=target,
                                        in1=g3[:, ci:ci + 1, o:o + 64], op=OP.add)
                                if t == 1:
                                    nc.vector.tensor_tensor(
                                        out=target, in0=target,
                                        in1=accV[:, k * 64:(k + 1) * 64], op=OP.add)
                                    sink(k, rows, acc)
                                c0 = cc
                            off += ncols

        # ================= L1 =================
        store1, flush1 = make_pair_store_sink(h1own, "st1")
        with tc.tile_pool(name="tp1", bufs=2, space="PSUM") as tp1, \
             tc.tile_pool(name="mm0", bufs=2, space="PSUM") as mm0, \
             tc.tile_pool(name="hT1", bufs=4) as hT1p, \
             tc.tile_pool(name="h1b", bufs=4) as h1bp:
            def sink1(k, rows, agg):
                # table = dvo * relu(dvo*(agg@W1) + b1); dvo passes through
                # the matmul (per-row scale), relu(s*x)=s*relu(x) for s>0.
                tp0 = tp1.tile([64, 128], F32, tag="tp0", name="tp0")
                nc.tensor.transpose(out=tp0[:], in_=agg[:], identity=ident[:])
                aT = hT1p.tile([64, 128], F32, tag="aT", name="aT")
                nc.scalar.activation(aT[:], tp0[:], AF.Copy)
                ps0 = mm0.tile([128, D1], F32, tag="ps0", name="ps0")
                nc.tensor.matmul(out=ps0[:], lhsT=aT[:D_IN, :], rhs=W1s[:],
                                 start=True, stop=True)
                h = h1bp.tile([128, D1], F32, tag="h1t", name="h1t")
                nc.vector.tensor_tensor(
                    out=h[:], in0=ps0[:],
                    in1=dvo[:, k:k + 1].to_broadcast([128, D1]), op=OP.mult)
                nc.vector.tensor_tensor(out=h[:], in0=h[:], in1=b1s[:], op=OP.add)
                store1(k, rows, h, func=AF.Relu, scale=dvo[:, k:k + 1])

            def mid1():
                flush1()
                with tc.high_priority():
                    nc.gpsimd.collective_compute(
                        "AllGather", OP.bypass, replica_groups=rg,
                        ins=[h1own[0][1:, :]], outs=[T1[0][1:, :]])
            l1_layer(sink1, mid_hook=mid1, park_ms=0.068)
            flush1()
        with tc.high_priority():
            nc.gpsimd.collective_compute(
                "AllGather", OP.bypass, replica_groups=rg,
                ins=[h1own[1][1:, :]], outs=[T1[1][1:, :]])

        # ================= L2 =================
        store2, flush2 = make_pair_store_sink(hW3own, "st2")
        with tc.tile_pool(name="tp2", bufs=2, space="PSUM") as tp2, \
             tc.tile_pool(name="mm2", bufs=2, space="PSUM") as mm2, \
             tc.tile_pool(name="mm3", bufs=2, space="PSUM") as mm3, \
             tc.tile_pool(name="hT2", bufs=4) as hT2p, \
             tc.tile_pool(name="h2b", bufs=4) as h2bp:
            def sink2(k, rows, acc):
                # h2' = dvo*relu(dvo*(acc@W2) + b2) = relu(dvo*(dvo*(acc@W2)+b2));
                # store f8((h2' @ W3))
                tp0 = tp2.tile([64, 128], F32, tag="tp0", name="tp0")
                nc.tensor.transpose(out=tp0[:], in_=acc[:], identity=ident[:])
                aT = hT2p.tile([64, 128], F32, tag="aT", name="aT")
                nc.scalar.activation(aT[:], tp0[:], AF.Copy)
                ps = mm2.tile([128, D2], F32, tag="ps", name="ps")
                nc.tensor.matmul(out=ps[:], lhsT=aT[:], rhs=W2s[:],
                                 start=True, stop=True)
                h = h2bp.tile([128, D2], F32, tag="h2t", name="h2t")
                nc.vector.tensor_tensor(
                    out=h[:], in0=ps[:],
                    in1=dvo[:, k:k + 1].to_broadcast([128, D2]), op=OP.mult)
                nc.vector.tensor_tensor(out=h[:], in0=h[:], in1=b2s[:], op=OP.add)
                nc.scalar.activation(h[:], h[:], AF.Relu,
                                     scale=dvo[:, k:k + 1])
                tp = tp2.tile([D2, 128], F32, tag="tp", name="tp")
                nc.tensor.transpose(out=tp[:], in_=h[:], identity=ident[:])
                hT = hT2p.tile([D2, 128], F32, tag="hT", name="hT")
                nc.vector.tensor_copy(out=hT[:], in_=tp[:])
                ps3 = mm3.tile([128, D3], F32, tag="ps3", name="ps3")
                nc.tensor.matmul(out=ps3[:], lhsT=hT[:], rhs=W3s[:],
                                 start=True, stop=True)
                store2(k, rows, ps3)

            def mid2():
                flush2()
                with tc.high_priority():
                    nc.gpsimd.collective_compute(
                        "AllGather", OP.bypass, replica_groups=rg,
                        ins=[hW3own[0][1:, :]], outs=[T3[0][1:, :]])
            pair_layer(T1, idxT_s, sink2, mid_hook=mid2, park_ms=0.27)
            flush2()
        with tc.high_priority():
            nc.gpsimd.collective_compute(
                "AllGather", OP.bypass, replica_groups=rg,
                ins=[hW3own[1][1:, :]], outs=[T3[1][1:, :]])

        # ================= L3 =================
        storeh3, flushh3 = make_store_sink(h3ot, D3, F32, "sth3", row_off=1)

        def sink3(k, rows, acc):
            nc.vector.tensor_tensor(
                out=acc[:], in0=acc[:],
                in1=dvo[:, k:k + 1].to_broadcast([128, D3]), op=OP.mult)
            nc.vector.tensor_tensor(out=acc[:], in0=acc[:], in1=b3s[:], op=OP.add)
            storeh3(k, rows, acc, func=AF.Relu)
        pair_layer(T3, idxT_s, sink3)
        flushh3()

        # ================= pooling =================
        with tc.tile_pool(name="gp", bufs=1) as gp, \
             tc.tile_pool(name="pstg", bufs=1) as pstg:
            stg = pstg.tile([128, GBLK * 64], F32, tag="pstg")
            spT = sum(SPu)
            gt = gp.tile([128, spT * 64], F32, tag="gtp")
            done = 0
            while done < spT:
                w = min(GW, spT - done)
                nc.gpsimd.dma_gather(
                    out_ap=gt[:, done * 64:(done + w) * 64].rearrange(
                        "p (c f) -> p c f", c=w),
                    in_ap=h3ot[:],
                    idxs_ap=idxP_s[:, 8 * done:8 * (done + w)],
                    num_idxs=128 * w, num_idxs_reg=128 * w, elem_size=64)
                done += w
            offP = 0
            for gb in range(GBLK):
                sp = SPu[gb]
                fold_plain(gt, 64, offP, sp, upto=2)
                nc.vector.tensor_tensor(
                    out=stg[:, 64 * gb:64 * (gb + 1)],
                    in0=gt[:, offP * 64:(offP + 1) * 64],
                    in1=gt[:, (offP + 1) * 64:(offP + 2) * 64], op=OP.add)
                offP += sp
            nc.scalar.dma_start(
                out=poolin[:, :].rearrange("(j p) f -> p j f", p=GPER),
                in_=stg[:GPER, :].rearrange("p (j f) -> p j f", j=GBLK))

        nc.gpsimd.collective_compute(
            "ReduceScatter", OP.add, replica_groups=rg,
            ins=[poolin[:]], outs=[rs_out[:]])

        # ================= MLP + LayerNorm =================
        with tc.tile_pool(name="mlp", bufs=1) as mlp, \
             tc.tile_pool(name="mps", bufs=2, space="PSUM") as mps:
            gtl = mlp.tile([128, 64], F32, tag="g0")
            nc.vector.memset(gtl[:], 0.0)
            nc.sync.dma_start(out=gtl[:GPER, :], in_=rs_out[:])
            nc.vector.tensor_tensor(
                out=gtl[:], in0=gtl[:],
                in1=ics[:].to_broadcast([128, 64]), op=OP.mult)
            tp = mps.tile([64, 128], F32, tag="t1")
            nc.tensor.transpose(out=tp[:], in_=gtl[:], identity=ident[:])
            gT = mlp.tile([64, 128], F32, tag="gT")
            nc.vector.tensor_copy(out=gT[:], in_=tp[:])
            p1 = mps.tile([128, 128], F32, tag="p1")
            nc.tensor.matmul(out=p1[:], lhsT=gT[:], rhs=Wf1s[:],
                             start=True, stop=True)
            g1t = mlp.tile([128, 128], F32, tag="g1t")
            nc.vector.tensor_tensor(out=g1t[:], in0=p1[:], in1=bf1s[:], op=OP.add)
            nc.scalar.activation(g1t[:], g1t[:], AF.Relu)
            tp2_ = mps.tile([128, 128], F32, tag="t2")
            nc.tensor.transpose(out=tp2_[:], in_=g1t[:], identity=ident[:])
            g1T = mlp.tile([128, 128], F32, tag="g1T")
            nc.vector.tensor_copy(out=g1T[:], in_=tp2_[:])
            p2 = mps.tile([128, 64], F32, tag="p2")
            nc.tensor.matmul(out=p2[:], lhsT=g1T[:], rhs=Wf2s[:],
                             start=True, stop=True)
            g2t = mlp.tile([128, 64], F32, tag="g2t")
            nc.vector.tensor_tensor(out=g2t[:], in0=p2[:], in1=bf2s[:], op=OP.add)
            nc.scalar.activation(g2t[:], g2t[:], AF.Relu)
            mu = mlp.tile([128, 1], F32, tag="mu")
            nc.vector.reduce_sum(mu[:], g2t[:], axis=AX.X)
            nc.vector.tensor_scalar_mul(mu[:], in0=mu[:], scalar1=1.0 / 64)
            xm = mlp.tile([128, 64], F32, tag="xm")
            nc.vector.tensor_tensor(out=xm[:], in0=g2t[:],
                                    in1=mu[:].to_broadcast([128, 64]),
                                    op=OP.subtract)
            sq = mlp.tile([128, 64], F32, tag="sq")
            nc.vector.tensor_tensor(out=sq[:], in0=xm[:], in1=xm[:], op=OP.mult)
            var = mlp.tile([128, 1], F32, tag="var")
            nc.vector.reduce_sum(var[:], sq[:], axis=AX.X)
            rstd = mlp.tile([128, 1], F32, tag="rstd")
            nc.vector.tensor_scalar_mul(var[:], in0=var[:], scalar1=1.0 / 64)
            nc.vector.tensor_tensor(out=var[:], in0=var[:], in1=epss[:],
                                    op=OP.add)
            nc.scalar.activation(rstd[:], var[:], AF.Sqrt)
            nc.vector.reciprocal(rstd[:], rstd[:])
            nc.vector.tensor_tensor(out=xm[:], in0=xm[:],
                                    in1=rstd[:].to_broadcast([128, 64]),
                                    op=OP.mult)
            nc.vector.tensor_tensor(out=xm[:], in0=xm[:], in1=gams[:], op=OP.mult)
            nc.vector.tensor_tensor(out=xm[:], in0=xm[:], in1=bets[:], op=OP.add)
            nc.sync.dma_start(out=out_slice[:, :], in_=xm[:GPER, :])

        if debug:
            with tc.tile_pool(name="dbg", bufs=2) as dp:
                def dump(srct, dstt, nrows, width, dt=F32):
                    for c in range((nrows + 127) // 128):
                        rows = min(128, nrows - 128 * c)
                        tl = dp.tile([128, width], dt, tag="dt")
                        nc.sync.dma_start(out=tl[:rows, :],
                                          in_=srct[128 * c:128 * c + rows, :])
                        nc.sync.dma_start(out=dstt[128 * c:128 * c + rows, :],
                                          in_=tl[:rows, :])
                dump(h1own[0], dbg["d_h1own0"], 1 + NPAIRS[0], 128, F16)
                dump(h1own[1], dbg["d_h1own1"], 1 + NPAIRS[1], 128, F16)
                dump(T1[0], dbg["d_T1_0"], 1 + NC * NPAIRS[0], 128, F16)
                dump(hW3own[0], dbg["d_hW3own0"], 1 + NPAIRS[0], 128, F16)
                dump(h3ot[1:, :], dbg["d_h3"], SHARD, D3)
                dump(poolin, dbg["d_poolin"], G, 64)
                dump(rs_out, dbg["d_rs"], GPER, 64)

    nc.compile()
    nc.finalize()
    return nc


# ==== SPMD runner (same as v2) ====
import jax
from jax.sharding import Mesh, PartitionSpec
from jax.experimental.shard_map import shard_map

from concourse import bass2jax


class SpmdRunner:
    def __init__(self, nc, n_cores=8):
        bass2jax.install_neuronx_cc_hook()
        self.nc = nc
        self.n_cores = n_cores
        partition_name = nc.partition_id_tensor.name if nc.partition_id_tensor else None
        in_names, out_names, out_avals, zero_outs = [], [], [], []
        for alloc in nc.m.functions[0].allocations:
            if not isinstance(alloc, mybir.MemoryLocationSet):
                continue
            name = alloc.memorylocations[0].name
            if alloc.kind == "ExternalInput":
                if name != partition_name:
                    in_names.append(name)
            elif alloc.kind == "ExternalOutput":
                shape = tuple(alloc.tensor_shape)
                dtype = mybir.dt.np(alloc.dtype)
                out_names.append(name)
                out_avals.append(jax.core.ShapedArray(shape, dtype))
                zero_outs.append(np.zeros(shape, dtype))
        self.in_names = list(in_names)
        self.out_names = out_names
        self.out_avals = out_avals
        self.zero_outs = zero_outs
        n_params = len(in_names)
        n_outs = len(out_avals)
        all_in_names = in_names + out_names + ([partition_name] if partition_name else [])
        self.n_params = n_params

        def _body(*args):
            operands = list(args)
            if partition_name is not None:
                operands.append(bass2jax.partition_id_tensor())
            outs = bass2jax._bass_exec_p.bind(
                *operands,
                out_avals=tuple(out_avals),
                in_names=tuple(all_in_names),
                out_names=tuple(out_names),
                lowering_input_output_aliases=(),
                sim_require_finite=True,
                sim_require_nnan=True,
                nc=nc,
            )
            return tuple(outs)

        try:
            devices = jax.devices("axon")[:n_cores]
        except RuntimeError:
            devices = jax.devices()[:n_cores]
        mesh = Mesh(np.asarray(devices), ("core",))
        in_specs = (PartitionSpec("core"),) * (n_params + n_outs)
        out_specs = (PartitionSpec("core"),) * n_outs
        self.fn = jax.jit(
            shard_map(_body, mesh=mesh, in_specs=in_specs, out_specs=out_specs,
                      check_rep=False),
            keep_unused=True,
        )

    def stage(self, in_maps):
        concat = [
            np.concatenate([np.asarray(in_maps[c][n]) for c in range(self.n_cores)], axis=0)
            for n in self.in_names
        ]
        zeros = [np.zeros((self.n_cores * z.shape[0], *z.shape[1:]), z.dtype)
                 for z in self.zero_outs]
        return concat + zeros

    def run(self, staged):
        out = self.fn(*staged)
        jax.block_until_ready(out)
        return out

    def unpack(self, out_arrs):
        return [
            {
                name: np.asarray(out_arrs[i]).reshape(
                    self.n_cores, *self.out_avals[i].shape)[c]
                for i, name in enumerate(self.out_names)
            }
            for c in range(self.n_cores)
        ]


_CACHE = {}


def kernel(**inputs):
    inputs = {k: np.asarray(v) for k, v in inputs.items()}
    P = preprocess(inputs["edge_index"], inputs["batch"])
    key = (tuple(map(tuple, P["SR"].reshape(2 * RQ, -1))),
           tuple(P["SA1"].tolist()), tuple(P["SB1"].tolist()),
           tuple(P["SPu"].tolist()))
    if key not in _CACHE:
        S = {k: P[k] for k in S_KEYS}
        nc = build(S, debug=False)
        _CACHE[key] = SpmdRunner(nc, 8)
    r = _CACHE[key]
    in_maps = make_in_maps(inputs, P)
    staged = r.stage(in_maps)
    res = r.unpack(r.run(staged))
    return np.ascontiguousarray(
        np.concatenate([res[c]["out_slice"] for c in range(NC)], axis=0),
        dtype=np.float32)


# revision 8
# speedup vs baseline: 1.0975x; 1.0156x over previous
"""Trainium2 Bass kernel for nn_LinkerEncoder — v3.

Structure vs v2:
- Tables between layers are fp16 PAIR-PACKED: row = two consecutive
  positions' 64-wide vectors (256B = min gather elem). AllGather ships half
  the bytes of v2 (6.4MB -> 2x ~3.2MB chunks).
- Node classes = (chunk, parity): position chunk 0 = blocks 0..24 (3200
  pos), chunk 1 = blocks 25..48 (3050). Host greedily balances classes per
  dest so the per-block per-class slot maxes stay tight.
- L2/L3 gather streams are flat per chunk: per block [E cols][O cols], one
  gather table per chunk; fold reads the wanted 64-wide half via strided
  views (parity known at build time).
- Each AllGather is split into 2 chunk collectives: chunk-0 fires mid-way
  through the producing layer and overlaps its tail; the consuming layer's
  chunk-0 pass overlaps the chunk-1 collective (pre-pass into an
  accumulator, like v2's O-pre-pass).
- L1 keeps the v2 A1/B1 int16-window streams over the f32 xdp table.
"""
import numpy as np

N = 50000
NC = 8
SHARD = N // NC          # 6250
NBLK = (SHARD + 127) // 128   # 49
BASE_B = 17234
B_PAD_IDX = N + 1 - BASE_B
CLS_OF_CORE = np.array([0, 0, 0, 1, 1, 2, 2, 2], np.int32)
G = 1000
GBLK = 8
GPER = G // GBLK

CHUNK_POS = [(0, 3200), (3200, 6250)]
CHUNK_BLK = [(0, 25), (25, 49)]
RQ = 4                       # positions packed per 256B fp8 table row
NQUADS = [800, 763]          # ceil(chunk_size / 4); last quad of chunk 1 is half
S_KEYS = ("SR", "SA1", "SB1", "SPu")


def wrap16(stream):
    L = len(stream)
    w = np.asarray(stream, np.int32).reshape(L // 16, 16).T.astype(np.int16)
    return np.tile(w, (8, 1))


def preprocess(edge_index, batch):
    src = np.asarray(edge_index[0], np.int64)
    dst = np.asarray(edge_index[1], np.int64)
    batch = np.asarray(batch, np.int64)

    indeg = np.bincount(dst, minlength=N).astype(np.int64) + 1
    dinv = 1.0 / np.sqrt(indeg.astype(np.float64))

    rank = np.argsort(-indeg, kind="stable")
    core_of = np.empty(N, np.int32)
    core_of[rank] = np.arange(N, dtype=np.int32) % NC

    order_d = np.argsort(dst, kind="stable")
    s_sorted = src[order_d]
    d_sorted = dst[order_d]
    d_starts = np.searchsorted(d_sorted, np.arange(N))
    d_ends = np.searchsorted(d_sorted, np.arange(N) + 1)

    order_s = np.argsort(src, kind="stable")
    d_by_s = dst[order_s]
    s_starts = np.searchsorted(src[order_s], np.arange(N))
    s_ends = np.searchsorted(src[order_s], np.arange(N) + 1)

    # ---- class assignment: K=8 (chunk x quad-slot), greedy balance per dest
    K = 8
    cnt = np.zeros((N, K), np.int32)
    cap = np.zeros((NC, K), np.int64)
    for t in range(2):
        size = CHUNK_POS[t][1] - CHUNK_POS[t][0]
        for r in range(RQ):
            # positions base+4q+r exist while 4q+r < size
            cap[:, RQ * t + r] = (size - r + RQ - 1) // RQ
    cls = np.empty(N, np.int32)
    outdeg = (s_ends - s_starts) + 1
    proc = np.argsort(-outdeg, kind="stable")
    BIG = 1 << 30
    for v in proc:
        c = core_of[v]
        ds = np.concatenate([d_by_s[s_starts[v]:s_ends[v]], [v]])
        loads = cnt[ds]
        mx = loads.max(axis=1, keepdims=True)
        score = (loads >= mx).sum(axis=0) * 1000 + loads.sum(axis=0)
        score = np.where(cap[c] > 0, score, BIG)
        k = int(np.argmin(score))
        cls[v] = k
        cap[c, k] -= 1
        cnt[ds, k] += 1

    # exact per-dest per-class counts (greedy's cnt drops multi-edge dups)
    cnt = np.zeros((N, K), np.int32)
    for k in range(K):
        cnt[:, k] = np.bincount(dst[cls[src] == k], minlength=N)
    cnt[np.arange(N), cls] += 1  # self loop

    # ---- L1 window balance (flat counts incl self)
    own_cls = CLS_OF_CORE[core_of]
    cls1 = CLS_OF_CORE[core_of[src]]

    def balance(sel_dst, sel_cls, extra_a, extra_l, extra_b):
        fa = np.bincount(sel_dst, weights=(sel_cls == 0).astype(np.float64),
                         minlength=N).astype(np.int64) + extra_a
        fl = np.bincount(sel_dst, weights=(sel_cls == 1).astype(np.float64),
                         minlength=N).astype(np.int64) + extra_l
        fb = np.bincount(sel_dst, weights=(sel_cls == 2).astype(np.float64),
                         minlength=N).astype(np.int64) + extra_b
        x = np.clip((fb - fa + fl + 1) // 2, 0, fl)
        return fa + x, fb + fl - x, x

    a1_cnt, b1_cnt, xflex1 = balance(
        dst, cls1, (own_cls == 0).astype(np.int64),
        (own_cls == 1).astype(np.int64), (own_cls == 2).astype(np.int64))

    # ---- position assignment
    mxK = cnt.max(axis=1)
    mx1 = np.maximum(a1_cnt, b1_cnt)
    pos_of = np.empty(N, np.int64)
    for c in range(NC):
        for t in range(2):
            base = CHUNK_POS[t][0]
            for r in range(RQ):
                k = RQ * t + r
                nodes = np.where((core_of == c) & (cls == k))[0]
                nodes = nodes[np.lexsort((mx1[nodes], mxK[nodes]))]
                pos_of[nodes] = SHARD * c + base + RQ * np.arange(len(nodes)) + r
    node_at = np.empty(N, np.int64)
    node_at[pos_of] = np.arange(N)

    local_of = pos_of % SHARD
    blk_of = local_of // 128
    SR = np.zeros((2, RQ, NBLK), np.int64)
    SA1 = np.zeros(NBLK, np.int64)
    SB1 = np.zeros(NBLK, np.int64)
    for k in range(NBLK):
        sel = blk_of == k
        for t in range(2):
            for r in range(RQ):
                SR[t, r, k] = max(int(cnt[sel, RQ * t + r].max()), 1)
        SA1[k] = max(int(a1_cnt[sel].max()), 1)
        SB1[k] = max(int(b1_cnt[sel].max()), 1)

    # ---- streams
    lenT = [int(SR[t].sum()) * 128 for t in range(2)]
    streamsT = [np.zeros((NC, lenT[t]), np.int32) for t in range(2)]
    lenA1 = int(SA1.sum()) * 128
    lenB1 = int(SB1.sum()) * 128
    streamsA1 = np.zeros((NC, lenA1), np.int32)
    streamsB1 = np.full((NC, lenB1), B_PAD_IDX, np.int32)

    chunk_of_local = (local_of >= CHUNK_POS[1][0]).astype(np.int64)
    quad_of = np.empty(N, np.int64)
    for t in range(2):
        selt = chunk_of_local == t
        quad_of[selt] = (local_of[selt] - CHUNK_POS[t][0]) // RQ
    r_of = (local_of - np.array([CHUNK_POS[t][0] for t in range(2)])[chunk_of_local]) % RQ
    trow = 1 + core_of * np.array(NQUADS)[chunk_of_local] + quad_of

    for c in range(NC):
        offT = [0, 0]
        offA1 = offB1 = 0
        for k in range(NBLK):
            lo, hi = 128 * k, min(128 * (k + 1), SHARD)
            gR = [[np.zeros((int(SR[t][r][k]), 128), np.int32) for r in range(RQ)]
                  for t in range(2)]
            gridA1 = np.zeros((int(SA1[k]), 128), np.int32)
            gridB1 = np.full((int(SB1[k]), 128), B_PAD_IDX, np.int32)
            for lane in range(hi - lo):
                d = node_at[SHARD * c + lo + lane]
                edge_srcs = s_sorted[d_starts[d]:d_ends[d]]
                srcs = np.concatenate([edge_srcs, [d]])
                fR = [[0] * RQ for _ in range(2)]
                for s in srcs:
                    t = int(chunk_of_local[s])
                    r = int(r_of[s])
                    gR[t][r][fR[t][r], lane] = trow[s]
                    fR[t][r] += 1
                aps = pos_of[srcs]
                acls = np.concatenate([CLS_OF_CORE[core_of[edge_srcs]],
                                       [CLS_OF_CORE[c]]])
                flex1 = aps[acls == 1]
                x1 = int(xflex1[d])
                pa1 = np.concatenate([aps[acls == 0], flex1[:x1]]) + 1
                pb1 = np.concatenate([aps[acls == 2], flex1[x1:]]) + 1 - BASE_B
                gridA1[: len(pa1), lane] = pa1
                gridB1[: len(pb1), lane] = pb1
            for t in range(2):
                bg = np.concatenate(gR[t], axis=0)
                streamsT[t][c, offT[t]:offT[t] + bg.size] = bg.ravel()
                offT[t] += bg.size
            streamsA1[c, offA1:offA1 + gridA1.size] = gridA1.ravel()
            streamsB1[c, offB1:offB1 + gridB1.size] = gridB1.ravel()
            offA1 += gridA1.size
            offB1 += gridB1.size

    # ---- pooling tables
    SP = np.zeros((NC, GBLK), np.int64)
    members = {}
    nodes_of_core = [node_at[SHARD * c:SHARD * (c + 1)] for c in range(NC)]
    for c in range(NC):
        g_of_local = batch[nodes_of_core[c]]
        for gb in range(GBLK):
            cnts = np.bincount(
                g_of_local[(g_of_local >= GPER * gb) & (g_of_local < GPER * (gb + 1))] - GPER * gb,
                minlength=GPER)
            SP[c, gb] = max(cnts.max(), 1)
        members[c] = g_of_local
    SPu = SP.max(axis=0)
    lenP = int(SPu.sum()) * 128
    streamsP = np.zeros((NC, lenP), np.int32)
    for c in range(NC):
        g_of_local = members[c]
        off = 0
        for gb in range(GBLK):
            grid = np.zeros((int(SPu[gb]), 128), np.int32)
            for gl in range(GPER):
                locs = np.where(g_of_local == GPER * gb + gl)[0] + 1
                grid[: len(locs), gl] = locs
            streamsP[c, off:off + grid.size] = grid.ravel()
            off += grid.size

    cnts = np.bincount(batch, minlength=G).astype(np.float64)
    inv_cnt = (1.0 / np.maximum(cnts, 1.0)).astype(np.float32)

    return dict(
        core_of=core_of, pos_of=pos_of, node_at=node_at,
        dinv=dinv.astype(np.float32),
        SR=SR, SA1=SA1, SB1=SB1, SPu=SPu,
        streamsT=streamsT, streamsA1=streamsA1, streamsB1=streamsB1,
        streamsP=streamsP, inv_cnt=inv_cnt,
    )


# ==== host IO staging ====

def make_in_maps(inputs, P):
    node_at = P["node_at"]
    dinv = P["dinv"]
    x = np.asarray(inputs["x"], np.float32)
    xd = x * dinv[:, None]
    xp = xd[node_at]
    xdp = np.zeros((N + 2, 64), np.float32)
    xdp[1:N + 1, :37] = xp

    dinv_pos = dinv[node_at].astype(np.float32)

    def rep(v, width):
        return np.tile(np.asarray(v, np.float32)[None, :], (128, 1))

    common = dict(
        xdp=xdp,
        W1e=np.vstack([np.asarray(inputs["W1"], np.float32),
                       np.asarray(inputs["b1"], np.float32)[None, :]]),
        W2e=np.vstack([np.asarray(inputs["W2"], np.float32),
                       np.asarray(inputs["b2"], np.float32)[None, :]]),
        W3=np.asarray(inputs["W3"], np.float32),
        Wf1=np.asarray(inputs["Wf1"], np.float32),
        Wf2=np.asarray(inputs["Wf2"], np.float32),
        b1r=rep(inputs["b1"], 64), b2r=rep(inputs["b2"], 128),
        b3r=rep(inputs["b3"], 64), bf1r=rep(inputs["bf1"], 128),
        bf2r=rep(inputs["bf2"], 64), gammar=rep(inputs["gamma"], 64),
        betar=rep(inputs["beta"], 64),
    )

    in_maps = []
    for c in range(NC):
        dvo = np.zeros((128, NBLK), np.float32)
        own = dinv_pos[SHARD * c:SHARD * (c + 1)]
        for k in range(NBLK):
            rows = min(128, SHARD - 128 * k)
            dvo[:rows, k] = own[128 * k:128 * k + rows]
        invc = np.zeros((128, 1), np.float32)
        invc[:GPER, 0] = P["inv_cnt"][GPER * c:GPER * (c + 1)]
        m = dict(common)
        m["dinv_own"] = dvo
        m["invc_own"] = invc
        m["idxT0"] = wrap16(P["streamsT"][0][c])
        m["idxT1"] = wrap16(P["streamsT"][1][c])
        m["idxA1"] = wrap16(P["streamsA1"][c])
        m["idxB1"] = wrap16(P["streamsB1"][c])
        m["idxP"] = wrap16(P["streamsP"][c])
        in_maps.append(m)
    return in_maps


# ==== bass kernel ====
from contextlib import ExitStack

import concourse.bass as bass
import concourse.bacc as bacc
import concourse.mybir as mybir
from concourse.tile import TileContext
from concourse.masks import make_identity

F32 = mybir.dt.float32
F16 = mybir.dt.float16
F8 = mybir.dt.float8e4
I16 = mybir.dt.int16
AX = mybir.AxisListType
AF = mybir.ActivationFunctionType
OP = mybir.AluOpType

D_IN, D1, D2, D3 = 37, 64, 128, 64
GW = 8
SUPER_COLS = 64


def make_supers(SAu, SBu, flush_blocks=()):
    """Group consecutive blocks with combined cols <= SUPER_COLS; force a
    group boundary at each block in flush_blocks."""
    supers = []
    k0 = 0
    na = nb = 0
    for k in range(NBLK):
        if k > k0 and (na + SAu[k] > SUPER_COLS or nb + SBu[k] > SUPER_COLS
                       or k - k0 >= 16 or k in flush_blocks):
            supers.append((k0, k))
            k0, na, nb = k, 0, 0
        na += SAu[k]
        nb += SBu[k]
    supers.append((k0, NBLK))
    if supers[-1][1] - supers[-1][0] > 1:
        k0, k1 = supers[-1]
        supers[-1] = (k0, k1 - 1)
        supers.append((k1 - 1, k1))
    return supers


def build(S, debug=False):
    SR = [[list(map(int, S["SR"][t][r])) for r in range(RQ)] for t in range(2)]
    SA1 = list(map(int, S["SA1"]))
    SB1 = list(map(int, S["SB1"]))
    SPu = list(map(int, S["SPu"]))
    ST = [[sum(SR[t][r][k] for r in range(RQ)) for k in range(NBLK)]
          for t in range(2)]
    lenT = [128 * sum(ST[t]) for t in range(2)]
    lenA1, lenB1, lenP = 128 * sum(SA1), 128 * sum(SB1), 128 * sum(SPu)
    supers1 = make_supers(SA1, SB1, flush_blocks=(CHUNK_BLK[1][0],))
    supersT = [make_supers(ST[t], ST[t], flush_blocks=(CHUNK_BLK[1][0],))
               for t in range(2)]

    nc = bacc.Bacc()

    # ---------------- IO ----------------
    xdp = nc.dram_tensor("xdp", [N + 2, 64], F32, kind="ExternalInput")
    dinv_own = nc.dram_tensor("dinv_own", [128, NBLK], F32, kind="ExternalInput")
    invc_own = nc.dram_tensor("invc_own", [128, 1], F32, kind="ExternalInput")
    idxT0 = nc.dram_tensor("idxT0", [128, lenT[0] // 16], I16, kind="ExternalInput")
    idxT1 = nc.dram_tensor("idxT1", [128, lenT[1] // 16], I16, kind="ExternalInput")
    idxA1 = nc.dram_tensor("idxA1", [128, lenA1 // 16], I16, kind="ExternalInput")
    idxB1 = nc.dram_tensor("idxB1", [128, lenB1 // 16], I16, kind="ExternalInput")
    idxP = nc.dram_tensor("idxP", [128, lenP // 16], I16, kind="ExternalInput")
    W1e = nc.dram_tensor("W1e", [D_IN + 1, D1], F32, kind="ExternalInput")
    W2e = nc.dram_tensor("W2e", [D1 + 1, D2], F32, kind="ExternalInput")
    W3 = nc.dram_tensor("W3", [D2, D3], F32, kind="ExternalInput")
    Wf1 = nc.dram_tensor("Wf1", [64, 128], F32, kind="ExternalInput")
    Wf2 = nc.dram_tensor("Wf2", [128, 64], F32, kind="ExternalInput")
    b1r = nc.dram_tensor("b1r", [128, D1], F32, kind="ExternalInput")
    b2r = nc.dram_tensor("b2r", [128, D2], F32, kind="ExternalInput")
    b3r = nc.dram_tensor("b3r", [128, D3], F32, kind="ExternalInput")
    bf1r = nc.dram_tensor("bf1r", [128, 128], F32, kind="ExternalInput")
    bf2r = nc.dram_tensor("bf2r", [128, 64], F32, kind="ExternalInput")
    gammar = nc.dram_tensor("gammar", [128, 64], F32, kind="ExternalInput")
    betar = nc.dram_tensor("betar", [128, 64], F32, kind="ExternalInput")

    out_slice = nc.dram_tensor("out_slice", [GPER, 64], F32, kind="ExternalOutput")

    # internal DRAM: fp8 quad tables per chunk
    h1own = [nc.dram_tensor(f"h1own{t}", [1 + NQUADS[t], 256], F8)
             for t in range(2)]
    T1 = [nc.dram_tensor(f"T1_{t}", [1 + NC * NQUADS[t], 256], F8,
                         addr_space="Shared") for t in range(2)]
    hW3own = [nc.dram_tensor(f"hW3own{t}", [1 + NQUADS[t], 256], F8)
              for t in range(2)]
    T3 = [nc.dram_tensor(f"T3_{t}", [1 + NC * NQUADS[t], 256], F8,
                         addr_space="Shared") for t in range(2)]
    h3ot = nc.dram_tensor("h3ot", [1 + SHARD, D3], F32)
    poolin = nc.dram_tensor("poolin", [G, 64], F32)
    rs_out = nc.dram_tensor("rs_out", [GPER, 64], F32)

    dbg = {}
    if debug:
        for name, shape, dt in [
                ("d_h1own0", [1 + NQUADS[0], 256], F8),
                ("d_h1own1", [1 + NQUADS[1], 256], F8),
                ("d_T1_0", [1 + NC * NQUADS[0], 256], F8),
                ("d_hW3own0", [1 + NQUADS[0], 256], F8),
                ("d_h3", [SHARD, D3], F32),
                ("d_poolin", [G, 64], F32), ("d_rs", [GPER, 64], F32)]:
            dbg[name] = nc.dram_tensor(name, shape, dt, kind="ExternalOutput")

    rg = [list(range(NC))]

    with TileContext(nc) as tc, ExitStack() as ctx:
        const = ctx.enter_context(tc.tile_pool(name="const", bufs=1))
        idxp = ctx.enter_context(tc.tile_pool(name="idxp", bufs=1))

        ident = const.tile([128, 128], F32, tag="ident")
        make_identity(nc, ident[:])
        zt = const.tile([128, 128], F32, tag="zt")
        nc.vector.memset(zt[:], 0.0)
        zth = const.tile([128, 128], F16, tag="zth")
        nc.vector.memset(zth[:], 0.0)
        zf8 = const.tile([128, 256], F8, tag="zf8")
        nc.vector.memset(zf8[:], 0.0)
        epss = const.tile([128, 1], F32, tag="epss")
        nc.vector.memset(epss[:], 1e-5)

        def csb(t, p0, p1, tag, dt=F32):
            tl = const.tile([p0, p1], dt, tag=tag)
            nc.sync.dma_start(out=tl[:], in_=t[:])
            return tl

        W1s = csb(W1e, D_IN + 1, D1, "W1s")
        W2s = csb(W2e, D1 + 1, D2, "W2s")
        W3s = csb(W3, D2, D3, "W3s")
        Wf1s = csb(Wf1, 64, 128, "Wf1s")
        Wf2s = csb(Wf2, 128, 64, "Wf2s")
        b1s = csb(b1r, 128, D1, "b1s")
        b2s = csb(b2r, 128, D2, "b2s")
        b3s = csb(b3r, 128, D3, "b3s")
        bf1s = csb(bf1r, 128, 128, "bf1s")
        bf2s = csb(bf2r, 128, 64, "bf2s")
        gams = csb(gammar, 128, 64, "gams")
        bets = csb(betar, 128, 64, "bets")
        dvo = csb(dinv_own, 128, NBLK, "dvo")
        dvo2 = const.tile([128, NBLK], F32, tag="dvo2")
        nc.vector.tensor_tensor(out=dvo2[:], in0=dvo[:], in1=dvo[:], op=OP.mult)
        invdvo = const.tile([128, NBLK], F32, tag="invdvo")
        nc.vector.reciprocal(invdvo[:], dvo[:])
        ics = csb(invc_own, 128, 1, "ics")

        idxT_s = []
        for t in range(2):
            tl = idxp.tile([128, lenT[t] // 16], I16, tag=f"it{t}")
            nc.sync.dma_start(out=tl[:], in_=[idxT0, idxT1][t][:])
            idxT_s.append(tl)
        idxA1_s = idxp.tile([128, lenA1 // 16], I16, tag="ia1")
        nc.sync.dma_start(out=idxA1_s[:], in_=idxA1[:])
        idxB1_s = idxp.tile([128, lenB1 // 16], I16, tag="ib1")
        nc.sync.dma_start(out=idxB1_s[:], in_=idxB1[:])
        idxP_s = idxp.tile([128, lenP // 16], I16, tag="ip")
        nc.sync.dma_start(out=idxP_s[:], in_=idxP[:])

        # guard rows
        for t in range(2):
            nc.scalar.dma_start(out=h1own[t][0:1, :], in_=zf8[0:1, :])
            nc.scalar.dma_start(out=T1[t][0:1, :], in_=zf8[0:1, :])
            nc.scalar.dma_start(out=hW3own[t][0:1, :], in_=zf8[0:1, :])
            nc.scalar.dma_start(out=T3[t][0:1, :], in_=zf8[0:1, :])
        nc.scalar.dma_start(out=h3ot[0:1, :], in_=zt[0:1, 0:D3])

        # ---- fp8 quad-table store sink ---------------------------------
        def make_pair_store_sink(dsts, pname):
            """Collects per-block [128, 64] tiles (stored as f8); flushes
            groups of up to 8 blocks into the chunk quad tables. Block
            groups never straddle the chunk boundary."""
            state = {"tile": None, "k0": None, "n": 0}
            pool = ctx.enter_context(tc.tile_pool(name=pname, bufs=2))

            def flush():
                tl, k0_, n_ = state["tile"], state["k0"], state["n"]
                if tl is None or n_ == 0:
                    return
                t = 0 if k0_ < CHUNK_BLK[1][0] else 1
                dst = dsts[t]
                qr0 = 32 * (k0_ - CHUNK_BLK[t][0])
                rows_total = min(SHARD, 128 * (k0_ + n_)) - 128 * k0_
                full = n_ - 1 if rows_total < 128 * n_ else n_
                if full > 0:
                    nc.scalar.dma_start(
                        out=dst[1 + qr0:1 + qr0 + 32 * full, :].rearrange(
                            "(j q) (r f) -> (q r) j f", q=32, r=4),
                        in_=tl[:, :full * 64].rearrange("p (j f) -> p j f", j=full))
                if full < n_:
                    rl = rows_total - 128 * full          # 106 for last block
                    nq = rl // 4                          # 26 full quads
                    r0 = 1 + qr0 + 32 * full
                    nc.scalar.dma_start(
                        out=dst[r0:r0 + nq, :].rearrange(
                            "q (r f) -> (q r) f", r=4),
                        in_=tl[:4 * nq, full * 64:(full + 1) * 64])
                    rem = rl - 4 * nq                     # 2 leftover rows
                    if rem:
                        nc.scalar.dma_start(
                            out=dst[r0 + nq:r0 + nq + 1, 0:rem * 64].rearrange(
                                "q (r f) -> (q r) f", r=rem),
                            in_=tl[4 * nq:rl, full * 64:(full + 1) * 64])
                state["tile"], state["k0"], state["n"] = None, None, 0

            def sink_store(k, rows, res, func=AF.Copy, scale=1.0):
                if state["tile"] is None:
                    state["tile"] = pool.tile([128, 8 * 64], F8, tag="stg",
                                              name=pname + "_stg")
                    state["k0"], state["n"] = k, 0
                j = state["n"]
                nc.scalar.activation(
                    state["tile"][:, j * 64:(j + 1) * 64], res[:], func,
                    scale=scale)
                state["n"] += 1
                if state["n"] == 8 or k == CHUNK_BLK[0][1] - 1:
                    flush()

            return sink_store, flush

        # ---- plain-row store sink (h3ot) -------------------------------
        def make_store_sink(dst, fout, dt, pname, row_off=0):
            state = {"tile": None, "k0": None, "n": 0}
            pool = ctx.enter_context(tc.tile_pool(name=pname, bufs=2))

            def flush():
                t, k0_, n_ = state["tile"], state["k0"], state["n"]
                if t is None or n_ == 0:
                    return
                row0 = row_off + 128 * k0_
                nrows = min(SHARD, 128 * (k0_ + n_)) - 128 * k0_
                full = n_ - 1 if nrows < 128 * n_ else n_
                if full > 0:
                    nc.scalar.dma_start(
                        out=dst[row0:row0 + 128 * full, :].rearrange(
                            "(j p) f -> p j f", p=128),
                        in_=t[:, :full * fout].rearrange("p (j f) -> p j f", j=full))
                if full < n_:
                    rl = nrows - 128 * full
                    nc.scalar.dma_start(
                        out=dst[row0 + 128 * full:row0 + nrows, :],
                        in_=t[:rl, full * fout:(full + 1) * fout])
                state["tile"], state["k0"], state["n"] = None, None, 0

            def sink_store(k, rows, res, func=AF.Copy, scale=1.0):
                if state["tile"] is None:
                    state["tile"] = pool.tile([128, 8 * fout], dt, tag="stg",
                                              name=pname + "_stg")
                    state["k0"], state["n"] = k, 0
                j = state["n"]
                nc.scalar.activation(
                    state["tile"][:, j * fout:(j + 1) * fout], res, func,
                    scale=scale)
                state["n"] += 1
                if state["n"] == 8:
                    flush()

            return sink_store, flush

        # ---- L1 gather driver (A1/B1 f32 windows, like v2 main pass) ----
        def fold_plain(gt, elem, c0, s, upto=1):
            while s > upto:
                h = min(s // 2, s - upto)
                nc.vector.tensor_tensor(
                    out=gt[:, c0 * elem:(c0 + h) * elem],
                    in0=gt[:, c0 * elem:(c0 + h) * elem],
                    in1=gt[:, (c0 + s - h) * elem:(c0 + s) * elem],
                    op=OP.add)
                s -= h

        def l1_layer(sink, mid_hook=None, park_ms=0.0):
            elem = 64
            with tc.tile_pool(name="gta", bufs=3) as gta_pool, \
                 tc.tile_pool(name="gtb", bufs=3) as gtb_pool, \
                 tc.tile_pool(name="hacc", bufs=4) as hacc_pool, \
                 ExitStack() as lctx:
                offA = offB = 0
                for (k0, k1) in supers1:
                    if k0 == CHUNK_BLK[1][0] and mid_hook is not None:
                        mid_hook()
                        if park_ms > 0:
                            lctx.enter_context(tc.tile_wait_until(park_ms))
                    nA = sum(SA1[k0:k1])
                    nB = sum(SB1[k0:k1])
                    gtA = gta_pool.tile([128, SUPER_COLS * elem], F32, tag="gtA",
                                        name="gtA")
                    done = 0
                    while done < nA:
                        w = min(GW, nA - done)
                        nc.gpsimd.dma_gather(
                            out_ap=gtA[:, done * elem:(done + w) * elem].rearrange(
                                "p (c f) -> p c f", c=w),
                            in_ap=xdp[:],
                            idxs_ap=idxA1_s[:, 8 * (offA + done):8 * (offA + done + w)],
                            num_idxs=128 * w, num_idxs_reg=128 * w,
                            elem_size=elem)
                        done += w
                    gtB = gtb_pool.tile([128, SUPER_COLS * elem], F32, tag="gtB",
                                        name="gtB")
                    done = 0
                    while done < nB:
                        w = min(GW, nB - done)
                        nc.gpsimd.dma_gather(
                            out_ap=gtB[:, done * elem:(done + w) * elem].rearrange(
                                "p (c f) -> p c f", c=w),
                            in_ap=xdp[BASE_B:, :],
                            idxs_ap=idxB1_s[:, 8 * (offB + done):8 * (offB + done + w)],
                            num_idxs=128 * w, num_idxs_reg=128 * w,
                            elem_size=elem)
                        done += w
                    cA = cB = 0
                    for k in range(k0, k1):
                        sa, sb_ = SA1[k], SB1[k]
                        rows = min(128, SHARD - 128 * k)
                        acc = hacc_pool.tile([128, elem], F32, tag="acc")
                        fold_plain(gtA, elem, cA, sa, upto=2)
                        if sa >= 2:
                            nc.vector.tensor_tensor(
                                out=acc[:],
                                in0=gtA[:, cA * elem:(cA + 1) * elem],
                                in1=gtA[:, (cA + 1) * elem:(cA + 2) * elem],
                                op=OP.add)
                        else:
                            nc.vector.tensor_copy(
                                out=acc[:], in_=gtA[:, cA * elem:(cA + 1) * elem])
                        fold_plain(gtB, elem, cB, sb_, upto=1)
                        nc.vector.tensor_tensor(
                            out=acc[:], in0=acc[:],
                            in1=gtB[:, cB * elem:(cB + 1) * elem], op=OP.add)
                        cA += sa
                        cB += sb_
                        sink(k, rows, acc)
                    offA += nA
                    offB += nB

        # ---- fp8 quad-table gather layer (two passes over chunk tables) ----
        def pair_layer(tabs, idx_tiles, sink, mid_hook=None, park_ms=0.0):
            """tabs: [T_0, T_1] dram quad tables; sink(k, rows, acc_f32_64).
            Fold: sequential accumulation into an f32 target so fp8 never
            holds a partial sum."""
            with tc.tile_pool(name="accv", bufs=1) as accv_pool, \
                 tc.tile_pool(name="ftmp", bufs=4) as ftmp_pool, \
                 tc.tile_pool(name="hacc", bufs=4) as hacc_pool:
                accV = accv_pool.tile([128, NBLK * 64], F32, tag="accV")
                for t in range(2):
                    with tc.tile_pool(name=f"gt{t}", bufs=3) as gt_pool, \
                         ExitStack() as lctx:
                        off = 0
                        for (k0, k1) in supersT[t]:
                            if (t == 1 and k0 == CHUNK_BLK[1][0]
                                    and mid_hook is not None):
                                mid_hook()
                                if park_ms > 0:
                                    lctx.enter_context(
                                        tc.tile_wait_until(park_ms))
                            ncols = sum(ST[t][k0:k1])
                            gt = gt_pool.tile([128, SUPER_COLS * 256], F8,
                                              tag="gt", name=f"gt{t}")
                            done = 0
                            while done < ncols:
                                w = min(GW, ncols - done)
                                nc.gpsimd.dma_gather(
                                    out_ap=gt[:, done * 256:(done + w) * 256].rearrange(
                                        "p (c f) -> p c f", c=w),
                                    in_ap=tabs[t][:],
                                    idxs_ap=idx_tiles[t][:, 8 * (off + done):8 * (off + done + w)],
                                    num_idxs=128 * w, num_idxs_reg=128 * w,
                                    elem_size=256)
                                done += w
                            c0 = 0
                            g3 = gt[:].rearrange("p (c f) -> p c f", f=256)
                            for k in range(k0, k1):
                                rows = min(128, SHARD - 128 * k)
                                cols = []
                                cc = c0
                                for r in range(RQ):
                                    for i in range(SR[t][r][k]):
                                        cols.append((cc + i, r * 64))
                                    cc += SR[t][r][k]
                                if t == 0:
                                    target = accV[:, k * 64:(k + 1) * 64]
                                else:
                                    acc = hacc_pool.tile([128, D1 + 1], F32,
                                                         tag="acc")
                                    target = acc[:, 0:64]
                                # pairwise f8+f8 -> f32 temp (independent ops),
                                # then wide f32 tree fold: short DVE chain.
                                npair = len(cols) // 2
                                odd = len(cols) % 2
                                tmp = ftmp_pool.tile([128, max(npair, 2) * 64],
                                                     F32, tag="ftmp")
                                for j in range(npair):
                                    (iA, oA), (iB, oB) = cols[2 * j], cols[2 * j + 1]
                                    nc.vector.tensor_tensor(
                                        out=tmp[:, j * 64:(j + 1) * 64],
                                        in0=g3[:, iA:iA + 1, oA:oA + 64],
                                        in1=g3[:, iB:iB + 1, oB:oB + 64],
                                        op=OP.add)
                                fold_plain(tmp, 64, 0, npair, upto=2)
                                nc.vector.tensor_tensor(
                                    out=target, in0=tmp[:, 0:64],
                                    in1=tmp[:, 64:128], op=OP.add)
                                if odd:
                                    (ci, o) = cols[-1]
                                    nc.vector.tensor_tensor(
                                        out=target, in0=target,
                                        in1=g3[:, ci:ci + 1, o:o + 64], op=OP.add)
                                if t == 1:
                                    nc.vector.tensor_tensor(
                                        out=target, in0=target,
                                        in1=accV[:, k * 64:(k + 1) * 64], op=OP.add)
                                    sink(k, rows, acc)
                                c0 = cc
                            off += ncols

        # ================= L1 =================
        store1, flush1 = make_pair_store_sink(h1own, "st1")
        with tc.tile_pool(name="tp1", bufs=2, space="PSUM") as tp1, \
             tc.tile_pool(name="mm0", bufs=2, space="PSUM") as mm0, \
             tc.tile_pool(name="hT1", bufs=4) as hT1p, \
             tc.tile_pool(name="h1b", bufs=4) as h1bp:
            def sink1(k, rows, agg):
                # psb = agg@W1 + (1/dvo)*b1 via the extra lhsT row; table =
                # dvo*relu(dvo*psb) = relu(dvo^2 * psb), fused into the f8
                # store copy.
                nc.vector.tensor_copy(out=agg[:, D_IN:D_IN + 1],
                                      in_=invdvo[:, k:k + 1])
                tp0 = tp1.tile([64, 128], F32, tag="tp0", name="tp0")
                nc.tensor.transpose(out=tp0[:], in_=agg[:], identity=ident[:])
                aT = hT1p.tile([64, 128], F32, tag="aT", name="aT")
                nc.scalar.activation(aT[:], tp0[:], AF.Copy)
                ps0 = mm0.tile([128, D1], F32, tag="ps0", name="ps0")
                nc.tensor.matmul(out=ps0[:], lhsT=aT[:D_IN + 1, :], rhs=W1s[:],
                                 start=True, stop=True)
                store1(k, rows, ps0, func=AF.Relu, scale=dvo2[:, k:k + 1])

            def mid1():
                flush1()
                with tc.high_priority():
                    nc.gpsimd.collective_compute(
                        "AllGather", OP.bypass, replica_groups=rg,
                        ins=[h1own[0][1:, :]], outs=[T1[0][1:, :]])
            l1_layer(sink1, mid_hook=mid1, park_ms=0.068)
            flush1()
        with tc.high_priority():
            nc.gpsimd.collective_compute(
                "AllGather", OP.bypass, replica_groups=rg,
                ins=[h1own[1][1:, :]], outs=[T1[1][1:, :]])

        # ================= L2 =================
        store2, flush2 = make_pair_store_sink(hW3own, "st2")
        with tc.tile_pool(name="tp2", bufs=2, space="PSUM") as tp2, \
             tc.tile_pool(name="mm2", bufs=2, space="PSUM") as mm2, \
             tc.tile_pool(name="mm3", bufs=2, space="PSUM") as mm3, \
             tc.tile_pool(name="hT2", bufs=4) as hT2p, \
             tc.tile_pool(name="h2b", bufs=4) as h2bp:
            def sink2(k, rows, acc):
                # psb = acc@W2 + (1/dvo)*b2 (extra lhsT row); h2' =
                # dvo*relu(dvo*psb) = relu(dvo^2*psb); store f8(h2' @ W3).
                nc.vector.tensor_copy(out=acc[:, D1:D1 + 1],
                                      in_=invdvo[:, k:k + 1])
                tp0 = tp2.tile([D1 + 1, 128], F32, tag="tp0", name="tp0")
                nc.tensor.transpose(out=tp0[:], in_=acc[:], identity=ident[:])
                aT = hT2p.tile([D1 + 1, 128], F32, tag="aT", name="aT")
                nc.scalar.activation(aT[:], tp0[:], AF.Copy)
                ps = mm2.tile([128, D2], F32, tag="ps", name="ps")
                nc.tensor.matmul(out=ps[:], lhsT=aT[:], rhs=W2s[:],
                                 start=True, stop=True)
                h = h2bp.tile([128, D2], F32, tag="h2t", name="h2t")
                nc.scalar.activation(h[:], ps[:], AF.Relu,
                                     scale=dvo2[:, k:k + 1])
                tp = tp2.tile([D2, 128], F32, tag="tp", name="tp")
                nc.tensor.transpose(out=tp[:], in_=h[:], identity=ident[:])
                hT = hT2p.tile([D2, 128], F32, tag="hT", name="hT")
                nc.vector.tensor_copy(out=hT[:], in_=tp[:])
                ps3 = mm3.tile([128, D3], F32, tag="ps3", name="ps3")
                nc.tensor.matmul(out=ps3[:], lhsT=hT[:], rhs=W3s[:],
                                 start=True, stop=True)
                store2(k, rows, ps3)

            def mid2():
                flush2()
                with tc.high_priority():
                    nc.gpsimd.collective_compute(
                        "AllGather", OP.bypass, replica_groups=rg,
                        ins=[hW3own[0][1:, :]], outs=[T3[0][1:, :]])
            pair_layer(T1, idxT_s, sink2, mid_hook=mid2, park_ms=0.27)
            flush2()
        with tc.high_priority():
            nc.gpsimd.collective_compute(
                "AllGather", OP.bypass, replica_groups=rg,
                ins=[hW3own[1][1:, :]], outs=[T3[1][1:, :]])

        # ================= L3 =================
        storeh3, flushh3 = make_store_sink(h3ot, D3, F32, "sth3", row_off=1)

        def sink3(k, rows, acc):
            a64 = acc[:, 0:D3]
            nc.vector.tensor_tensor(
                out=a64, in0=a64,
                in1=dvo[:, k:k + 1].to_broadcast([128, D3]), op=OP.mult)
            nc.vector.tensor_tensor(out=a64, in0=a64, in1=b3s[:], op=OP.add)
            storeh3(k, rows, a64, func=AF.Relu)
        pair_layer(T3, idxT_s, sink3)
        flushh3()

        # ================= pooling =================
        with tc.tile_pool(name="gp", bufs=1) as gp, \
             tc.tile_pool(name="pstg", bufs=1) as pstg:
            stg = pstg.tile([128, GBLK * 64], F32, tag="pstg")
            spT = sum(SPu)
            gt = gp.tile([128, spT * 64], F32, tag="gtp")
            done = 0
            while done < spT:
                w = min(GW, spT - done)
                nc.gpsimd.dma_gather(
                    out_ap=gt[:, done * 64:(done + w) * 64].rearrange(
                        "p (c f) -> p c f", c=w),
                    in_ap=h3ot[:],
                    idxs_ap=idxP_s[:, 8 * done:8 * (done + w)],
                    num_idxs=128 * w, num_idxs_reg=128 * w, elem_size=64)
                done += w
            offP = 0
            for gb in range(GBLK):
                sp = SPu[gb]
                fold_plain(gt, 64, offP, sp, upto=2)
                nc.vector.tensor_tensor(
                    out=stg[:, 64 * gb:64 * (gb + 1)],
                    in0=gt[:, offP * 64:(offP + 1) * 64],
                    in1=gt[:, (offP + 1) * 64:(offP + 2) * 64], op=OP.add)
                offP += sp
            nc.scalar.dma_start(
                out=poolin[:, :].rearrange("(j p) f -> p j f", p=GPER),
                in_=stg[:GPER, :].rearrange("p (j f) -> p j f", j=GBLK))

        nc.gpsimd.collective_compute(
            "ReduceScatter", OP.add, replica_groups=rg,
            ins=[poolin[:]], outs=[rs_out[:]])

        # ================= MLP + LayerNorm =================
        with tc.tile_pool(name="mlp", bufs=1) as mlp, \
             tc.tile_pool(name="mps", bufs=2, space="PSUM") as mps:
            gtl = mlp.tile([128, 64], F32, tag="g0")
            nc.vector.memset(gtl[:], 0.0)
            nc.sync.dma_start(out=gtl[:GPER, :], in_=rs_out[:])
            nc.vector.tensor_tensor(
                out=gtl[:], in0=gtl[:],
                in1=ics[:].to_broadcast([128, 64]), op=OP.mult)
            tp = mps.tile([64, 128], F32, tag="t1")
            nc.tensor.transpose(out=tp[:], in_=gtl[:], identity=ident[:])
            gT = mlp.tile([64, 128], F32, tag="gT")
            nc.vector.tensor_copy(out=gT[:], in_=tp[:])
            p1 = mps.tile([128, 128], F32, tag="p1")
            nc.tensor.matmul(out=p1[:], lhsT=gT[:], rhs=Wf1s[:],
                             start=True, stop=True)
            g1t = mlp.tile([128, 128], F32, tag="g1t")
            nc.vector.tensor_tensor(out=g1t[:], in0=p1[:], in1=bf1s[:], op=OP.add)
            nc.scalar.activation(g1t[:], g1t[:], AF.Relu)
            tp2_ = mps.tile([128, 128], F32, tag="t2")
            nc.tensor.transpose(out=tp2_[:], in_=g1t[:], identity=ident[:])
            g1T = mlp.tile([128, 128], F32, tag="g1T")
            nc.vector.tensor_copy(out=g1T[:], in_=tp2_[:])
            p2 = mps.tile([128, 64], F32, tag="p2")
            nc.tensor.matmul(out=p2[:], lhsT=g1T[:], rhs=Wf2s[:],
                             start=True, stop=True)
            g2t = mlp.tile([128, 64], F32, tag="g2t")
            nc.vector.tensor_tensor(out=g2t[:], in0=p2[:], in1=bf2s[:], op=OP.add)
            nc.scalar.activation(g2t[:], g2t[:], AF.Relu)
            mu = mlp.tile([128, 1], F32, tag="mu")
            nc.vector.reduce_sum(mu[:], g2t[:], axis=AX.X)
            nc.vector.tensor_scalar_mul(mu[:], in0=mu[:], scalar1=1.0 / 64)
            xm = mlp.tile([128, 64], F32, tag="xm")
            nc.vector.tensor_tensor(out=xm[:], in0=g2t[:],
                                    in1=mu[:].to_broadcast([128, 64]),
                                    op=OP.subtract)
            sq = mlp.tile([128, 64], F32, tag="sq")
            nc.vector.tensor_tensor(out=sq[:], in0=xm[:], in1=xm[:], op=OP.mult)
            var = mlp.tile([128, 1], F32, tag="var")
            nc.vector.reduce_sum(var[:], sq[:], axis=AX.X)
            rstd = mlp.tile([128, 1], F32, tag="rstd")
            nc.vector.tensor_scalar_mul(var[:], in0=var[:], scalar1=1.0 / 64)
            nc.vector.tensor_tensor(out=var[:], in0=var[:], in1=epss[:],
                                    op=OP.add)
            nc.scalar.activation(rstd[:], var[:], AF.Sqrt)
            nc.vector.reciprocal(rstd[:], rstd[:])
            nc.vector.tensor_tensor(out=xm[:], in0=xm[:],
                                    in1=rstd[:].to_broadcast([128, 64]),
                                    op=OP.mult)
            nc.vector.tensor_tensor(out=xm[:], in0=xm[:], in1=gams[:], op=OP.mult)
            nc.vector.tensor_tensor(out=xm[:], in0=xm[:], in1=bets[:], op=OP.add)
            nc.sync.dma_start(out=out_slice[:, :], in_=xm[:GPER, :])

        if debug:
            with tc.tile_pool(name="dbg", bufs=2) as dp:
                def dump(srct, dstt, nrows, width, dt=F32):
                    for c in range((nrows + 127) // 128):
                        rows = min(128, nrows - 128 * c)
                        tl = dp.tile([128, width], dt, tag="dt")
                        nc.sync.dma_start(out=tl[:rows, :],
                                          in_=srct[128 * c:128 * c + rows, :])
                        nc.sync.dma_start(out=dstt[128 * c:128 * c + rows, :],
                                          in_=tl[:rows, :])
                dump(h1own[0], dbg["d_h1own0"], 1 + NPAIRS[0], 128, F16)
                dump(h1own[1], dbg["d_h1own1"], 1 + NPAIRS[1], 128, F16)
                dump(T1[0], dbg["d_T1_0"], 1 + NC * NPAIRS[0], 128, F16)
                dump(hW3own[0], dbg["d_hW3own0"], 1 + NPAIRS[0], 128, F16)
                dump(h3ot[1:, :], dbg["d_h3"], SHARD, D3)
                dump(poolin, dbg["d_poolin"], G, 64)
                dump(rs_out, dbg["d_rs"], GPER, 64)

    nc.compile()
    nc.finalize()
    return nc


# ==== SPMD runner (same as v2) ====
import jax
from jax.sharding import Mesh, PartitionSpec
from jax.experimental.shard_map import shard_map

from concourse import bass2jax


class SpmdRunner:
    def __init__(self, nc, n_cores=8):
        bass2jax.install_neuronx_cc_hook()
        self.nc = nc
        self.n_cores = n_cores
        partition_name = nc.partition_id_tensor.name if nc.partition_id_tensor else None
        in_names, out_names, out_avals, zero_outs = [], [], [], []
        for alloc in nc.m.functions[0].allocations:
            if not isinstance(alloc, mybir.MemoryLocationSet):
                continue
            name = alloc.memorylocations[0].name
            if alloc.kind == "ExternalInput":
                if name != partition_name:
                    in_names.append(name)
            elif alloc.kind == "ExternalOutput":
                shape = tuple(alloc.tensor_shape)
                dtype = mybir.dt.np(alloc.dtype)
                out_names.append(name)
                out_avals.append(jax.core.ShapedArray(shape, dtype))
                zero_outs.append(np.zeros(shape, dtype))
        self.in_names = list(in_names)
        self.out_names = out_names
        self.out_avals = out_avals
        self.zero_outs = zero_outs
        n_params = len(in_names)
        n_outs = len(out_avals)
        all_in_names = in_names + out_names + ([partition_name] if partition_name else [])
        self.n_params = n_params

        def _body(*args):
            operands = list(args)
            if partition_name is not None:
                operands.append(bass2jax.partition_id_tensor())
            outs = bass2jax._bass_exec_p.bind(
                *operands,
                out_avals=tuple(out_avals),
                in_names=tuple(all_in_names),
                out_names=tuple(out_names),
                lowering_input_output_aliases=(),
                sim_require_finite=True,
                sim_require_nnan=True,
                nc=nc,
            )
            return tuple(outs)

        try:
            devices = jax.devices("axon")[:n_cores]
        except RuntimeError:
            devices = jax.devices()[:n_cores]
        mesh = Mesh(np.asarray(devices), ("core",))
        in_specs = (PartitionSpec("core"),) * (n_params + n_outs)
        out_specs = (PartitionSpec("core"),) * n_outs
        self.fn = jax.jit(
            shard_map(_body, mesh=mesh, in_specs=in_specs, out_specs=out_specs,
                      check_rep=False),
            keep_unused=True,
        )

    def stage(self, in_maps):
        concat = [
            np.concatenate([np.asarray(in_maps[c][n]) for c in range(self.n_cores)], axis=0)
            for n in self.in_names
        ]
        zeros = [np.zeros((self.n_cores * z.shape[0], *z.shape[1:]), z.dtype)
                 for z in self.zero_outs]
        return concat + zeros

    def run(self, staged):
        out = self.fn(*staged)
        jax.block_until_ready(out)
        return out

    def unpack(self, out_arrs):
        return [
            {
                name: np.asarray(out_arrs[i]).reshape(
                    self.n_cores, *self.out_avals[i].shape)[c]
                for i, name in enumerate(self.out_names)
            }
            for c in range(self.n_cores)
        ]


_CACHE = {}


def kernel(**inputs):
    inputs = {k: np.asarray(v) for k, v in inputs.items()}
    P = preprocess(inputs["edge_index"], inputs["batch"])
    key = (tuple(map(tuple, P["SR"].reshape(2 * RQ, -1))),
           tuple(P["SA1"].tolist()), tuple(P["SB1"].tolist()),
           tuple(P["SPu"].tolist()))
    if key not in _CACHE:
        S = {k: P[k] for k in S_KEYS}
        nc = build(S, debug=False)
        _CACHE[key] = SpmdRunner(nc, 8)
    r = _CACHE[key]
    in_maps = make_in_maps(inputs, P)
    staged = r.stage(in_maps)
    res = r.unpack(r.run(staged))
    return np.ascontiguousarray(
        np.concatenate([res[c]["out_slice"] for c in range(NC)], axis=0),
        dtype=np.float32)
